# revision 1
# baseline (speedup 1.0000x reference)
"""Trainium2 Bass kernel for nn_MetricLoss (lifted-structure-style metric loss).

Reference computation (N=4096 rows, F=512 features, 16 label classes):
    Dsq = ||b_i||^2 + ||a_j||^2 - 2 b@a.T ;  D = sqrt(max(Dsq,0))   [N,N]
    Dexpm = exp(1 - D)
    row_negsum[i] = sum_{j: lbl_j != lbl_i} Dexpm[i,j]
    J = log(row_negsum[i] + row_negsum[j]) + D
    loss = sum_{i!=j, lbl_i==lbl_j} relu(J)^2 / (2 * num_pos)

Design (~1.5-1.9x faster than the original baseline; measured 106-134us
vs 159-204us, run-to-run spread is multi-core launch skew):
  * Rows are SORTED BY LABEL on the host (joint permutation of a, b, labels;
    the loss is permutation-invariant). Positive pairs for core c (rows
    [512c, 512c+512) of sorted b) then live in a contiguous column band of
    <= nt2 (=8 here) j-tiles, so phase 2 (Ln + hinge^2) covers 8 tiles
    instead of 32, and only the 8-tile D window is kept in SBUF.
  * Per-core COLUMN PERMUTATION: core c's at/aat/onehotj inputs present the
    global j-tiles reordered as [own 4 tiles, rest of its class window,
    pad, rest], so phase-2 tile indices are core-independent (pure SPMD),
    group 0 (own tiles, ns known locally) plus the diagonal correction run
    BEFORE the AllGather completes, and each core streams `a` from a
    different HBM region. A per-core [32, nt2] permutation matrix maps the
    gathered row_negsum into local tile order via one tiny matmul.
  * The aa[j]+bb[i] norm terms are added by DVE scalar_tensor_tensor
    (per-partition aa scalar + bb broadcast tile, fp32 exact) on the
    PSUM->SBUF hop -- no augmented matmuls in the GEMM stream (the v1
    baseline spent ~10us of PE on K=4 hi/lo aug matmuls), and PSUM tiles
    recycle faster.
  * Scalar runs sqrt and exp(1-D) in BATCHES ({0..2},{3..5},{6..7} chunks
    of 4 tiles) with explicit queue-order deps: 6 ACT table loads total
    instead of one ~1.3us reload per sqrt<->exp alternation, and each exp
    batch starts as soon as its sqrts are done. _pin_combined_act_set
    makes Exp/Ln resolve to natural_log_exp_and_others so phase-2's Ln
    needs no extra load after the exp batches.
  * Resident loads ride the sync queue behind the first at super-tile
    (v1 queued them on gpsimd BEHIND the warmup collectives -- a wallclock
    barrier that stalled early-starting cores ~13us); bt2 is split into
    per-k DMAs so the first matmul fires after ~384KB.
  * negsum-by-label matmuls (onehot [128->16]) interleave into the GEMM
    stream ~2 psum tiles behind the producing matmuls.

The GEMM runs in bf16 (b @ a.T as 128 matmuls of [128k,128j]x[128,512i]);
masked reductions are TensorE matmuls against one-hot label matrices.
"""

import re
import operator
import numpy as np
import ml_dtypes
from contextlib import ExitStack

import concourse.bass as bass
import concourse.tile as tile
from concourse import bacc, mybir
from concourse import dve_ops
from concourse.dve_spec import Spec, Src0, Src1, C0, relu, sq
from concourse.bass_utils import run_bass_kernel_spmd
from concourse.tile_rust import add_dep_helper

F32 = mybir.dt.float32
BF16 = mybir.dt.bfloat16
NPBF16 = ml_dtypes.bfloat16
AF = mybir.ActivationFunctionType
ALU = mybir.AluOpType

N = 4096          # rows (a and b)
F = 512           # features
NCORES = 8
R = N // NCORES   # rows of b per core = 512
NT = N // 128     # j-tiles of 128 partitions = 32
NP = NT // 2      # psum tiles of [128, 2, 512] = 16
NCLS = 16         # label classes


def _register_sqrelu_add():
    """Custom fused DVE op: out = relu(in0 + in1)^2, accum_out = c0 + sum(out)."""
    name = "SQRELU_ADD_ANT"
    for op in dve_ops.OPS:
        if op.name == name:
            return op
    op = dve_ops.DveOp(
        name,
        Spec(body=sq(relu(Src0 + Src1)), accum=operator.add, accum_init=C0),
        subdim=False,
        uops_sha={},
    )
    dve_ops._SUB_OPCODE_FOR_NAME[name] = (
        max(dve_ops._SUB_OPCODE_FOR_NAME.values()) + 1)
    assert dve_ops._SUB_OPCODE_FOR_NAME[name] < 0x20
    for ver in ("v3", "v4"):
        try:
            op.compile(ver)
        except ValueError as e:
            m = re.search(r"\(%s: ([0-9a-f]+) " % ver, str(e))
            if not m:
                raise
            op.uops_sha[ver] = m.group(1)
            op.compile(ver)
    dve_ops.OPS.append(op)
    dve_ops.CUSTOM_DVE_SPECS[name] = op.spec
    return op


def _pin_combined_act_set(arch: str):
    """Make `natural_log_exp_and_others` the only ACT table set offering Exp
    and Ln, so the table-load inserter uses ONE set for the whole kernel
    (its per-function choice otherwise alternates exp_and_others /
    natural_log, costing a ~1.3us table reload at every Ln<->Exp switch).
    get_activation_tables is functools.cache'd; mutating the returned dict
    (set indices unchanged) updates what the compile pass sees."""
    from concourse.hw_specs import get_activation_tables
    tabs = get_activation_tables(arch)
    assert AF.Exp in tabs["natural_log_exp_and_others"]
    assert AF.Ln in tabs["natural_log_exp_and_others"]
    for name, fns in tabs.items():
        if name != "natural_log_exp_and_others":
            fns.discard(AF.Exp)
            fns.discard(AF.Ln)


def build_bass(nt2: int):
    """nt2: phase-2 window tile count (multiple of 4)."""
    sqrelu_add = _register_sqrelu_add()
    ng2 = nt2 // 4  # phase-2 groups of 4 tiles

    nc = bacc.Bacc("TRN2", target_bir_lowering=False, debug=False,
                   num_devices=NCORES)
    _pin_combined_act_set(nc.m.arch)

    # ---- kernel I/O (per-core shards prepared on host; j pre-rotated) ----
    at = nc.dram_tensor("at", [F, N], BF16, kind="ExternalInput").ap()          # a.T, cols rotated
    bt2 = nc.dram_tensor("bt2", [128, 4, R], BF16, kind="ExternalInput").ap()   # (-2 b_c).T  [p,k,ii]
    atmy = nc.dram_tensor("atmy", [128, 4, R], BF16, kind="ExternalInput").ap() # a_c.T       [p,k,ii]
    aat = nc.dram_tensor("aat", [128, NT], F32, kind="ExternalInput").ap()      # aa[j] per rotated tile
    bbbc = nc.dram_tensor("bbbc", [128, R], F32, kind="ExternalInput").ap()     # bb broadcast over partitions
    onehotj = nc.dram_tensor("onehotj", [128, NT * NCLS], BF16, kind="ExternalInput").ap()  # rotated
    ohmy = nc.dram_tensor("ohmy", [NCLS, R], F32, kind="ExternalInput").ap()
    nohmy = nc.dram_tensor("nohmy", [NCLS, R], F32, kind="ExternalInput").ap()
    ddbias = nc.dram_tensor("ddbias", [1, R], F32, kind="ExternalInput").ap()   # aa_my + bb_c
    permt = nc.dram_tensor("permt", [32, nt2], F32, kind="ExternalInput").ap()  # canonical tile -> window tile perm

    out_same = nc.dram_tensor("out_same", [1, 1], F32, kind="ExternalOutput").ap()
    out_diag = nc.dram_tensor("out_diag", [1, 1], F32, kind="ExternalOutput").ap()
    out_ns = nc.dram_tensor("out_ns", [1, R], F32, kind="ExternalOutput").ap()

    with tile.TileContext(nc) as tc, ExitStack() as ctx:
        sb = ctx.enter_context(tc.tile_pool(name="sb", bufs=1))
        atp = ctx.enter_context(tc.tile_pool(name="atp", bufs=16))
        lp = ctx.enter_context(tc.tile_pool(name="lp", bufs=5))      # Dsq chunks f32
        dp = ctx.enter_context(tc.tile_pool(name="dp", bufs=4))      # non-window D f32
        ep = ctx.enter_context(tc.tile_pool(name="ep", bufs=3))      # Dexpm bf16
        work = ctx.enter_context(tc.tile_pool(name="work", bufs=2))
        small = ctx.enter_context(tc.tile_pool(name="small", bufs=2))
        tail = ctx.enter_context(tc.tile_pool(name="tail", bufs=1))
        dram = ctx.enter_context(tc.tile_pool(name="dram", bufs=1, space="DRAM"))

        # tiny dummy ACT op: forces the first (sqrt) table load to happen
        # during the initial DMA wait instead of before the first real sqrt
        dummy = sb.tile([1, 8], F32)
        nc.vector.memset(dummy, 1.0)
        last_sc = nc.scalar.activation(out=dummy, in_=dummy, func=AF.Sqrt)

        def chain_sc(inst):
            # the Tile scheduler orders engine queues by data-readiness;
            # chain scalar ops explicitly so sqrt/exp run in batches per
            # ACT table set instead of alternating (1.3us reload each)
            nonlocal last_sc
            add_dep_helper(inst.ins, last_sc.ins, False, "scalar batch order")
            last_sc = inst

        # ---- resident SBUF tensors (GEMM-critical ones first, sync queue) ----
        # bt2 split per-k so the first matmul starts after ~384KB, not 2.8MB
        bt_sb = sb.tile([128, 4, R], BF16)
        for k in range(4):
            nc.sync.dma_start(out=bt_sb[:, k, :], in_=bt2[:, k, :])
        aat_sb = sb.tile([128, NT], F32)
        bbbc_sb = sb.tile([128, R], F32)
        # remaining residents DMA'd inside the s-loop (after the first
        # at super-tile) so they don't delay GEMM start
        onehotj_sb = sb.tile([128, NT * NCLS], BF16)
        atmy_sb = sb.tile([128, 4, R], BF16)
        ohmy_sb = sb.tile([NCLS, R], F32)
        nohmy_sb = sb.tile([NCLS, R], F32)
        ddbias_sb = sb.tile([1, R], F32)
        permt_sb = sb.tile([32, nt2], F32)

        dT = sb.tile([128, nt2, R], F32)           # window D, 16KB/partition
        ones128 = sb.tile([1, 128], F32)
        nc.vector.memset(ones128, 1.0)
        ones128c = sb.tile([128, 1], BF16)
        nc.vector.memset(ones128c, 1.0)
        ones16 = sb.tile([NCLS, 1], F32)
        nc.vector.memset(ones16, 1.0)

        cc_in = dram.tile([1, R], F32)
        cc_out = dram.tile([1, N], F32)
        warm_in = dram.tile([1, 8], F32)
        warm_out = dram.tile([1, 8 * NCORES], F32)
        warm2_in = dram.tile([1, R], F32)
        warm2_out = dram.tile([1, N], F32)

        # warm up the collective path off the critical path; the gpsimd
        # queue carries ONLY collectives (v1 queued resident loads behind
        # these, stalling early cores ~13us)
        warm_sb = sb.tile([1, 8], F32)
        nc.vector.memset(warm_sb, 0.0)
        nc.sync.dma_start(out=warm_in, in_=warm_sb)
        w1 = nc.gpsimd.collective_compute(
            "AllGather", ALU.bypass,
            replica_groups=[list(range(NCORES))],
            ins=[warm_in[:].opt()], outs=[warm_out[:].opt()])
        warm2_sb = sb.tile([1, R], F32)
        nc.vector.memset(warm2_sb, 0.0)
        nc.sync.dma_start(out=warm2_in, in_=warm2_sb)
        w2 = nc.gpsimd.collective_compute(
            "AllGather", ALU.bypass,
            replica_groups=[list(range(NCORES))],
            ins=[warm2_in[:].opt()], outs=[warm2_out[:].opt()])
        add_dep_helper(w2.ins, w1.ins, True, "chain warmup collectives")

        # ================= PHASE 1: GEMM -> ln -> exp(.5) -> exp(1-D) =====
        with tc.tile_pool(name="bl_ps", bufs=1, space="PSUM") as bl_pool, \
             tc.tile_pool(name="dd_ps", bufs=1, space="PSUM") as dd_pool:

            dsq_ctx = tc.tile_pool(name="dsq_ps", bufs=3, space="PSUM")
            dsq_pool = dsq_ctx.__enter__()

            bl_ps = bl_pool.tile([NCLS, R], F32)   # negsum-by-label accumulator

            L4 = None
            pend_D = []    # (D4 tile, first local tile) awaiting its exp
            pend_E = []    # (E4 tile, first local tile) awaiting bylabel
            nbl = 0        # bylabel matmuls emitted (0..NT)

            def emit_bylabel():
                nonlocal nbl
                E4, t0 = pend_E.pop(0)
                for r_ in range(4):
                    t = t0 + r_
                    nc.tensor.matmul(
                        out=bl_ps,
                        lhsT=onehotj_sb[:, t * NCLS:(t + 1) * NCLS],
                        rhs=E4[:, r_, :],
                        start=(nbl == 0), stop=(nbl == NT - 1))
                    nbl += 1

            for s in range(4):
                at_t = []
                for k in range(4):
                    t_ = atp.tile([128, 1024], BF16, tag="at")
                    nc.sync.dma_start(
                        out=t_, in_=at[k * 128:(k + 1) * 128, s * 1024:(s + 1) * 1024])
                    at_t.append(t_)
                if s == 0:
                    # residents needed mid-GEMM, behind the first super-tile
                    # (must be EMITTED before their readers at p==2/3)
                    nc.sync.dma_start(out=aat_sb, in_=aat)
                    nc.sync.dma_start(out=bbbc_sb, in_=bbbc)
                    nc.sync.dma_start(out=atmy_sb, in_=atmy)
                    nc.sync.dma_start(out=onehotj_sb, in_=onehotj)
                    nc.sync.dma_start(out=ddbias_sb, in_=ddbias)
                elif s == 1:
                    nc.sync.dma_start(out=ohmy_sb, in_=ohmy)
                    nc.sync.dma_start(out=nohmy_sb, in_=nohmy)
                    nc.sync.dma_start(out=permt_sb, in_=permt)
                for v in range(4):
                    p = 4 * s + v          # psum tile index, 0..15
                    dsq = dsq_pool.tile([128, 2, 512], F32, tag="dsq")
                    for u in range(2):
                        w = 2 * v + u
                        for k in range(4):
                            nc.tensor.matmul(
                                out=dsq[:, u, :],
                                lhsT=at_t[k][:, w * 128:(w + 1) * 128],
                                rhs=bt_sb[:, k, :],
                                start=(k == 0), stop=(k == 3))
                    # interleave pending bylabel matmuls into the PE stream
                    # (lag ~2 psum tiles behind the producing matmuls)
                    if p >= 3 and (p % 2) == 1 and pend_E:
                        emit_bylabel()

                    # DVE adds the norm terms (fp32 exact): Dsq = g + aa + bb
                    # (psum -> half of an L4 chunk; frees the psum tile fast)
                    if (p % 2) == 0:
                        L4 = lp.tile([128, 4, 512], F32, tag="L4")
                    for u in range(2):
                        t = 2 * p + u      # local j-tile
                        nc.vector.scalar_tensor_tensor(
                            out=L4[:, 2 * (p % 2) + u, :], in0=dsq[:, u, :],
                            scalar=aat_sb[:, t:t + 1], in1=bbbc_sb,
                            op0=ALU.add, op1=ALU.add)

                    if (p % 2) == 1:
                        q = p // 2         # chunk of 4 local tiles
                        # D = sqrt(Dsq): window chunks persist in dT.
                        # sqrt and exp live in different ACT table sets;
                        # exps run in three batches ({0..2},{3..5},{6..7}):
                        # 6 table loads, but each exp batch starts ~10us
                        # sooner than a 2-batch split would allow
                        if q < ng2:
                            D4 = dT[:, 4 * q:4 * q + 4, :]
                        else:
                            D4 = dp.tile([128, 4, 512], F32, tag="D4")
                        chain_sc(nc.scalar.activation(out=D4, in_=L4,
                                                      func=AF.Sqrt))
                        pend_D.append((D4, 4 * q))
                        if q == 2:
                            # diag D_ii rides the first sqrt batch
                            ddiag_d = sb.tile([1, R], F32)
                            chain_sc(nc.scalar.activation(
                                out=ddiag_d, in_=ddsq_sb, func=AF.Sqrt))
                        if q in (2, 5, 7):
                            # Dexpm = exp(1 - D) for the completed batch
                            while pend_D:
                                D4b, t0b = pend_D.pop(0)
                                E4 = ep.tile([128, 4, 512], BF16, tag="E4")
                                chain_sc(nc.scalar.activation(
                                    out=E4, in_=D4b, func=AF.Exp,
                                    scale=-1.0, bias=1.0))
                                pend_E.append((E4, t0b))

                    # diag chain PE work, early (inputs are resident by now)
                    if p == 2:
                        dd_ps = dd_pool.tile([1, R], F32, name="dd_ps")
                        for k in range(4):
                            pr = work.tile([128, R], BF16, tag="dprod")
                            nc.vector.tensor_mul(pr, bt_sb[:, k, :], atmy_sb[:, k, :])
                            nc.tensor.matmul(out=dd_ps, lhsT=ones128c,
                                             rhs=pr, start=(k == 0), stop=(k == 3))
                        ddsq_sb = tail.tile([1, R], F32, tag="ddsq")
                        nc.vector.scalar_tensor_tensor(
                            out=ddsq_sb, in0=dd_ps, scalar=0.0, in1=ddbias_sb,
                            op0=ALU.bypass, op1=ALU.add)

            # drain remaining bylabel matmuls
            while pend_E:
                emit_bylabel()

            dsq_ctx.__exit__(None, None, None)   # free the 6 dsq banks

            with tc.tile_pool(name="ns_ps", bufs=1, space="PSUM") as ns_pool:
                # -- row_negsum: mask out own-label bucket, col-sum --
                prod_sb = tail.tile([NCLS, R], F32, tag="prod16a")
                nc.vector.tensor_mul(prod_sb, bl_ps, nohmy_sb)
                ns_ps = ns_pool.tile([1, R], F32, name="ns_ps")
                nc.tensor.matmul(out=ns_ps, lhsT=ones16, rhs=prod_sb,
                                 start=True, stop=True)
                ns_my = sb.tile([1, R], F32)
                nc.vector.tensor_copy(out=ns_my, in_=ns_ps)

                # broadcast ns_my across partitions: [128, R]
                nsbc_ps = ns_pool.tile([128, R], F32, name="nsbc_ps")
                nc.tensor.matmul(out=nsbc_ps, lhsT=ones128, rhs=ns_my,
                                 start=True, stop=True)
                ns_bc = sb.tile([128, R], F32)
                nc.vector.tensor_copy(out=ns_bc, in_=nsbc_ps)

        # ================= AllGather row_negsum ===========================
        nc.sync.dma_start(out=cc_in, in_=ns_my)
        nc.sync.dma_start(out=out_ns, in_=ns_my)
        cc_inst = nc.gpsimd.collective_compute(
            "AllGather", ALU.bypass,
            replica_groups=[list(range(NCORES))],
            ins=[cc_in[:].opt()], outs=[cc_out[:].opt()])
        add_dep_helper(cc_inst.ins, w2.ins, True, "gather after warmups")
        # contiguous DMA of the gathered vector; permt then maps canonical
        # tile rows -> the core's rotated window tiles
        nsflat_sb = sb.tile([32, 128], F32)
        # issued on the gpsimd queue (in-order behind the collective) to
        # skip the ~4us cross-engine semaphore hop before the read
        rd = nc.gpsimd.dma_start(out=nsflat_sb, in_=cc_out[0, :].rearrange("(t p) -> t p", p=128))
        add_dep_helper(rd.ins, cc_inst.ins, True, "read gathered ns after collective")

        # ================= PHASE 2: J = ln(ns_i+ns_j) + D; hinge^2 =======
        # local tiles 0..3 are the core's OWN tiles (sigma puts them first),
        # whose ns is known locally -> group 0 and the diag correction run
        # BEFORE the gather completes, hiding its latency
        with tc.tile_pool(name="hb_ps", bufs=1, space="PSUM") as hb_pool, \
             tc.tile_pool(name="ps2", bufs=2, space="PSUM") as ps2:

            # transpose ns_my [1, 512] -> nsmyT [128, 4] via 4 tiny matmuls
            ones11 = sb.tile([1, 1], F32)
            nc.vector.memset(ones11, 1.0)
            nsmyT_ps = ps2.tile([128, 4], F32, tag="nsmyT")
            for tt in range(4):
                nc.tensor.matmul(out=nsmyT_ps[:, tt:tt + 1],
                                 lhsT=ns_my[0:1, 128 * tt:128 * (tt + 1)],
                                 rhs=ones11, start=True, stop=True)
            nsmyT_sb = sb.tile([128, 4], F32)
            nc.vector.tensor_copy(out=nsmyT_sb, in_=nsmyT_ps)

            hb_ps = hb_pool.tile([NCLS, R], F32)   # hinge^2-by-label accumulator

            def phase2_group(g, bias_sb, bias_off):
                L4b = work.tile([128, 4, R], F32, tag="L")
                for u in range(4):
                    t = 4 * g + u
                    nc.scalar.activation(
                        out=L4b[:, u, :], in_=ns_bc, func=AF.Ln,
                        bias=bias_sb[:, bias_off + u:bias_off + u + 1], scale=1.0)
                for h in range(2):
                    h2 = work.tile([128, 2, R], BF16, tag="h2")
                    acc_d = small.tile([128, 1], F32, tag="accd")
                    nc.vector._custom_dve(
                        sqrelu_add, out=h2, in0=L4b[:, 2 * h:2 * h + 2, :],
                        in1=dT[:, 4 * g + 2 * h:4 * g + 2 * h + 2, :],
                        s0=0.0, accum_out=acc_d)
                    for u in range(2):
                        t = 4 * g + 2 * h + u
                        nc.tensor.matmul(
                            out=hb_ps,
                            lhsT=onehotj_sb[:, t * NCLS:(t + 1) * NCLS],
                            rhs=h2[:, u, :],
                            start=(t == 0), stop=(t == nt2 - 1))

            # group 0: own tiles, pre-gather (Ln bias reads PSUM directly)
            phase2_group(0, nsmyT_sb, 0)

            # diagonal correction relu(ln(2 ns_i) + D_ii)^2, also pre-gather
            lnterm = tail.tile([1, R], F32, tag="lnt")
            nc.scalar.activation(out=lnterm, in_=ns_my, func=AF.Ln, scale=2.0)
            dh2 = tail.tile([1, R], F32, tag="dh2")
            diag_acc = tail.tile([1, 1], F32, tag="dacc")
            nc.vector._custom_dve(sqrelu_add, out=dh2, in0=lnterm, in1=ddiag_d,
                                  s0=0.0, accum_out=diag_acc)
            nc.sync.dma_start(out=out_diag, in_=diag_acc)

            # remaining groups need the gathered neighbor ns
            nst_ps = ps2.tile([128, nt2], F32, tag="nst")
            nc.tensor.matmul(out=nst_ps, lhsT=nsflat_sb, rhs=permt_sb,
                             start=True, stop=True)
            nsall_sb = sb.tile([128, nt2], F32)
            nc.vector.tensor_copy(out=nsall_sb, in_=nst_ps)
            for g in range(1, ng2):
                phase2_group(g, nsall_sb, 4 * g)

            # -- combine: same-label sum (incl. diagonal); the ohmy mask
            # multiply carries a per-partition accumulator so the final
            # reduction is a 1-column matmul instead of a 512-wide fp32
            # matmul + reduce --
            prod2 = tail.tile([NCLS, R], F32, tag="prod16b")
            acc16 = small.tile([NCLS, 1], F32, tag="acc16")
            nc.vector.scalar_tensor_tensor(
                out=prod2, in0=hb_ps, scalar=0.0, in1=ohmy_sb,
                op0=ALU.bypass, op1=ALU.mult, accum_out=acc16)
            pos_ps = ps2.tile([1, 1], F32, tag="small")
            nc.tensor.matmul(out=pos_ps, lhsT=ones16, rhs=acc16,
                             start=True, stop=True)
            same_sum = tail.tile([1, 1], F32, tag="ssum")
            nc.vector.tensor_copy(out=same_sum, in_=pos_ps)
            nc.sync.dma_start(out=out_same, in_=same_sum)

    nc.compile()
    return nc


_CACHE: dict = {}


def _get_nc(nt2: int):
    key = ("nc", nt2)
    if key not in _CACHE:
        _CACHE[key] = build_bass(nt2)
    return _CACHE[key]


def prepare_inputs(a: np.ndarray, b: np.ndarray, labels: np.ndarray):
    """Host-side label sort, sharding and per-core rotated layout prep.

    Returns (per-core input maps, nt2, sorted labels)."""
    a = np.asarray(a, np.float32)
    b = np.asarray(b, np.float32)
    labels = np.asarray(labels)

    order = np.argsort(labels, kind="stable")
    a = a[order]
    b = b[order]
    sl = labels[order]

    # per-core phase-2 window: tiles covering all classes that overlap the
    # core's row range
    starts = np.searchsorted(sl, np.arange(NCLS), "left")
    ends = np.searchsorted(sl, np.arange(NCLS), "right")
    t0s, cnts = [], []
    for c in range(NCORES):
        r0 = starts[sl[c * R]]
        r1 = ends[sl[c * R + R - 1]]
        t0 = int(r0 // 128)
        cnt = int(-(-r1 // 128) - t0)
        t0s.append(t0)
        cnts.append(cnt)
    nt2 = -(-max(cnts) // 4) * 4     # round up to a multiple of 4
    assert nt2 <= NT

    at_full = np.ascontiguousarray(a.T).astype(NPBF16)   # [F, N] sorted
    aa = np.sum(a * a, axis=1, dtype=np.float32)
    bb = np.sum(b * b, axis=1, dtype=np.float32)
    oh = (sl[:, None] == np.arange(NCLS)[None, :]).astype(np.float32)  # [N,16]

    in_maps = []
    for c in range(NCORES):
        # local tile order: [my 4 tiles] + [window remainder] + pad + rest.
        # Own tiles first lets phase-2 group 0 run pre-gather; the window
        # (all same-class columns) stays within local tiles 0..nt2-1.
        my = list(range(4 * c, 4 * c + 4))
        window = [(t0s[c] + i) % NT for i in range(cnts[c])]
        rem = [t for t in window if t not in my]
        assert len(rem) <= nt2 - 4
        used = set(my) | set(rem)
        pad = [t for t in range(NT) if t not in used][:nt2 - 4 - len(rem)]
        used |= set(pad)
        rest = [t for t in range(NT) if t not in used]
        sigma = np.array(my + rem + pad + rest)
        assert sorted(sigma.tolist()) == list(range(NT))

        cols = (sigma[:, None] * 128 + np.arange(128)[None, :]).reshape(-1)
        at_c = np.ascontiguousarray(at_full[:, cols])
        aat_c = np.ascontiguousarray(aa[cols].reshape(NT, 128).T)  # [128, NT]
        onehotj_c = np.ascontiguousarray(
            oh[cols].reshape(NT, 128, NCLS).transpose(1, 0, 2)
            .reshape(128, NT * NCLS)).astype(NPBF16)

        perm = np.zeros((32, nt2), np.float32)
        for tl in range(nt2):
            perm[sigma[tl], tl] = 1.0

        slc = slice(c * R, (c + 1) * R)
        bt2 = np.ascontiguousarray(
            (-2.0 * b[slc]).T.reshape(4, 128, R).transpose(1, 0, 2)).astype(NPBF16)
        atmy = np.ascontiguousarray(
            a[slc].T.reshape(4, 128, R).transpose(1, 0, 2)).astype(NPBF16)
        bbbc_c = np.ascontiguousarray(
            np.broadcast_to(bb[slc][None, :], (128, R)))
        ohmy = np.ascontiguousarray(oh[slc].T)           # [16, R]
        nohmy = np.ascontiguousarray(1.0 - ohmy)
        ddbias = (aa[slc] + bb[slc]).reshape(1, R)
        in_maps.append({
            "at": at_c, "bt2": bt2, "atmy": atmy, "aat": aat_c,
            "bbbc": bbbc_c,
            "onehotj": onehotj_c, "ohmy": ohmy, "nohmy": nohmy,
            "ddbias": np.ascontiguousarray(ddbias), "permt": perm,
        })
    return in_maps, nt2, sl


def run(a, b, labels, trace=False, trace_kwargs=None):
    """Run on 8 NeuronCores; returns (loss, BassKernelResults)."""
    in_maps, nt2, sl = prepare_inputs(a, b, labels)
    nc = _get_nc(nt2)
    kw = {}
    if trace:
        kw = dict(trace=True, **(trace_kwargs or {}))
    res = run_bass_kernel_spmd(nc, in_maps, core_ids=list(range(NCORES)), **kw)

    counts = np.bincount(np.asarray(labels).astype(np.int64), minlength=NCLS)
    num_pos = float((counts.astype(np.float64) ** 2).sum() - N)

    total = 0.0
    for c in range(NCORES):
        r = res.results[c]
        total += float(r["out_same"][0, 0]) - float(r["out_diag"][0, 0])
    loss = total / (2.0 * num_pos)
    return np.asarray(np.float32(loss)), res


def kernel(a, b, labels):
    loss, _ = run(a, b, labels)
    return loss



# revision 7
# speedup vs baseline: 2.4444x; 2.4444x over previous
"""Trainium2 Bass kernel for nn_MetricLoss (lifted-structure-style metric loss).

Reference computation (N=4096 rows, F=512 features, 16 label classes):
    Dsq = ||b_i||^2 + ||a_j||^2 - 2 b@a.T ;  D = sqrt(max(Dsq,0))   [N,N]
    Dexpm = exp(1 - D)
    row_negsum[i] = sum_{j: lbl_j != lbl_i} Dexpm[i,j]
    J = log(row_negsum[i] + row_negsum[j]) + D
    loss = sum_{i!=j, lbl_i==lbl_j} relu(J)^2 / (2 * num_pos)

v2 design — fully decoupled cores (NO collectives):
  * Rows are sorted by label on the host; label classes are PAIRED
    (largest-with-smallest) and each core owns all rows of its 2 classes
    (padded with zero-rows to a common R_pad). Every positive pair (i, j)
    then has both ns_i and ns_j computed locally, so the AllGather of
    row_negsum is gone: no inter-core dependency at all. Per-core HW exec
    time no longer includes multi-core launch-skew waits (the v1 kernel
    showed 96us on the last-launched core vs 200us+ on early cores).
  * Per-core column permutation sigma puts the core's window j-tiles
    (tiles overlapping its 2 classes) at slots 0..nt2-1, so the phase-2
    loop structure is core-independent (pure SPMD); all class masks are
    input data, not program structure.
  * negsum via 3-column one-hot matmuls per j-tile (ones/classP/classQ);
    ns = total - own-class, combined with a [3,1] +-1 matmul.
  * ns_j in partition layout (nsT[128, t]) is built with a 5-step
    transpose + per-chunk (TI mask x PM permutation-matmul) accumulation,
    all from per-core input matrices - SPMD-safe despite per-core offsets.
  * D_ii (diagonal) is host-precomputed (same O(N F) class as aa/bb).
  * ACT runs in 4 table blocks (sqrt 0-3, exp 0-3, sqrt 4-7 + ddiag,
    exp 4..7) = 3 visible table loads; exp of the LAST chunk runs first
    in its block... (actually last block order 4,5,6,7 with bylabel
    matmuls trailing each exp so only the last chunk's bylabel gates ns).
  * GEMM free dim is chunked 2x272 (R_pad=544) so matmul outputs stay
    within PSUM banks; weight loads stay at 128 (chunks share lhsT).
  * Input DMAs are spread across the sync/scalar/vector/tensor queues,
    with slot-0/1 `at` strips first so the first matmul fires early.

The GEMM runs in bf16 (b @ a.T as 256 matmuls of [128k,128j]x[128,272i]).
"""

import re
import operator
import numpy as np
import ml_dtypes
from contextlib import ExitStack

import concourse.bass as bass
import concourse.tile as tile
from concourse import bacc, mybir
from concourse import dve_ops
from concourse.dve_spec import Spec, Src0, Src1, C0, relu, sq
from concourse.bass_utils import run_bass_kernel_spmd
from concourse.tile_rust import add_dep_helper

F32 = mybir.dt.float32
BF16 = mybir.dt.bfloat16
NPBF16 = ml_dtypes.bfloat16
AF = mybir.ActivationFunctionType
ALU = mybir.AluOpType

N = 4096          # rows (a and b)
F = 512           # features
NCORES = 8
NT = N // 128     # j-tiles of 128 partitions = 32
NCLS = 16         # label classes
MARGIN = 1.0


def _register_sqrelu_add():
    """Custom fused DVE op: out = relu(in0 + in1)^2, accum_out = c0 + sum(out)."""
    name = "SQRELU_ADD_ANT"
    for op in dve_ops.OPS:
        if op.name == name:
            return op
    op = dve_ops.DveOp(
        name,
        Spec(body=sq(relu(Src0 + Src1)), accum=operator.add, accum_init=C0),
        subdim=False,
        uops_sha={},
    )
    dve_ops._SUB_OPCODE_FOR_NAME[name] = (
        max(dve_ops._SUB_OPCODE_FOR_NAME.values()) + 1)
    assert dve_ops._SUB_OPCODE_FOR_NAME[name] < 0x20
    for ver in ("v3", "v4"):
        try:
            op.compile(ver)
        except ValueError as e:
            m = re.search(r"\(%s: ([0-9a-f]+) " % ver, str(e))
            if not m:
                raise
            op.uops_sha[ver] = m.group(1)
            op.compile(ver)
    dve_ops.OPS.append(op)
    dve_ops.CUSTOM_DVE_SPECS[name] = op.spec
    return op


def _pin_combined_act_set(arch: str):
    """Make `natural_log_exp_and_others` the only ACT table set offering Exp
    and Ln, so Ln needs no extra load after the exp batches."""
    from concourse.hw_specs import get_activation_tables
    tabs = get_activation_tables(arch)
    assert AF.Exp in tabs["natural_log_exp_and_others"]
    assert AF.Ln in tabs["natural_log_exp_and_others"]
    for name, fns in tabs.items():
        if name != "natural_log_exp_and_others":
            fns.discard(AF.Exp)
            fns.discard(AF.Ln)


def build_bass(R_pad: int, nt2: int, nt2p: int):
    """R_pad: padded rows/core; nt2: window tiles; nt2p: dT slots (mult of 4)."""
    sqrelu_add = _register_sqrelu_add()
    CH = R_pad // 2           # psum free-dim chunk (<=512)
    assert CH <= 512
    nU = -(-R_pad // 128)     # 128-chunks of the local row range

    nc = bacc.Bacc("TRN2", target_bir_lowering=False, debug=False,
                   num_devices=NCORES)
    _pin_combined_act_set(nc.m.arch)

    # ---- kernel I/O (per-core shards prepared on host; j permuted) ----
    at = nc.dram_tensor("at", [F, N], BF16, kind="ExternalInput").ap()
    bt2 = nc.dram_tensor("bt2", [128, 4, R_pad], BF16, kind="ExternalInput").ap()
    aat = nc.dram_tensor("aat", [128, NT], F32, kind="ExternalInput").ap()
    bbbc = nc.dram_tensor("bbbc", [128, R_pad], F32, kind="ExternalInput").ap()
    oh3 = nc.dram_tensor("oh3", [128, NT * 3], BF16, kind="ExternalInput").ap()
    oh2 = nc.dram_tensor("oh2", [128, nt2 * 2], BF16, kind="ExternalInput").ap()
    sel3 = nc.dram_tensor("sel3", [3, R_pad], F32, kind="ExternalInput").ap()
    ohmy2 = nc.dram_tensor("ohmy2", [2, R_pad], F32, kind="ExternalInput").ap()
    ti = nc.dram_tensor("ti", [128, nU * nt2p], BF16, kind="ExternalInput").ap()
    pm = nc.dram_tensor("pm", [128, nU * 128], BF16, kind="ExternalInput").ap()
    fillm = nc.dram_tensor("fillm", [128, nt2p], F32, kind="ExternalInput").ap()
    validm = nc.dram_tensor("validm", [1, R_pad], F32, kind="ExternalInput").ap()
    ddiag = nc.dram_tensor("ddiag", [1, R_pad], F32, kind="ExternalInput").ap()

    out_pos = nc.dram_tensor("out_pos", [2, 1], F32, kind="ExternalOutput").ap()
    out_diag = nc.dram_tensor("out_diag", [1, 1], F32, kind="ExternalOutput").ap()
    out_ns = nc.dram_tensor("out_ns", [1, R_pad], F32, kind="ExternalOutput").ap()

    with tile.TileContext(nc) as tc, ExitStack() as ctx:
        sb = ctx.enter_context(tc.tile_pool(name="sb", bufs=1))
        atp = ctx.enter_context(tc.tile_pool(name="atp", bufs=16))
        lp = ctx.enter_context(tc.tile_pool(name="lp", bufs=4))      # Dsq chunks f32
        dp = ctx.enter_context(tc.tile_pool(name="dp", bufs=4))      # non-window D f32
        ep = ctx.enter_context(tc.tile_pool(name="ep", bufs=4))      # Dexpm bf16
        work = ctx.enter_context(tc.tile_pool(name="work", bufs=2))
        small = ctx.enter_context(tc.tile_pool(name="small", bufs=2))
        tail = ctx.enter_context(tc.tile_pool(name="tail", bufs=1))

        # tiny dummy ACT op: forces the first (sqrt) table load during DMA wait
        dummy = sb.tile([1, 8], F32)
        nc.vector.memset(dummy, 1.0)
        last_sc = nc.scalar.activation(out=dummy, in_=dummy, func=AF.Sqrt)

        def chain_sc(inst):
            # explicit scalar-queue order: keeps sqrt/exp in table batches
            nonlocal last_sc
            add_dep_helper(inst.ins, last_sc.ins, False, "scalar batch order")
            last_sc = inst

        # ---- resident SBUF tensors / DMA issue plan ----
        # scalar queue: bt2 (behind the dummy act only)
        bt_sb = sb.tile([128, 4, R_pad], BF16)
        nc.scalar.dma_start(out=bt_sb, in_=bt2)
        # scalar queue: aat+bbbc (needed by the first stt; scalar is idle
        # until the first sqrt anyway)
        aat_sb = sb.tile([128, NT], F32)
        bbbc_sb = sb.tile([128, R_pad], F32)
        nc.scalar.dma_start(out=aat_sb, in_=aat)
        nc.scalar.dma_start(out=bbbc_sb, in_=bbbc)
        # sync queue: at strips, slots 0-1 first ([128,256] per k)
        s0a = [atp.tile([128, 256], BF16, tag="at0a", name=f"s0a{k}")
               for k in range(4)]
        for k in range(4):
            nc.sync.dma_start(out=s0a[k], in_=at[k * 128:(k + 1) * 128, 0:256])
        s0b = [atp.tile([128, 768], BF16, tag="at0b", name=f"s0b{k}")
               for k in range(4)]
        for k in range(4):
            nc.sync.dma_start(out=s0b[k], in_=at[k * 128:(k + 1) * 128, 256:1024])

        # remaining residents ride the tensor/vector queues mid-GEMM
        oh3_sb = sb.tile([128, NT * 3], BF16)
        oh2_sb = sb.tile([128, nt2 * 2], BF16)
        sel3_sb = sb.tile([3, R_pad], F32)
        ohmy2_sb = sb.tile([2, R_pad], F32)
        ti_sb = sb.tile([128, nU * nt2p], BF16)
        pm_sb = sb.tile([128, nU * 128], BF16)
        fill_sb = sb.tile([128, nt2p], F32)
        valid_sb = sb.tile([1, R_pad], F32)
        ddiag_sb = sb.tile([1, R_pad], F32)

        dT = sb.tile([128, nt2p, R_pad], F32)      # window D
        ones128c = sb.tile([1, 128], BF16)
        nc.vector.memset(ones128c, 1.0)
        ones11 = sb.tile([1, 1], F32)
        nc.vector.memset(ones11, 1.0)
        w3 = sb.tile([3, 1], F32)
        nc.vector.memset(w3, -1.0)
        nc.vector.memset(w3[0:1], 1.0)

        # ================= PHASE 1: GEMM -> +norms -> sqrt -> exp =========
        with tc.tile_pool(name="bl_ps", bufs=1, space="PSUM") as bl_pool:
            dsq_ctx = tc.tile_pool(name="dsq_ps", bufs=3, space="PSUM")
            dsq_pool = dsq_ctx.__enter__()

            bl_ps = bl_pool.tile([3, 2, 512], F32)   # negsum accumulator

            L4 = None
            pend_D = []    # (D4 tile, first slot) awaiting exp
            pend_E = []    # (E4 tile, first slot) awaiting bylabel
            nbl = 0        # bylabel slots emitted (0..NT)

            def emit_bylabel():
                nonlocal nbl
                E4, t0 = pend_E.pop(0)
                for r_ in range(4):
                    t = t0 + r_
                    for c_ in range(2):
                        nc.tensor.matmul(
                            out=bl_ps[:, c_, 0:CH],
                            lhsT=oh3_sb[:, t * 3:(t + 1) * 3],
                            rhs=E4[:, r_, c_ * CH:(c_ + 1) * CH],
                            start=(nbl == 0), stop=(nbl == NT - 1))
                    nbl += 1

            def emit_exp(n=100):
                while pend_D and n > 0:
                    D4b, t0b = pend_D.pop(0)
                    E4 = ep.tile([128, 4, R_pad], BF16, tag="E4")
                    chain_sc(nc.scalar.activation(
                        out=E4, in_=D4b, func=AF.Exp,
                        scale=-1.0, bias=float(MARGIN)))
                    pend_E.append((E4, t0b))
                    n -= 1

            for s in range(4):
                if s > 0:
                    at_t = []
                    for k in range(4):
                        t_ = atp.tile([128, 1024], BF16, tag="at")
                        nc.sync.dma_start(
                            out=t_,
                            in_=at[k * 128:(k + 1) * 128, s * 1024:(s + 1) * 1024])
                        at_t.append(t_)

                def lhs(k, w):
                    if s == 0:
                        return (s0a[k][:, w * 128:(w + 1) * 128] if w < 2
                                else s0b[k][:, (w - 2) * 128:(w - 1) * 128])
                    return at_t[k][:, w * 128:(w + 1) * 128]

                for w in range(8):
                    jt = 8 * s + w         # j-slot index, 0..31
                    dsq = dsq_pool.tile([128, 2, 512], F32, tag="dsq")
                    for c_ in range(2):
                        for k in range(4):
                            nc.tensor.matmul(
                                out=dsq[:, c_, 0:CH],
                                lhsT=lhs(k, w),
                                rhs=bt_sb[:, k, c_ * CH:(c_ + 1) * CH],
                                start=(k == 0), stop=(k == 3))

                    # resident DMA issues ride the (otherwise idle) gpsimd
                    # queue, scattered so they trail the early at strips
                    if jt == 1:
                        nc.gpsimd.dma_start(out=oh3_sb, in_=oh3)
                        nc.gpsimd.dma_start(out=pm_sb, in_=pm)
                        nc.gpsimd.dma_start(out=oh2_sb, in_=oh2)
                    elif jt == 3:
                        nc.gpsimd.dma_start(out=sel3_sb, in_=sel3)
                        nc.gpsimd.dma_start(out=ohmy2_sb, in_=ohmy2)
                        nc.gpsimd.dma_start(out=ti_sb, in_=ti)
                    elif jt == 5:
                        nc.gpsimd.dma_start(out=fill_sb, in_=fillm)
                        nc.gpsimd.dma_start(out=valid_sb, in_=validm)
                        nc.gpsimd.dma_start(out=ddiag_sb, in_=ddiag)

                    # DVE adds the norm terms: L4 = dsq + aa[j] + bb[i]
                    if jt % 4 == 0:
                        L4 = lp.tile([128, 4, R_pad], F32, tag="L4")
                    nc.vector.scalar_tensor_tensor(
                        out=L4[:, jt % 4, :].rearrange("p (c f) -> p c f", c=2),
                        in0=dsq[:, :, 0:CH],
                        scalar=aat_sb[:, jt:jt + 1],
                        in1=bbbc_sb.rearrange("p (c f) -> p c f", c=2),
                        op0=ALU.add, op1=ALU.add)

                    # interleave bylabel matmuls for batch-0 exps late in GEMM
                    if jt >= 26 and pend_E:
                        emit_bylabel()

                    if jt % 4 == 3:
                        q = jt // 4        # chunk of 4 slots
                        if 4 * q < nt2p:
                            D4 = dT[:, 4 * q:4 * q + 4, :]
                        else:
                            D4 = dp.tile([128, 4, R_pad], F32, tag="D4")
                        chain_sc(nc.scalar.activation(out=D4, in_=L4,
                                                      func=AF.Sqrt))
                        pend_D.append((D4, 4 * q))
                        if q == 3:
                            emit_exp()     # exp chunks 0..3

            # sqrt chunks 4..7 happened above; now exp 4..7 with bylabel
            # trailing each exp so only the last chunk's bylabel gates ns
            while pend_D:
                emit_exp(1)
                while len(pend_E) > 1:
                    emit_bylabel()
            while pend_E:
                emit_bylabel()

            dsq_ctx.__exit__(None, None, None)   # free the 6 dsq banks

            with tc.tile_pool(name="ns_ps", bufs=1, space="PSUM") as ns_pool:
                # -- ns = total - own-class:  w3.T @ (bl * sel3) --
                prod_sb = tail.tile([3, 2, CH], F32, tag="prod3")
                nc.vector.scalar_tensor_tensor(
                    out=prod_sb, in0=bl_ps[:, :, 0:CH], scalar=0.0,
                    in1=sel3_sb.rearrange("p (c f) -> p c f", c=2),
                    op0=ALU.bypass, op1=ALU.mult)
                ns_ps = ns_pool.tile([1, 2, 512], F32, name="ns_ps")
                for c_ in range(2):
                    nc.tensor.matmul(out=ns_ps[:, c_, 0:CH], lhsT=w3,
                                     rhs=prod_sb[:, c_, :],
                                     start=True, stop=True)
                ns_my = sb.tile([1, R_pad], F32)
                nc.vector.tensor_copy(
                    out=ns_my.rearrange("p (c f) -> p c f", c=2),
                    in_=ns_ps[:, :, 0:CH])
                nc.sync.dma_start(out=out_ns, in_=ns_my)
                ns_bf = sb.tile([1, R_pad], BF16)
                nc.vector.tensor_copy(out=ns_bf, in_=ns_my)

                # broadcast ns_my across partitions: [128, R_pad] bf16
                nsbc_ps = ns_pool.tile([128, 2, 512], F32, name="nsbc_ps")
                for c_ in range(2):
                    nc.tensor.matmul(out=nsbc_ps[:, c_, 0:CH], lhsT=ones128c,
                                     rhs=ns_bf[:, c_ * CH:(c_ + 1) * CH],
                                     start=True, stop=True)
                ns_bc = sb.tile([128, R_pad], BF16)
                nc.vector.tensor_copy(
                    out=ns_bc.rearrange("p (c f) -> p c f", c=2),
                    in_=nsbc_ps[:, :, 0:CH])

                # -- nsT: ns_j in [128, slot] layout via transpose+perm --
                nsL_ps = ns_pool.tile([128, nU], F32, name="nsL_ps")
                for u in range(nU):
                    lo = 128 * u
                    hi = min(R_pad, lo + 128)
                    nc.tensor.matmul(out=nsL_ps[0:hi - lo, u:u + 1],
                                     lhsT=ns_my[0:1, lo:hi], rhs=ones11,
                                     start=True, stop=True)
                nsL_sb = sb.tile([128, nU], BF16)
                nc.vector.memset(nsL_sb, 0.0)
                full = (nU - 1) if R_pad % 128 else nU
                nc.vector.tensor_copy(out=nsL_sb[:, 0:full],
                                      in_=nsL_ps[:, 0:full])
                if R_pad % 128:
                    rem = R_pad % 128
                    nc.vector.tensor_copy(out=nsL_sb[0:rem, full:nU],
                                          in_=nsL_ps[0:rem, full:nU])
                nsT_ps = ns_pool.tile([128, nt2p], F32, name="nsT_ps")
                for u in range(nU):
                    rhs_u = small.tile([128, nt2p], BF16, tag="rhsu")
                    nc.vector.scalar_tensor_tensor(
                        out=rhs_u, in0=ti_sb[:, u * nt2p:(u + 1) * nt2p],
                        scalar=nsL_sb[:, u:u + 1],
                        in1=ti_sb[:, u * nt2p:(u + 1) * nt2p],
                        op0=ALU.mult, op1=ALU.bypass)
                    nc.tensor.matmul(out=nsT_ps,
                                     lhsT=pm_sb[:, u * 128:(u + 1) * 128],
                                     rhs=rhs_u,
                                     start=(u == 0), stop=(u == nU - 1))
                nsT_sb = sb.tile([128, nt2p], F32)
                nc.vector.scalar_tensor_tensor(
                    out=nsT_sb, in0=nsT_ps, scalar=0.0, in1=fill_sb,
                    op0=ALU.bypass, op1=ALU.add)

        # ================= PHASE 2: J = ln(ns_i+ns_j) + D; hinge^2 =======
        with tc.tile_pool(name="hb_ps", bufs=1, space="PSUM") as hb_pool:
            hb_ps = hb_pool.tile([2, 2, 512], F32)

            # diagonal correction relu(ln(2 ns_i) + D_ii)^2, masked by valid
            lnterm = tail.tile([1, R_pad], F32, tag="lnt")
            chain_sc(nc.scalar.activation(out=lnterm, in_=ns_my, func=AF.Ln,
                                          scale=2.0))
            dh2 = tail.tile([1, R_pad], F32, tag="dh2")
            dummy_acc = small.tile([1, 1], F32, tag="dumacc")
            nc.vector._custom_dve(sqrelu_add, out=dh2, in0=lnterm,
                                  in1=ddiag_sb, s0=0.0, accum_out=dummy_acc)
            diag_acc = tail.tile([1, 1], F32, tag="dacc")
            dh2m = tail.tile([1, R_pad], F32, tag="dh2m")
            nc.vector.scalar_tensor_tensor(
                out=dh2m, in0=dh2, scalar=0.0, in1=valid_sb,
                op0=ALU.bypass, op1=ALU.mult, accum_out=diag_acc)
            nc.sync.dma_start(out=out_diag, in_=diag_acc)

            for t in range(nt2):
                Lt = work.tile([128, R_pad], F32, tag="L")
                chain_sc(nc.scalar.activation(
                    out=Lt, in_=ns_bc, func=AF.Ln,
                    bias=nsT_sb[:, t:t + 1], scale=1.0))
                h2 = work.tile([128, R_pad], BF16, tag="h2")
                acc_d = small.tile([128, 1], F32, tag="accd")
                nc.vector._custom_dve(
                    sqrelu_add, out=h2, in0=Lt, in1=dT[:, t, :],
                    s0=0.0, accum_out=acc_d)
                for c_ in range(2):
                    nc.tensor.matmul(
                        out=hb_ps[:, c_, 0:CH],
                        lhsT=oh2_sb[:, t * 2:(t + 1) * 2],
                        rhs=h2[:, c_ * CH:(c_ + 1) * CH],
                        start=(t == 0), stop=(t == nt2 - 1))

            # -- combine: mask by i-side class match, accumulate --
            prod2 = tail.tile([2, 2, CH], F32, tag="prod2")
            acc2 = small.tile([2, 1], F32, tag="acc2")
            nc.vector.scalar_tensor_tensor(
                out=prod2, in0=hb_ps[:, :, 0:CH], scalar=0.0,
                in1=ohmy2_sb.rearrange("p (c f) -> p c f", c=2),
                op0=ALU.bypass, op1=ALU.mult, accum_out=acc2)
            nc.sync.dma_start(out=out_pos, in_=acc2)

    nc.compile()
    return nc


_CACHE: dict = {}


def _get_nc(R_pad: int, nt2: int, nt2p: int):
    key = ("nc", R_pad, nt2, nt2p)
    if key not in _CACHE:
        _CACHE[key] = build_bass(R_pad, nt2, nt2p)
    return _CACHE[key]


def prepare_inputs(a: np.ndarray, b: np.ndarray, labels: np.ndarray):
    """Host-side label sort, class pairing, per-core shard + mask prep.

    Returns (per-core input maps, (R_pad, nt2, nt2p), meta)."""
    a = np.asarray(a, np.float32)
    b = np.asarray(b, np.float32)
    labels = np.asarray(labels)

    order = np.argsort(labels, kind="stable")
    a_s = a[order]
    b_s = b[order]
    sl = labels[order]
    counts = np.bincount(sl.astype(np.int64), minlength=NCLS)
    startscum = np.concatenate([[0], np.cumsum(counts)])

    co = np.argsort(counts)
    pairs = [(int(co[i]), int(co[NCLS - 1 - i])) for i in range(NCORES)]
    R_pad = int(max(counts[p] + counts[q] for p, q in pairs))
    R_pad = -(-R_pad // 32) * 32
    nU = -(-R_pad // 128)

    cores = []
    nt2 = 0
    for p, q in pairs:
        grows = np.concatenate([
            np.arange(startscum[p], startscum[p + 1]),
            np.arange(startscum[q], startscum[q + 1])])
        wtiles = sorted(set((grows // 128).tolist()))
        nt2 = max(nt2, len(wtiles))
        cores.append((p, q, grows, wtiles))
    nt2p = -(-nt2 // 4) * 4

    at_full = np.ascontiguousarray(a_s.T).astype(NPBF16)   # [F, N] sorted
    aa = np.sum(a_s * a_s, axis=1, dtype=np.float32)
    bb_s = np.sum(b_s * b_s, axis=1, dtype=np.float32)

    in_maps = []
    meta = []
    for c in range(NCORES):
        p, q, grows, wtiles = cores[c]
        Rc = len(grows)
        rest = [t for t in range(NT) if t not in wtiles]
        sigma = np.array(list(wtiles) + rest)
        slot_of = {t: s_ for s_, t in enumerate(sigma)}

        cols = (sigma[:, None] * 128 + np.arange(128)[None, :]).reshape(-1)
        at_c = np.ascontiguousarray(at_full[:, cols])
        aat_c = np.ascontiguousarray(aa[cols].reshape(NT, 128).T)  # [128, NT]

        glbl = sl[cols].reshape(NT, 128)                   # labels per slot
        oh3_c = np.zeros((NT, 128, 3), np.float32)
        oh3_c[:, :, 0] = 1.0
        oh3_c[:, :, 1] = glbl == p
        oh3_c[:, :, 2] = glbl == q
        oh3_c = np.ascontiguousarray(
            oh3_c.transpose(1, 0, 2).reshape(128, NT * 3)).astype(NPBF16)
        oh2_c = np.zeros((nt2, 128, 2), np.float32)
        oh2_c[:, :, 0] = glbl[:nt2] == p
        oh2_c[:, :, 1] = glbl[:nt2] == q
        oh2_c = np.ascontiguousarray(
            oh2_c.transpose(1, 0, 2).reshape(128, nt2 * 2)).astype(NPBF16)

        b_loc = np.zeros((R_pad, F), np.float32)
        b_loc[:Rc] = b_s[grows]
        a_my = np.zeros((R_pad, F), np.float32)
        a_my[:Rc] = a_s[grows]
        bb_loc = np.zeros(R_pad, np.float32)
        bb_loc[:Rc] = bb_s[grows]
        bt2_c = np.ascontiguousarray(
            (-2.0 * b_loc).T.reshape(4, 128, R_pad).transpose(1, 0, 2)
        ).astype(NPBF16)
        bbbc_c = np.ascontiguousarray(
            np.broadcast_to(bb_loc[None, :], (128, R_pad)))

        lbl_loc = np.full(R_pad, -1, np.int64)
        lbl_loc[:Rc] = sl[grows]
        selP = (lbl_loc == p).astype(np.float32)
        selQ = (lbl_loc == q).astype(np.float32)
        sel3_c = np.ascontiguousarray(
            np.stack([np.ones(R_pad, np.float32), selP, selQ], 0))
        ohmy2_c = np.ascontiguousarray(np.stack([selP, selQ], 0))

        ti_c = np.zeros((nU, 128, nt2p), np.float32)
        pm_c = np.zeros((nU, 128, 128), np.float32)
        used = np.zeros((128, nt2p), bool)
        for r in range(Rc):
            gr = grows[r]
            u, cc = r // 128, r % 128
            t_ = slot_of[gr // 128]
            ti_c[u, cc, t_] = 1.0
            pm_c[u, cc, gr % 128] = 1.0
            used[gr % 128, t_] = True
        ti_c = np.ascontiguousarray(
            ti_c.transpose(1, 0, 2).reshape(128, nU * nt2p)).astype(NPBF16)
        pm_c = np.ascontiguousarray(
            pm_c.transpose(1, 0, 2).reshape(128, nU * 128)).astype(NPBF16)
        fill_c = np.where(used, 0.0, 1.0).astype(np.float32)

        valid_c = (np.arange(R_pad) < Rc).astype(np.float32).reshape(1, R_pad)
        dd = np.sum(np.square(b_loc - a_my), axis=1, dtype=np.float32)
        ddiag_c = np.sqrt(np.maximum(dd, 0.0)).reshape(1, R_pad)

        in_maps.append({
            "at": at_c, "bt2": bt2_c, "aat": aat_c, "bbbc": bbbc_c,
            "oh3": oh3_c, "oh2": oh2_c, "sel3": sel3_c, "ohmy2": ohmy2_c,
            "ti": ti_c, "pm": pm_c, "fillm": np.ascontiguousarray(fill_c),
            "validm": valid_c, "ddiag": ddiag_c,
        })
        meta.append({"grows": grows, "Rc": Rc})
    return in_maps, (R_pad, nt2, nt2p), {"order": order, "cores": meta}


def run(a, b, labels, trace=False, trace_kwargs=None):
    """Run on 8 NeuronCores; returns (loss, BassKernelResults, meta)."""
    in_maps, dims, meta = prepare_inputs(a, b, labels)
    nc = _get_nc(*dims)
    kw = {}
    if trace:
        kw = dict(trace=True, **(trace_kwargs or {}))
    res = run_bass_kernel_spmd(nc, in_maps, core_ids=list(range(NCORES)), **kw)

    counts = np.bincount(np.asarray(labels).astype(np.int64), minlength=NCLS)
    num_pos = float((counts.astype(np.float64) ** 2).sum() - N)

    total = 0.0
    for c in range(NCORES):
        r = res.results[c]
        total += (float(r["out_pos"][0, 0]) + float(r["out_pos"][1, 0])
                  - float(r["out_diag"][0, 0]))
    loss = total / (2.0 * num_pos)
    return np.asarray(np.float32(loss)), res, meta


def kernel(a, b, labels):
    loss, _, _ = run(a, b, labels)
    return loss


# revision 12
# speedup vs baseline: 2.4639x; 1.0080x over previous
"""Trainium2 Bass kernel for nn_MetricLoss (lifted-structure-style metric loss).

Reference computation (N=4096 rows, F=512 features, 16 label classes):
    Dsq = ||b_i||^2 + ||a_j||^2 - 2 b@a.T ;  D = sqrt(max(Dsq,0))   [N,N]
    Dexpm = exp(1 - D)
    row_negsum[i] = sum_{j: lbl_j != lbl_i} Dexpm[i,j]
    J = log(row_negsum[i] + row_negsum[j]) + D
    loss = sum_{i!=j, lbl_i==lbl_j} relu(J)^2 / (2 * num_pos)

v2 design — fully decoupled cores (NO collectives):
  * Rows are sorted by label on the host; label classes are PAIRED
    (largest-with-smallest) and each core owns all rows of its 2 classes
    (padded with zero-rows to a common R_pad). Every positive pair (i, j)
    then has both ns_i and ns_j computed locally, so the AllGather of
    row_negsum is gone: no inter-core dependency at all. Per-core HW exec
    time no longer includes multi-core launch-skew waits (the v1 kernel
    showed 96us on the last-launched core vs 200us+ on early cores).
  * Per-core column permutation sigma puts the core's window j-tiles
    (tiles overlapping its 2 classes) at slots 0..nt2-1, so the phase-2
    loop structure is core-independent (pure SPMD); all class masks are
    input data, not program structure.
  * negsum via 3-column one-hot matmuls per j-tile (ones/classP/classQ);
    ns = total - own-class, combined with a [3,1] +-1 matmul.
  * ns_j in partition layout (nsT[128, t]) is built with a 5-step
    transpose + per-chunk (TI mask x PM permutation-matmul) accumulation,
    all from per-core input matrices - SPMD-safe despite per-core offsets.
  * D_ii (diagonal) is host-precomputed (same O(N F) class as aa/bb).
  * ACT runs in 4 table blocks (sqrt 0-3, exp 0-3, sqrt 4-7 + ddiag,
    exp 4..7) = 3 visible table loads; exp of the LAST chunk runs first
    in its block... (actually last block order 4,5,6,7 with bylabel
    matmuls trailing each exp so only the last chunk's bylabel gates ns).
  * GEMM free dim is chunked 2x272 (R_pad=544) so matmul outputs stay
    within PSUM banks; weight loads stay at 128 (chunks share lhsT).
  * Input DMAs are spread across the sync/scalar/vector/tensor queues,
    with slot-0/1 `at` strips first so the first matmul fires early.

The GEMM runs in bf16 (b @ a.T as 256 matmuls of [128k,128j]x[128,272i]).
"""

import re
import operator
import numpy as np
import ml_dtypes
from contextlib import ExitStack

import concourse.bass as bass
import concourse.tile as tile
from concourse import bacc, mybir
from concourse import dve_ops
from concourse.dve_spec import Spec, Src0, Src1, C0, relu, sq
from concourse.bass_utils import run_bass_kernel_spmd
from concourse.tile_rust import add_dep_helper

F32 = mybir.dt.float32
BF16 = mybir.dt.bfloat16
NPBF16 = ml_dtypes.bfloat16
AF = mybir.ActivationFunctionType
ALU = mybir.AluOpType

N = 4096          # rows (a and b)
F = 512           # features
NCORES = 8
NT = N // 128     # j-tiles of 128 partitions = 32
NCLS = 16         # label classes
MARGIN = 1.0


def _register_sqrelu_add():
    """Custom fused DVE op: out = relu(in0 + in1)^2, accum_out = c0 + sum(out)."""
    name = "SQRELU_ADD_ANT"
    for op in dve_ops.OPS:
        if op.name == name:
            return op
    op = dve_ops.DveOp(
        name,
        Spec(body=sq(relu(Src0 + Src1)), accum=operator.add, accum_init=C0),
        subdim=False,
        uops_sha={},
    )
    dve_ops._SUB_OPCODE_FOR_NAME[name] = (
        max(dve_ops._SUB_OPCODE_FOR_NAME.values()) + 1)
    assert dve_ops._SUB_OPCODE_FOR_NAME[name] < 0x20
    for ver in ("v3", "v4"):
        try:
            op.compile(ver)
        except ValueError as e:
            m = re.search(r"\(%s: ([0-9a-f]+) " % ver, str(e))
            if not m:
                raise
            op.uops_sha[ver] = m.group(1)
            op.compile(ver)
    dve_ops.OPS.append(op)
    dve_ops.CUSTOM_DVE_SPECS[name] = op.spec
    return op


def _pin_combined_act_set(arch: str):
    """Make `natural_log_exp_and_others` the only ACT table set offering Exp
    and Ln, so Ln needs no extra load after the exp batches."""
    from concourse.hw_specs import get_activation_tables
    tabs = get_activation_tables(arch)
    assert AF.Exp in tabs["natural_log_exp_and_others"]
    assert AF.Ln in tabs["natural_log_exp_and_others"]
    for name, fns in tabs.items():
        if name != "natural_log_exp_and_others":
            fns.discard(AF.Exp)
            fns.discard(AF.Ln)


def build_bass(R_pad: int, nt2: int, nt2p: int):
    """R_pad: padded rows/core; nt2: window tiles; nt2p: dT slots (mult of 4)."""
    sqrelu_add = _register_sqrelu_add()
    CH = R_pad // 2           # psum free-dim chunk (<=512)
    assert CH <= 512
    nU = -(-R_pad // 128)     # 128-chunks of the local row range

    nc = bacc.Bacc("TRN2", target_bir_lowering=False, debug=False,
                   num_devices=NCORES)
    _pin_combined_act_set(nc.m.arch)

    # ---- kernel I/O (per-core shards prepared on host; j permuted) ----
    at = nc.dram_tensor("at", [F, N], BF16, kind="ExternalInput").ap()
    bt2 = nc.dram_tensor("bt2", [128, 4, R_pad], BF16, kind="ExternalInput").ap()
    aat = nc.dram_tensor("aat", [128, NT], F32, kind="ExternalInput").ap()
    bbbc = nc.dram_tensor("bbbc", [128, R_pad], F32, kind="ExternalInput").ap()
    oh3 = nc.dram_tensor("oh3", [128, NT * 3], BF16, kind="ExternalInput").ap()
    oh2 = nc.dram_tensor("oh2", [128, nt2 * 2], BF16, kind="ExternalInput").ap()
    sel3 = nc.dram_tensor("sel3", [3, R_pad], F32, kind="ExternalInput").ap()
    ohmy2 = nc.dram_tensor("ohmy2", [2, R_pad], F32, kind="ExternalInput").ap()
    ti = nc.dram_tensor("ti", [128, nU * nt2p], BF16, kind="ExternalInput").ap()
    pm = nc.dram_tensor("pm", [128, nU * 128], BF16, kind="ExternalInput").ap()
    fillm = nc.dram_tensor("fillm", [128, nt2p], F32, kind="ExternalInput").ap()
    validm = nc.dram_tensor("validm", [1, R_pad], F32, kind="ExternalInput").ap()
    ddiag = nc.dram_tensor("ddiag", [1, R_pad], F32, kind="ExternalInput").ap()

    out_pos = nc.dram_tensor("out_pos", [2, 1], F32, kind="ExternalOutput").ap()
    out_diag = nc.dram_tensor("out_diag", [1, 1], F32, kind="ExternalOutput").ap()
    out_ns = nc.dram_tensor("out_ns", [1, R_pad], F32, kind="ExternalOutput").ap()

    with tile.TileContext(nc) as tc, ExitStack() as ctx:
        sb = ctx.enter_context(tc.tile_pool(name="sb", bufs=1))
        atp = ctx.enter_context(tc.tile_pool(name="atp", bufs=16))
        lp = ctx.enter_context(tc.tile_pool(name="lp", bufs=4))      # Dsq chunks f32
        dp = ctx.enter_context(tc.tile_pool(name="dp", bufs=4))      # non-window D f32
        ep = ctx.enter_context(tc.tile_pool(name="ep", bufs=4))      # Dexpm bf16
        work = ctx.enter_context(tc.tile_pool(name="work", bufs=2))
        small = ctx.enter_context(tc.tile_pool(name="small", bufs=2))
        tail = ctx.enter_context(tc.tile_pool(name="tail", bufs=1))

        # tiny dummy ACT op: forces the first (sqrt) table load during DMA wait
        dummy = sb.tile([1, 8], F32)
        nc.vector.memset(dummy, 1.0)
        last_sc = nc.scalar.activation(out=dummy, in_=dummy, func=AF.Sqrt)

        def chain_sc(inst):
            # explicit scalar-queue order: keeps sqrt/exp in table batches
            nonlocal last_sc
            add_dep_helper(inst.ins, last_sc.ins, False, "scalar batch order")
            last_sc = inst

        # ---- resident SBUF tensors / DMA issue plan ----
        # scalar queue: bt2 split in two (k0-1 lands before the first
        # psum group needs k2-3)
        bt_sb = sb.tile([128, 4, R_pad], BF16)
        nc.scalar.dma_start(out=bt_sb[:, 0:2, :], in_=bt2[:, 0:2, :])
        nc.scalar.dma_start(out=bt_sb[:, 2:4, :], in_=bt2[:, 2:4, :])
        # gpsimd queue (own DMA engine): aat+bbbc first - the first stt
        # needs them; behind the sync queue's at stream they'd land ~19us
        aat_sb = sb.tile([128, NT], F32)
        bbbc_sb = sb.tile([128, R_pad], F32)
        nc.gpsimd.dma_start(out=aat_sb, in_=aat)
        nc.gpsimd.dma_start(out=bbbc_sb, in_=bbbc)
        # sync queue: at strips, slots 0-1 first ([128,256] per k)
        s0a = [atp.tile([128, 256], BF16, tag="at0a", name=f"s0a{k}")
               for k in range(4)]
        for k in range(4):
            nc.sync.dma_start(out=s0a[k], in_=at[k * 128:(k + 1) * 128, 0:256])
        s0b = [atp.tile([128, 768], BF16, tag="at0b", name=f"s0b{k}")
               for k in range(4)]
        for k in range(4):
            nc.sync.dma_start(out=s0b[k], in_=at[k * 128:(k + 1) * 128, 256:1024])

        # remaining residents ride the tensor/vector queues mid-GEMM
        oh3_sb = sb.tile([128, NT * 3], BF16)
        oh2_sb = sb.tile([128, nt2 * 2], BF16)
        sel3_sb = sb.tile([3, R_pad], F32)
        ohmy2_sb = sb.tile([2, R_pad], F32)
        ti_sb = sb.tile([128, nU * nt2p], BF16)
        pm_sb = sb.tile([128, nU * 128], BF16)
        fill_sb = sb.tile([128, nt2p], F32)
        valid_sb = sb.tile([1, R_pad], F32)
        ddiag_sb = sb.tile([1, R_pad], F32)

        dT = sb.tile([128, nt2p, R_pad], F32)      # window D
        ones128c = sb.tile([1, 128], BF16)
        nc.vector.memset(ones128c, 1.0)
        ones11 = sb.tile([1, 1], F32)
        nc.vector.memset(ones11, 1.0)
        w3 = sb.tile([3, 1], F32)
        nc.vector.memset(w3, -1.0)
        nc.vector.memset(w3[0:1], 1.0)

        # ================= PHASE 1: GEMM -> +norms -> sqrt -> exp =========
        with tc.tile_pool(name="bl_ps", bufs=1, space="PSUM") as bl_pool:
            dsq_ctx = tc.tile_pool(name="dsq_ps", bufs=3, space="PSUM")
            dsq_pool = dsq_ctx.__enter__()

            bl_ps = bl_pool.tile([3, 2, 512], F32)   # negsum accumulator

            L4 = None
            pend_D = []    # (D4 tile, first slot) awaiting exp
            pend_E = []    # (E4 tile, first slot) awaiting bylabel
            nbl = 0        # bylabel slots emitted (0..NT)

            def emit_bylabel():
                nonlocal nbl
                E4, t0, ntiles = pend_E.pop(0)
                for r_ in range(ntiles):
                    t = t0 + r_
                    for c_ in range(2):
                        nc.tensor.matmul(
                            out=bl_ps[:, c_, 0:CH],
                            lhsT=oh3_sb[:, t * 3:(t + 1) * 3],
                            rhs=E4[:, r_, c_ * CH:(c_ + 1) * CH],
                            start=(nbl == 0), stop=(nbl == NT - 1))
                    nbl += 1

            def emit_exp(n=100):
                while pend_D and n > 0:
                    D4b, t0b = pend_D.pop(0)
                    E4 = ep.tile([128, 4, R_pad], BF16, tag="E4")
                    chain_sc(nc.scalar.activation(
                        out=E4, in_=D4b, func=AF.Exp,
                        scale=-1.0, bias=float(MARGIN)))
                    pend_E.append((E4, t0b, 4))
                    n -= 1

            for s in range(4):
                if s > 0:
                    at_t = []
                    for k in range(4):
                        t_ = atp.tile([128, 1024], BF16, tag="at")
                        nc.sync.dma_start(
                            out=t_,
                            in_=at[k * 128:(k + 1) * 128, s * 1024:(s + 1) * 1024])
                        at_t.append(t_)

                def lhs(k, w):
                    if s == 0:
                        return (s0a[k][:, w * 128:(w + 1) * 128] if w < 2
                                else s0b[k][:, (w - 2) * 128:(w - 1) * 128])
                    return at_t[k][:, w * 128:(w + 1) * 128]

                for w in range(8):
                    jt = 8 * s + w         # j-slot index, 0..31
                    dsq = dsq_pool.tile([128, 2, 512], F32, tag="dsq")
                    for c_ in range(2):
                        for k in range(4):
                            nc.tensor.matmul(
                                out=dsq[:, c_, 0:CH],
                                lhsT=lhs(k, w),
                                rhs=bt_sb[:, k, c_ * CH:(c_ + 1) * CH],
                                start=(k == 0), stop=(k == 3))

                    # resident DMA issues ride the (otherwise idle) gpsimd
                    # queue, scattered so they trail the early at strips
                    if jt == 1:
                        nc.gpsimd.dma_start(out=oh3_sb, in_=oh3)
                        nc.gpsimd.dma_start(out=pm_sb, in_=pm)
                        nc.gpsimd.dma_start(out=oh2_sb, in_=oh2)
                    elif jt == 3:
                        nc.gpsimd.dma_start(out=sel3_sb, in_=sel3)
                        nc.gpsimd.dma_start(out=ohmy2_sb, in_=ohmy2)
                        nc.gpsimd.dma_start(out=ti_sb, in_=ti)
                    elif jt == 5:
                        nc.gpsimd.dma_start(out=fill_sb, in_=fillm)
                        nc.gpsimd.dma_start(out=valid_sb, in_=validm)
                        nc.gpsimd.dma_start(out=ddiag_sb, in_=ddiag)

                    # DVE adds the norm terms: L4 = dsq + aa[j] + bb[i]
                    if jt % 4 == 0:
                        L4 = lp.tile([128, 4, R_pad], F32, tag="L4")
                    nc.vector.scalar_tensor_tensor(
                        out=L4[:, jt % 4, :].rearrange("p (c f) -> p c f", c=2),
                        in0=dsq[:, :, 0:CH],
                        scalar=aat_sb[:, jt:jt + 1],
                        in1=bbbc_sb.rearrange("p (c f) -> p c f", c=2),
                        op0=ALU.add, op1=ALU.add)

                    # interleave bylabel matmuls for batch-0 exps late in GEMM
                    if jt >= 26 and pend_E:
                        emit_bylabel()

                    if jt % 4 == 3:
                        q = jt // 4        # chunk of 4 slots
                        if 4 * q < nt2p:
                            D4 = dT[:, 4 * q:4 * q + 4, :]
                        else:
                            D4 = dp.tile([128, 4, R_pad], F32, tag="D4")
                        chain_sc(nc.scalar.activation(out=D4, in_=L4,
                                                      func=AF.Sqrt))
                        pend_D.append((D4, 4 * q))
                        if q == 3:
                            emit_exp()     # exp chunks 0..3

            # sqrt chunks 4..7 happened above; now exp 4..7 with bylabel
            # trailing each exp so only the last chunk's bylabel gates ns.
            # The final chunk's exp is split in half so its bylabel tail
            # is ~1 us shorter.
            while pend_D:
                if len(pend_D) == 1:
                    D4b, t0b = pend_D.pop(0)
                    for h_ in range(2):
                        E2 = ep.tile([128, 2, R_pad], BF16, tag="E4",
                                     name=f"E2h{h_}")
                        chain_sc(nc.scalar.activation(
                            out=E2, in_=D4b[:, 2 * h_:2 * h_ + 2, :],
                            func=AF.Exp, scale=-1.0, bias=float(MARGIN)))
                        pend_E.append((E2, t0b + 2 * h_, 2))
                        while len(pend_E) > 1:
                            emit_bylabel()
                else:
                    emit_exp(1)
                while len(pend_E) > 1:
                    emit_bylabel()
            while pend_E:
                emit_bylabel()

            dsq_ctx.__exit__(None, None, None)   # free the 6 dsq banks

            with tc.tile_pool(name="ns_ps", bufs=1, space="PSUM") as ns_pool:
                # -- ns = total - own-class:  w3.T @ (bl * sel3) --
                prod_sb = tail.tile([3, 2, CH], F32, tag="prod3")
                nc.vector.scalar_tensor_tensor(
                    out=prod_sb, in0=bl_ps[:, :, 0:CH], scalar=0.0,
                    in1=sel3_sb.rearrange("p (c f) -> p c f", c=2),
                    op0=ALU.bypass, op1=ALU.mult)
                ns_ps = ns_pool.tile([1, 2, 512], F32, name="ns_ps")
                for c_ in range(2):
                    nc.tensor.matmul(out=ns_ps[:, c_, 0:CH], lhsT=w3,
                                     rhs=prod_sb[:, c_, :],
                                     start=True, stop=True)
                ns_my = sb.tile([1, R_pad], F32)
                nc.vector.tensor_copy(
                    out=ns_my.rearrange("p (c f) -> p c f", c=2),
                    in_=ns_ps[:, :, 0:CH])
                ns_bf = sb.tile([1, R_pad], BF16)
                nc.vector.tensor_copy(out=ns_bf, in_=ns_my)

                # -- nsT: ns_j in [128, slot] layout via transpose+perm --
                nsL_ps = ns_pool.tile([128, nU], F32, name="nsL_ps")
                for u in range(nU):
                    lo = 128 * u
                    hi = min(R_pad, lo + 128)
                    nc.tensor.matmul(out=nsL_ps[0:hi - lo, u:u + 1],
                                     lhsT=ns_my[0:1, lo:hi], rhs=ones11,
                                     start=True, stop=True)
                # broadcast ns_my across partitions: [128, R_pad] bf16
                nsbc_ps = ns_pool.tile([128, 2, 512], F32, name="nsbc_ps")
                for c_ in range(2):
                    nc.tensor.matmul(out=nsbc_ps[:, c_, 0:CH], lhsT=ones128c,
                                     rhs=ns_bf[:, c_ * CH:(c_ + 1) * CH],
                                     start=True, stop=True)

                nsL_sb = sb.tile([128, nU], BF16)
                nc.vector.memset(nsL_sb, 0.0)
                full = (nU - 1) if R_pad % 128 else nU
                nc.vector.tensor_copy(out=nsL_sb[:, 0:full],
                                      in_=nsL_ps[:, 0:full])
                if R_pad % 128:
                    rem = R_pad % 128
                    nc.vector.tensor_copy(out=nsL_sb[0:rem, full:nU],
                                          in_=nsL_ps[0:rem, full:nU])
                nsT_ps = ns_pool.tile([128, nt2p], F32, name="nsT_ps")
                for u in range(nU):
                    rhs_u = small.tile([128, nt2p], BF16, tag="rhsu")
                    nc.vector.scalar_tensor_tensor(
                        out=rhs_u, in0=ti_sb[:, u * nt2p:(u + 1) * nt2p],
                        scalar=nsL_sb[:, u:u + 1],
                        in1=ti_sb[:, u * nt2p:(u + 1) * nt2p],
                        op0=ALU.mult, op1=ALU.bypass)
                    nc.tensor.matmul(out=nsT_ps,
                                     lhsT=pm_sb[:, u * 128:(u + 1) * 128],
                                     rhs=rhs_u,
                                     start=(u == 0), stop=(u == nU - 1))
                nsT_sb = sb.tile([128, nt2p], F32)
                nc.vector.scalar_tensor_tensor(
                    out=nsT_sb, in0=nsT_ps, scalar=0.0, in1=fill_sb,
                    op0=ALU.bypass, op1=ALU.add)
                ns_bc = sb.tile([128, R_pad], BF16)
                nc.vector.tensor_copy(
                    out=ns_bc.rearrange("p (c f) -> p c f", c=2),
                    in_=nsbc_ps[:, :, 0:CH])
                nc.sync.dma_start(out=out_ns, in_=ns_my)

        # ================= PHASE 2: J = ln(ns_i+ns_j) + D; hinge^2 =======
        with tc.tile_pool(name="hb_ps", bufs=1, space="PSUM") as hb_pool:
            hb_ps = hb_pool.tile([2, 2, 512], F32)

            for t in range(nt2):
                Lt = work.tile([128, R_pad], F32, tag="L")
                chain_sc(nc.scalar.activation(
                    out=Lt, in_=ns_bc, func=AF.Ln,
                    bias=nsT_sb[:, t:t + 1], scale=1.0))
                h2 = work.tile([128, R_pad], BF16, tag="h2")
                acc_d = small.tile([128, 1], F32, tag="accd")
                nc.vector._custom_dve(
                    sqrelu_add, out=h2, in0=Lt, in1=dT[:, t, :],
                    s0=0.0, accum_out=acc_d)
                for c_ in range(2):
                    nc.tensor.matmul(
                        out=hb_ps[:, c_, 0:CH],
                        lhsT=oh2_sb[:, t * 2:(t + 1) * 2],
                        rhs=h2[:, c_ * CH:(c_ + 1) * CH],
                        start=(t == 0), stop=(t == nt2 - 1))

            # diagonal correction relu(ln(2 ns_i) + D_ii)^2, masked by valid
            # (emitted after the phase-2 Lns so it doesn't delay Ln0; it
            # completes during the hb drain)
            lnterm = tail.tile([1, R_pad], F32, tag="lnt")
            chain_sc(nc.scalar.activation(out=lnterm, in_=ns_my, func=AF.Ln,
                                          scale=2.0))
            dh2 = tail.tile([1, R_pad], F32, tag="dh2")
            dummy_acc = small.tile([1, 1], F32, tag="dumacc")
            nc.vector._custom_dve(sqrelu_add, out=dh2, in0=lnterm,
                                  in1=ddiag_sb, s0=0.0, accum_out=dummy_acc)
            diag_acc = tail.tile([1, 1], F32, tag="dacc")
            dh2m = tail.tile([1, R_pad], F32, tag="dh2m")
            nc.vector.scalar_tensor_tensor(
                out=dh2m, in0=dh2, scalar=0.0, in1=valid_sb,
                op0=ALU.bypass, op1=ALU.mult, accum_out=diag_acc)
            nc.sync.dma_start(out=out_diag, in_=diag_acc)

            # -- combine: mask by i-side class match, accumulate --
            prod2 = tail.tile([2, 2, CH], F32, tag="prod2")
            acc2 = small.tile([2, 1], F32, tag="acc2")
            nc.vector.scalar_tensor_tensor(
                out=prod2, in0=hb_ps[:, :, 0:CH], scalar=0.0,
                in1=ohmy2_sb.rearrange("p (c f) -> p c f", c=2),
                op0=ALU.bypass, op1=ALU.mult, accum_out=acc2)
            nc.sync.dma_start(out=out_pos, in_=acc2)

    nc.compile()
    return nc


_CACHE: dict = {}


def _get_nc(R_pad: int, nt2: int, nt2p: int):
    key = ("nc", R_pad, nt2, nt2p)
    if key not in _CACHE:
        _CACHE[key] = build_bass(R_pad, nt2, nt2p)
    return _CACHE[key]


def prepare_inputs(a: np.ndarray, b: np.ndarray, labels: np.ndarray):
    """Host-side label sort, class pairing, per-core shard + mask prep.

    Returns (per-core input maps, (R_pad, nt2, nt2p), meta)."""
    a = np.asarray(a, np.float32)
    b = np.asarray(b, np.float32)
    labels = np.asarray(labels)

    order = np.argsort(labels, kind="stable")
    a_s = a[order]
    b_s = b[order]
    sl = labels[order]
    counts = np.bincount(sl.astype(np.int64), minlength=NCLS)
    startscum = np.concatenate([[0], np.cumsum(counts)])

    co = np.argsort(counts)
    pairs = [(int(co[i]), int(co[NCLS - 1 - i])) for i in range(NCORES)]
    R_pad = int(max(counts[p] + counts[q] for p, q in pairs))
    R_pad = -(-R_pad // 32) * 32
    nU = -(-R_pad // 128)

    cores = []
    nt2 = 0
    for p, q in pairs:
        grows = np.concatenate([
            np.arange(startscum[p], startscum[p + 1]),
            np.arange(startscum[q], startscum[q + 1])])
        wtiles = sorted(set((grows // 128).tolist()))
        nt2 = max(nt2, len(wtiles))
        cores.append((p, q, grows, wtiles))
    nt2p = -(-nt2 // 4) * 4

    at_full = np.ascontiguousarray(a_s.T).astype(NPBF16)   # [F, N] sorted
    aa = np.sum(a_s * a_s, axis=1, dtype=np.float32)
    bb_s = np.sum(b_s * b_s, axis=1, dtype=np.float32)

    in_maps = []
    meta = []
    for c in range(NCORES):
        p, q, grows, wtiles = cores[c]
        Rc = len(grows)
        rest = [t for t in range(NT) if t not in wtiles]
        sigma = np.array(list(wtiles) + rest)
        slot_of = {t: s_ for s_, t in enumerate(sigma)}

        cols = (sigma[:, None] * 128 + np.arange(128)[None, :]).reshape(-1)
        at_c = np.ascontiguousarray(at_full[:, cols])
        aat_c = np.ascontiguousarray(aa[cols].reshape(NT, 128).T)  # [128, NT]

        glbl = sl[cols].reshape(NT, 128)                   # labels per slot
        oh3_c = np.zeros((NT, 128, 3), np.float32)
        oh3_c[:, :, 0] = 1.0
        oh3_c[:, :, 1] = glbl == p
        oh3_c[:, :, 2] = glbl == q
        oh3_c = np.ascontiguousarray(
            oh3_c.transpose(1, 0, 2).reshape(128, NT * 3)).astype(NPBF16)
        oh2_c = np.zeros((nt2, 128, 2), np.float32)
        oh2_c[:, :, 0] = glbl[:nt2] == p
        oh2_c[:, :, 1] = glbl[:nt2] == q
        oh2_c = np.ascontiguousarray(
            oh2_c.transpose(1, 0, 2).reshape(128, nt2 * 2)).astype(NPBF16)

        b_loc = np.zeros((R_pad, F), np.float32)
        b_loc[:Rc] = b_s[grows]
        a_my = np.zeros((R_pad, F), np.float32)
        a_my[:Rc] = a_s[grows]
        bb_loc = np.zeros(R_pad, np.float32)
        bb_loc[:Rc] = bb_s[grows]
        bt2_c = np.ascontiguousarray(
            (-2.0 * b_loc).T.reshape(4, 128, R_pad).transpose(1, 0, 2)
        ).astype(NPBF16)
        bbbc_c = np.ascontiguousarray(
            np.broadcast_to(bb_loc[None, :], (128, R_pad)))

        lbl_loc = np.full(R_pad, -1, np.int64)
        lbl_loc[:Rc] = sl[grows]
        selP = (lbl_loc == p).astype(np.float32)
        selQ = (lbl_loc == q).astype(np.float32)
        sel3_c = np.ascontiguousarray(
            np.stack([np.ones(R_pad, np.float32), selP, selQ], 0))
        ohmy2_c = np.ascontiguousarray(np.stack([selP, selQ], 0))

        ti_c = np.zeros((nU, 128, nt2p), np.float32)
        pm_c = np.zeros((nU, 128, 128), np.float32)
        used = np.zeros((128, nt2p), bool)
        for r in range(Rc):
            gr = grows[r]
            u, cc = r // 128, r % 128
            t_ = slot_of[gr // 128]
            ti_c[u, cc, t_] = 1.0
            pm_c[u, cc, gr % 128] = 1.0
            used[gr % 128, t_] = True
        ti_c = np.ascontiguousarray(
            ti_c.transpose(1, 0, 2).reshape(128, nU * nt2p)).astype(NPBF16)
        pm_c = np.ascontiguousarray(
            pm_c.transpose(1, 0, 2).reshape(128, nU * 128)).astype(NPBF16)
        fill_c = np.where(used, 0.0, 1.0).astype(np.float32)

        valid_c = (np.arange(R_pad) < Rc).astype(np.float32).reshape(1, R_pad)
        dd = np.sum(np.square(b_loc - a_my), axis=1, dtype=np.float32)
        ddiag_c = np.sqrt(np.maximum(dd, 0.0)).reshape(1, R_pad)

        in_maps.append({
            "at": at_c, "bt2": bt2_c, "aat": aat_c, "bbbc": bbbc_c,
            "oh3": oh3_c, "oh2": oh2_c, "sel3": sel3_c, "ohmy2": ohmy2_c,
            "ti": ti_c, "pm": pm_c, "fillm": np.ascontiguousarray(fill_c),
            "validm": valid_c, "ddiag": ddiag_c,
        })
        meta.append({"grows": grows, "Rc": Rc})
    return in_maps, (R_pad, nt2, nt2p), {"order": order, "cores": meta}


def run(a, b, labels, trace=False, trace_kwargs=None):
    """Run on 8 NeuronCores; returns (loss, BassKernelResults, meta)."""
    in_maps, dims, meta = prepare_inputs(a, b, labels)
    nc = _get_nc(*dims)
    kw = {}
    if trace:
        kw = dict(trace=True, **(trace_kwargs or {}))
    res = run_bass_kernel_spmd(nc, in_maps, core_ids=list(range(NCORES)), **kw)

    counts = np.bincount(np.asarray(labels).astype(np.int64), minlength=NCLS)
    num_pos = float((counts.astype(np.float64) ** 2).sum() - N)

    total = 0.0
    for c in range(NCORES):
        r = res.results[c]
        total += (float(r["out_pos"][0, 0]) + float(r["out_pos"][1, 0])
                  - float(r["out_diag"][0, 0]))
    loss = total / (2.0 * num_pos)
    return np.asarray(np.float32(loss)), res, meta


def kernel(a, b, labels):
    loss, _, _ = run(a, b, labels)
    return loss


# revision 23
# speedup vs baseline: 2.6542x; 1.0772x over previous
"""Trainium2 Bass kernel for nn_MetricLoss (lifted-structure-style metric loss).

Reference computation (N=4096 rows, F=512 features, 16 label classes):
    Dsq = ||b_i||^2 + ||a_j||^2 - 2 b@a.T ;  D = sqrt(max(Dsq,0))   [N,N]
    Dexpm = exp(1 - D)
    row_negsum[i] = sum_{j: lbl_j != lbl_i} Dexpm[i,j]
    J = log(row_negsum[i] + row_negsum[j]) + D
    loss = sum_{i!=j, lbl_i==lbl_j} relu(J)^2 / (2 * num_pos)

v2 design — fully decoupled cores (NO collectives):
  * Rows are sorted by label on the host; label classes are PAIRED
    (largest-with-smallest) and each core owns all rows of its 2 classes
    (padded with zero-rows to a common R_pad). Every positive pair (i, j)
    then has both ns_i and ns_j computed locally, so the AllGather of
    row_negsum is gone: no inter-core dependency at all. Per-core HW exec
    time no longer includes multi-core launch-skew waits (the v1 kernel
    showed 96us on the last-launched core vs 200us+ on early cores).
  * Per-core column permutation sigma puts the core's window j-tiles
    (tiles overlapping its 2 classes) at slots 0..nt2-1, so the phase-2
    loop structure is core-independent (pure SPMD); all class masks are
    input data, not program structure.
  * negsum via 3-column one-hot matmuls per j-tile (ones/classP/classQ);
    ns = total - own-class, combined with a [3,1] +-1 matmul.
  * ns_j in partition layout (nsT[128, t]) is built with a 5-step
    transpose + per-chunk (TI mask x PM permutation-matmul) accumulation,
    all from per-core input matrices - SPMD-safe despite per-core offsets.
  * D_ii (diagonal) is host-precomputed (same O(N F) class as aa/bb).
  * ACT runs in 4 table blocks (sqrt 0-3, exp 0-3, sqrt 4-7 + ddiag,
    exp 4..7) = 3 visible table loads; exp of the LAST chunk runs first
    in its block... (actually last block order 4,5,6,7 with bylabel
    matmuls trailing each exp so only the last chunk's bylabel gates ns).
  * GEMM free dim is chunked 2x272 (R_pad=544) so matmul outputs stay
    within PSUM banks; weight loads stay at 128 (chunks share lhsT).
  * Input DMAs are spread across the sync/scalar/vector/tensor queues,
    with slot-0/1 `at` strips first so the first matmul fires early.

The GEMM runs in bf16 (b @ a.T as 256 matmuls of [128k,128j]x[128,272i]).
"""

import re
import operator
import numpy as np
import ml_dtypes
from contextlib import ExitStack

import concourse.bass as bass
import concourse.tile as tile
from concourse import bacc, mybir
from concourse import dve_ops
from concourse.dve_spec import Spec, Src0, Src1, C0, relu, sq
from concourse.bass_utils import run_bass_kernel_spmd
from concourse.tile_rust import add_dep_helper

F32 = mybir.dt.float32
BF16 = mybir.dt.bfloat16
FP8 = mybir.dt.float8e4
NPBF16 = ml_dtypes.bfloat16
NPFP8 = mybir.dt.np(mybir.dt.float8e4)
AF = mybir.ActivationFunctionType
ALU = mybir.AluOpType
DR = mybir.MatmulPerfMode.DoubleRow

N = 4096          # rows (a and b)
F = 512           # features
NCORES = 8
NT = N // 128     # j-tiles of 128 partitions = 32
NCLS = 16         # label classes
MARGIN = 1.0


def _register_sqrelu_add():
    """Custom fused DVE op: out = relu(in0 + in1)^2, accum_out = c0 + sum(out)."""
    name = "SQRELU_ADD_ANT"
    for op in dve_ops.OPS:
        if op.name == name:
            return op
    op = dve_ops.DveOp(
        name,
        Spec(body=sq(relu(Src0 + Src1)), accum=operator.add, accum_init=C0),
        subdim=False,
        uops_sha={},
    )
    dve_ops._SUB_OPCODE_FOR_NAME[name] = (
        max(dve_ops._SUB_OPCODE_FOR_NAME.values()) + 1)
    assert dve_ops._SUB_OPCODE_FOR_NAME[name] < 0x20
    for ver in ("v3", "v4"):
        try:
            op.compile(ver)
        except ValueError as e:
            m = re.search(r"\(%s: ([0-9a-f]+) " % ver, str(e))
            if not m:
                raise
            op.uops_sha[ver] = m.group(1)
            op.compile(ver)
    dve_ops.OPS.append(op)
    dve_ops.CUSTOM_DVE_SPECS[name] = op.spec
    return op


def _pin_combined_act_set(arch: str):
    """Make `natural_log_exp_and_others` the only ACT table set offering Exp
    and Ln, so Ln needs no extra load after the exp batches."""
    from concourse.hw_specs import get_activation_tables
    tabs = get_activation_tables(arch)
    assert AF.Exp in tabs["natural_log_exp_and_others"]
    assert AF.Ln in tabs["natural_log_exp_and_others"]
    for name, fns in tabs.items():
        if name != "natural_log_exp_and_others":
            fns.discard(AF.Exp)
            fns.discard(AF.Ln)


def build_bass(R_pad: int, nt2: int, nt2p: int):
    """R_pad: padded rows/core; nt2: window tiles; nt2p: dT slots (mult of 4)."""
    sqrelu_add = _register_sqrelu_add()
    CH = R_pad // 2           # psum free-dim chunk (<=512)
    assert CH <= 512
    nU = -(-R_pad // 128)     # 128-chunks of the local row range

    nc = bacc.Bacc("TRN2", target_bir_lowering=False, debug=False,
                   num_devices=NCORES)
    _pin_combined_act_set(nc.m.arch)

    # ---- kernel I/O (per-core shards prepared on host; j permuted) ----
    # at/bt2 are fp8e4m3: the -2ab cross term at fp8 perturbs the final
    # loss by ~1e-4 rel (validated offline); norm terms stay fp32 exact.
    at = nc.dram_tensor("at", [128, 4, N], FP8, kind="ExternalInput").ap()
    bt2 = nc.dram_tensor("bt2", [128, 4, R_pad], FP8, kind="ExternalInput").ap()
    aat = nc.dram_tensor("aat", [128, NT], F32, kind="ExternalInput").ap()
    bbbc = nc.dram_tensor("bbbc", [128, R_pad], F32, kind="ExternalInput").ap()
    oh3 = nc.dram_tensor("oh3", [128, NT * 3], BF16, kind="ExternalInput").ap()
    oh2 = nc.dram_tensor("oh2", [128, nt2 * 2], BF16, kind="ExternalInput").ap()
    sel3 = nc.dram_tensor("sel3", [3, R_pad], F32, kind="ExternalInput").ap()
    ohmy2 = nc.dram_tensor("ohmy2", [2, R_pad], F32, kind="ExternalInput").ap()
    ti = nc.dram_tensor("ti", [128, nU * nt2p], BF16, kind="ExternalInput").ap()
    pm = nc.dram_tensor("pm", [128, nU * 128], BF16, kind="ExternalInput").ap()
    fillm = nc.dram_tensor("fillm", [128, nt2p], F32, kind="ExternalInput").ap()
    validm = nc.dram_tensor("validm", [1, R_pad], F32, kind="ExternalInput").ap()
    ddiag = nc.dram_tensor("ddiag", [1, R_pad], F32, kind="ExternalInput").ap()

    out_pos = nc.dram_tensor("out_pos", [2, 1], F32, kind="ExternalOutput").ap()
    out_diag = nc.dram_tensor("out_diag", [1, 1], F32, kind="ExternalOutput").ap()
    out_ns = nc.dram_tensor("out_ns", [1, R_pad], F32, kind="ExternalOutput").ap()

    with tile.TileContext(nc) as tc, ExitStack() as ctx:
        sb = ctx.enter_context(tc.tile_pool(name="sb", bufs=1))
        lp = ctx.enter_context(tc.tile_pool(name="lp", bufs=5))      # Dsq chunks f32
        dp = ctx.enter_context(tc.tile_pool(name="dp", bufs=4))      # non-window D f32
        ep = ctx.enter_context(tc.tile_pool(name="ep", bufs=4))      # Dexpm bf16
        work = ctx.enter_context(tc.tile_pool(name="work", bufs=2))
        small = ctx.enter_context(tc.tile_pool(name="small", bufs=2))
        tail = ctx.enter_context(tc.tile_pool(name="tail", bufs=1))

        # tiny dummy ACT op: forces the first (sqrt) table load during DMA wait
        dummy = sb.tile([1, 8], F32)
        nc.vector.memset(dummy, 1.0)
        last_sc = nc.scalar.activation(out=dummy, in_=dummy, func=AF.Sqrt)

        def chain_sc(inst):
            # explicit scalar-queue order: keeps sqrt/exp in table batches
            nonlocal last_sc
            add_dep_helper(inst.ins, last_sc.ins, False, "scalar batch order")
            last_sc = inst

        # ---- resident SBUF tensors / DMA issue plan ----
        # scalar queue: bt2 (fp8, 139KB, lands early)
        bt_sb = sb.tile([128, 4, R_pad], FP8)
        nc.scalar.dma_start(out=bt_sb, in_=bt2)
        # gpsimd queue (own DMA engine): aat+bbbc first - the first stt
        # needs them; behind the sync queue's at stream they'd land ~19us
        aat_sb = sb.tile([128, NT], F32)
        bbbc_sb = sb.tile([128, R_pad], F32)
        nc.gpsimd.dma_start(out=aat_sb, in_=aat)
        nc.gpsimd.dma_start(out=bbbc_sb, in_=bbbc)
        # sync queue: at fully resident (fp8 = 16KB/partition), split so
        # the first slots land fast and the tail keeps ahead of the GEMM
        at_sb = sb.tile([128, 4, N], FP8)
        for lo, hi in ((0, 256), (256, 1280), (1280, 2560), (2560, N)):
            nc.sync.dma_start(out=at_sb[:, :, lo:hi], in_=at[:, :, lo:hi])

        # remaining residents ride the tensor/vector queues mid-GEMM
        oh3_sb = sb.tile([128, NT * 3], BF16)
        oh2_sb = sb.tile([128, nt2 * 2], BF16)
        sel3_sb = sb.tile([3, R_pad], F32)
        ohmy2_sb = sb.tile([2, R_pad], F32)
        ti_sb = sb.tile([128, nU * nt2p], BF16)
        pm_sb = sb.tile([128, nU * 128], BF16)
        fill_sb = sb.tile([128, nt2p], F32)
        valid_sb = sb.tile([1, R_pad], F32)
        ddiag_sb = sb.tile([1, R_pad], F32)

        dT = sb.tile([128, nt2p, R_pad], F32)      # window D
        ones128c = sb.tile([1, 128], F32)
        nc.vector.memset(ones128c, 1.0)
        ones11 = sb.tile([1, 1], F32)
        nc.vector.memset(ones11, 1.0)
        w3 = sb.tile([3, 1], F32)
        nc.vector.memset(w3, -1.0)
        nc.vector.memset(w3[0:1], 1.0)

        # ================= PHASE 1: GEMM -> +norms -> sqrt -> exp =========
        with tc.tile_pool(name="bl_ps", bufs=1, space="PSUM") as bl_pool:
            dsq_ctx = tc.tile_pool(name="dsq_ps", bufs=3, space="PSUM")
            dsq_pool = dsq_ctx.__enter__()

            bl_ps = bl_pool.tile([3, 2, 512], F32)   # negsum accumulator

            L4 = None
            pend_D = []    # (D tile, first slot, n slots) awaiting exp
            pend_E = []    # (E tile, first slot, n slots) awaiting bylabel
            nbl = 0        # bylabel slots emitted (0..NT)

            def emit_bylabel():
                nonlocal nbl
                E4, t0, ntiles = pend_E.pop(0)
                for r_ in range(ntiles):
                    t = t0 + r_
                    for c_ in range(2):
                        nc.tensor.matmul(
                            out=bl_ps[:, c_, 0:CH],
                            lhsT=oh3_sb[:, t * 3:(t + 1) * 3],
                            rhs=E4[:, r_, c_ * CH:(c_ + 1) * CH],
                            start=(nbl == 0), stop=(nbl == NT - 1))
                    nbl += 1

            def emit_exp(n=100):
                while pend_D and n > 0:
                    D4b, t0b, csz_ = pend_D.pop(0)
                    E4 = ep.tile([128, csz_, R_pad], BF16, tag="E4")
                    chain_sc(nc.scalar.activation(
                        out=E4, in_=D4b, func=AF.Exp,
                        scale=-1.0, bias=float(MARGIN)))
                    pend_E.append((E4, t0b, csz_))
                    n -= 1

            # sqrt chunk plan: two 2-slot chunks first (earlier ACT start),
            # then 4-slot chunks; table block 1 = chunks 0..4 (slots 0-15)
            chunk_sizes = [2, 2] + [4] * 7
            chunk_start = [0]
            for csz_ in chunk_sizes[:-1]:
                chunk_start.append(chunk_start[-1] + csz_)
            slot2chunk = {}
            for ci_, (cs_, csz_) in enumerate(zip(chunk_start, chunk_sizes)):
                for o_ in range(csz_):
                    slot2chunk[cs_ + o_] = (ci_, o_, csz_)
            NBLK1 = 5

            for jt in range(NT):
                dsq = dsq_pool.tile([128, 2, 512], F32, tag="dsq")
                for c_ in range(2):
                    for g_ in range(2):
                        nc.tensor.matmul(
                            out=dsq[:, c_, 0:CH],
                            lhsT=at_sb[:, 2 * g_:2 * g_ + 2,
                                       jt * 128:(jt + 1) * 128],
                            rhs=bt_sb[:, 2 * g_:2 * g_ + 2,
                                      c_ * CH:(c_ + 1) * CH],
                            start=(g_ == 0), stop=(g_ == 1), perf_mode=DR)

                # resident DMA issues ride the (otherwise idle) gpsimd queue
                if jt == 1:
                    nc.gpsimd.dma_start(out=oh3_sb, in_=oh3)
                    nc.gpsimd.dma_start(out=pm_sb, in_=pm)
                    nc.gpsimd.dma_start(out=oh2_sb, in_=oh2)
                elif jt == 3:
                    nc.gpsimd.dma_start(out=sel3_sb, in_=sel3)
                    nc.gpsimd.dma_start(out=ohmy2_sb, in_=ohmy2)
                    nc.gpsimd.dma_start(out=ti_sb, in_=ti)
                elif jt == 5:
                    nc.gpsimd.dma_start(out=fill_sb, in_=fillm)
                    nc.gpsimd.dma_start(out=valid_sb, in_=validm)
                    nc.gpsimd.dma_start(out=ddiag_sb, in_=ddiag)

                # DVE adds the norm terms: L4 = dsq + aa[j] + bb[i]
                ci_, off_, csz_ = slot2chunk[jt]
                if off_ == 0:
                    L4 = lp.tile([128, csz_, R_pad], F32, tag="L4")
                nc.vector.scalar_tensor_tensor(
                    out=L4[:, off_, :].rearrange("p (c f) -> p c f", c=2),
                    in0=dsq[:, :, 0:CH],
                    scalar=aat_sb[:, jt:jt + 1],
                    in1=bbbc_sb.rearrange("p (c f) -> p c f", c=2),
                    op0=ALU.add, op1=ALU.add)

                # interleave bylabel matmuls for block-1 exps late in GEMM
                if jt >= 26 and pend_E:
                    emit_bylabel()

                if off_ == csz_ - 1:
                    cs_ = chunk_start[ci_]
                    if cs_ + csz_ <= nt2p:
                        D4 = dT[:, cs_:cs_ + csz_, :]
                    else:
                        D4 = dp.tile([128, csz_, R_pad], F32, tag="D4")
                    chain_sc(nc.scalar.activation(out=D4, in_=L4,
                                                  func=AF.Sqrt))
                    pend_D.append((D4, cs_, csz_))
                    if ci_ == NBLK1 - 1:
                        emit_exp()     # exp chunks 0..NBLK1-1

            # sqrt chunks NBLK1.. happened above; now their exps with
            # bylabel trailing each exp so only the last chunk's bylabel
            # gates ns. The final chunk's exp is split in half so its
            # bylabel tail is ~1 us shorter.
            while pend_D:
                if len(pend_D) == 1:
                    D4b, t0b, csz_ = pend_D.pop(0)
                    h2sz = csz_ // 2
                    for h_ in range(2):
                        E2 = ep.tile([128, h2sz, R_pad], BF16, tag="E4",
                                     name=f"E2h{h_}")
                        chain_sc(nc.scalar.activation(
                            out=E2, in_=D4b[:, h_ * h2sz:(h_ + 1) * h2sz, :],
                            func=AF.Exp, scale=-1.0, bias=float(MARGIN)))
                        pend_E.append((E2, t0b + h_ * h2sz, h2sz))
                        while len(pend_E) > 1:
                            emit_bylabel()
                else:
                    emit_exp(1)
                while len(pend_E) > 1:
                    emit_bylabel()
            while pend_E:
                emit_bylabel()

            dsq_ctx.__exit__(None, None, None)   # free the 6 dsq banks

            with tc.tile_pool(name="ns_ps", bufs=1, space="PSUM") as ns_pool:
                # -- ns = total - own-class:  w3.T @ (bl * sel3) --
                prod_sb = tail.tile([3, 2, CH], F32, tag="prod3")
                nc.vector.scalar_tensor_tensor(
                    out=prod_sb, in0=bl_ps[:, :, 0:CH], scalar=0.0,
                    in1=sel3_sb.rearrange("p (c f) -> p c f", c=2),
                    op0=ALU.bypass, op1=ALU.mult)
                ns_ps = ns_pool.tile([1, 2, 512], F32, name="ns_ps")
                for c_ in range(2):
                    nc.tensor.matmul(out=ns_ps[:, c_, 0:CH], lhsT=w3,
                                     rhs=prod_sb[:, c_, :],
                                     start=True, stop=True)
                ns_my = sb.tile([1, R_pad], F32)
                nc.vector.tensor_copy(
                    out=ns_my.rearrange("p (c f) -> p c f", c=2),
                    in_=ns_ps[:, :, 0:CH])

                # -- nsT: ns_j in [128, slot] layout via transpose+perm --
                nsL_ps = ns_pool.tile([128, nU], F32, name="nsL_ps")
                for u in range(nU):
                    lo = 128 * u
                    hi = min(R_pad, lo + 128)
                    nc.tensor.matmul(out=nsL_ps[0:hi - lo, u:u + 1],
                                     lhsT=ns_my[0:1, lo:hi], rhs=ones11,
                                     start=True, stop=True)
                # broadcast ns_my across partitions (fp32 matmul; free dim is
                # small so the 4 cycles/row cost is negligible)
                nsbc_ps = ns_pool.tile([128, 2, 512], F32, name="nsbc_ps")
                for c_ in range(2):
                    nc.tensor.matmul(out=nsbc_ps[:, c_, 0:CH], lhsT=ones128c,
                                     rhs=ns_my[:, c_ * CH:(c_ + 1) * CH],
                                     start=True, stop=True)

                nsL_sb = sb.tile([128, nU], BF16)
                nc.vector.memset(nsL_sb, 0.0)
                full = (nU - 1) if R_pad % 128 else nU
                nc.vector.tensor_copy(out=nsL_sb[:, 0:full],
                                      in_=nsL_ps[:, 0:full])
                if R_pad % 128:
                    rem = R_pad % 128
                    nc.vector.tensor_copy(out=nsL_sb[0:rem, full:nU],
                                          in_=nsL_ps[0:rem, full:nU])
                nsT_ps = ns_pool.tile([128, nt2p], F32, name="nsT_ps")
                for u in range(nU):
                    rhs_u = small.tile([128, nt2p], BF16, tag="rhsu")
                    nc.vector.scalar_tensor_tensor(
                        out=rhs_u, in0=ti_sb[:, u * nt2p:(u + 1) * nt2p],
                        scalar=nsL_sb[:, u:u + 1],
                        in1=ti_sb[:, u * nt2p:(u + 1) * nt2p],
                        op0=ALU.mult, op1=ALU.bypass)
                    nc.tensor.matmul(out=nsT_ps,
                                     lhsT=pm_sb[:, u * 128:(u + 1) * 128],
                                     rhs=rhs_u,
                                     start=(u == 0), stop=(u == nU - 1))
                nsT_sb = sb.tile([128, nt2p], F32)
                nc.vector.scalar_tensor_tensor(
                    out=nsT_sb, in0=nsT_ps, scalar=0.0, in1=fill_sb,
                    op0=ALU.bypass, op1=ALU.add)
                ns_bc = sb.tile([128, R_pad], BF16)
                nc.vector.tensor_copy(
                    out=ns_bc.rearrange("p (c f) -> p c f", c=2),
                    in_=nsbc_ps[:, :, 0:CH])
                nc.sync.dma_start(out=out_ns, in_=ns_my)

        # ================= PHASE 2: J = ln(ns_i+ns_j) + D; hinge^2 =======
        with tc.tile_pool(name="hb_ps", bufs=1, space="PSUM") as hb_pool:
            hb_ps = hb_pool.tile([2, 2, 512], F32)

            for t in range(nt2):
                Lt = work.tile([128, R_pad], F32, tag="L")
                chain_sc(nc.scalar.activation(
                    out=Lt, in_=ns_bc, func=AF.Ln,
                    bias=nsT_sb[:, t:t + 1], scale=1.0))
                h2 = work.tile([128, R_pad], BF16, tag="h2")
                acc_d = small.tile([128, 1], F32, tag="accd")
                nc.vector._custom_dve(
                    sqrelu_add, out=h2, in0=Lt, in1=dT[:, t, :],
                    s0=0.0, accum_out=acc_d)
                for c_ in range(2):
                    nc.tensor.matmul(
                        out=hb_ps[:, c_, 0:CH],
                        lhsT=oh2_sb[:, t * 2:(t + 1) * 2],
                        rhs=h2[:, c_ * CH:(c_ + 1) * CH],
                        start=(t == 0), stop=(t == nt2 - 1))

            # diagonal correction relu(ln(2 ns_i) + D_ii)^2, masked by valid
            # (emitted after the phase-2 Lns so it doesn't delay Ln0; it
            # completes during the hb drain)
            lnterm = tail.tile([1, R_pad], F32, tag="lnt")
            chain_sc(nc.scalar.activation(out=lnterm, in_=ns_my, func=AF.Ln,
                                          scale=2.0))
            dh2 = tail.tile([1, R_pad], F32, tag="dh2")
            dummy_acc = small.tile([1, 1], F32, tag="dumacc")
            nc.vector._custom_dve(sqrelu_add, out=dh2, in0=lnterm,
                                  in1=ddiag_sb, s0=0.0, accum_out=dummy_acc)
            diag_acc = tail.tile([1, 1], F32, tag="dacc")
            dh2m = tail.tile([1, R_pad], F32, tag="dh2m")
            nc.vector.scalar_tensor_tensor(
                out=dh2m, in0=dh2, scalar=0.0, in1=valid_sb,
                op0=ALU.bypass, op1=ALU.mult, accum_out=diag_acc)
            nc.sync.dma_start(out=out_diag, in_=diag_acc)

            # -- combine: mask by i-side class match, accumulate --
            prod2 = tail.tile([2, 2, CH], F32, tag="prod2")
            acc2 = small.tile([2, 1], F32, tag="acc2")
            nc.vector.scalar_tensor_tensor(
                out=prod2, in0=hb_ps[:, :, 0:CH], scalar=0.0,
                in1=ohmy2_sb.rearrange("p (c f) -> p c f", c=2),
                op0=ALU.bypass, op1=ALU.mult, accum_out=acc2)
            nc.sync.dma_start(out=out_pos, in_=acc2)

    nc.compile()
    return nc


_CACHE: dict = {}


def _get_nc(R_pad: int, nt2: int, nt2p: int):
    key = ("nc", R_pad, nt2, nt2p)
    if key not in _CACHE:
        _CACHE[key] = build_bass(R_pad, nt2, nt2p)
    return _CACHE[key]


def prepare_inputs(a: np.ndarray, b: np.ndarray, labels: np.ndarray):
    """Host-side label sort, class pairing, per-core shard + mask prep.

    Returns (per-core input maps, (R_pad, nt2, nt2p), meta)."""
    a = np.asarray(a, np.float32)
    b = np.asarray(b, np.float32)
    labels = np.asarray(labels)

    order = np.argsort(labels, kind="stable")
    a_s = a[order]
    b_s = b[order]
    sl = labels[order]
    counts = np.bincount(sl.astype(np.int64), minlength=NCLS)
    startscum = np.concatenate([[0], np.cumsum(counts)])

    co = np.argsort(counts)
    pairs = [(int(co[i]), int(co[NCLS - 1 - i])) for i in range(NCORES)]
    R_pad = int(max(counts[p] + counts[q] for p, q in pairs))
    R_pad = -(-R_pad // 32) * 32
    nU = -(-R_pad // 128)

    cores = []
    nt2 = 0
    for p, q in pairs:
        grows = np.concatenate([
            np.arange(startscum[p], startscum[p + 1]),
            np.arange(startscum[q], startscum[q + 1])])
        wtiles = sorted(set((grows // 128).tolist()))
        nt2 = max(nt2, len(wtiles))
        cores.append((p, q, grows, wtiles))
    nt2p = -(-nt2 // 4) * 4

    at_full = np.ascontiguousarray(a_s.T)                  # [F, N] sorted
    aa = np.sum(a_s * a_s, axis=1, dtype=np.float32)
    bb_s = np.sum(b_s * b_s, axis=1, dtype=np.float32)

    in_maps = []
    meta = []
    for c in range(NCORES):
        p, q, grows, wtiles = cores[c]
        Rc = len(grows)
        rest = [t for t in range(NT) if t not in wtiles]
        sigma = np.array(list(wtiles) + rest)
        slot_of = {t: s_ for s_, t in enumerate(sigma)}

        cols = (sigma[:, None] * 128 + np.arange(128)[None, :]).reshape(-1)
        # [128 kpart, 4 ksub, N] fp8 for DoubleRow lhsT slices
        at_c = np.ascontiguousarray(
            at_full[:, cols].reshape(4, 128, N).transpose(1, 0, 2)
        ).astype(NPFP8)
        aat_c = np.ascontiguousarray(aa[cols].reshape(NT, 128).T)  # [128, NT]

        glbl = sl[cols].reshape(NT, 128)                   # labels per slot
        oh3_c = np.zeros((NT, 128, 3), np.float32)
        oh3_c[:, :, 0] = 1.0
        oh3_c[:, :, 1] = glbl == p
        oh3_c[:, :, 2] = glbl == q
        oh3_c = np.ascontiguousarray(
            oh3_c.transpose(1, 0, 2).reshape(128, NT * 3)).astype(NPBF16)
        oh2_c = np.zeros((nt2, 128, 2), np.float32)
        oh2_c[:, :, 0] = glbl[:nt2] == p
        oh2_c[:, :, 1] = glbl[:nt2] == q
        oh2_c = np.ascontiguousarray(
            oh2_c.transpose(1, 0, 2).reshape(128, nt2 * 2)).astype(NPBF16)

        b_loc = np.zeros((R_pad, F), np.float32)
        b_loc[:Rc] = b_s[grows]
        a_my = np.zeros((R_pad, F), np.float32)
        a_my[:Rc] = a_s[grows]
        bb_loc = np.zeros(R_pad, np.float32)
        bb_loc[:Rc] = bb_s[grows]
        bt2_c = np.ascontiguousarray(
            (-2.0 * b_loc).T.reshape(4, 128, R_pad).transpose(1, 0, 2)
        ).astype(NPFP8)
        bbbc_c = np.ascontiguousarray(
            np.broadcast_to(bb_loc[None, :], (128, R_pad)))

        lbl_loc = np.full(R_pad, -1, np.int64)
        lbl_loc[:Rc] = sl[grows]
        selP = (lbl_loc == p).astype(np.float32)
        selQ = (lbl_loc == q).astype(np.float32)
        sel3_c = np.ascontiguousarray(
            np.stack([np.ones(R_pad, np.float32), selP, selQ], 0))
        ohmy2_c = np.ascontiguousarray(np.stack([selP, selQ], 0))

        ti_c = np.zeros((nU, 128, nt2p), np.float32)
        pm_c = np.zeros((nU, 128, 128), np.float32)
        used = np.zeros((128, nt2p), bool)
        for r in range(Rc):
            gr = grows[r]
            u, cc = r // 128, r % 128
            t_ = slot_of[gr // 128]
            ti_c[u, cc, t_] = 1.0
            pm_c[u, cc, gr % 128] = 1.0
            used[gr % 128, t_] = True
        ti_c = np.ascontiguousarray(
            ti_c.transpose(1, 0, 2).reshape(128, nU * nt2p)).astype(NPBF16)
        pm_c = np.ascontiguousarray(
            pm_c.transpose(1, 0, 2).reshape(128, nU * 128)).astype(NPBF16)
        fill_c = np.where(used, 0.0, 1.0).astype(np.float32)

        valid_c = (np.arange(R_pad) < Rc).astype(np.float32).reshape(1, R_pad)
        dd = np.sum(np.square(b_loc - a_my), axis=1, dtype=np.float32)
        ddiag_c = np.sqrt(np.maximum(dd, 0.0)).reshape(1, R_pad)

        in_maps.append({
            "at": at_c, "bt2": bt2_c, "aat": aat_c, "bbbc": bbbc_c,
            "oh3": oh3_c, "oh2": oh2_c, "sel3": sel3_c, "ohmy2": ohmy2_c,
            "ti": ti_c, "pm": pm_c, "fillm": np.ascontiguousarray(fill_c),
            "validm": valid_c, "ddiag": ddiag_c,
        })
        meta.append({"grows": grows, "Rc": Rc})
    return in_maps, (R_pad, nt2, nt2p), {"order": order, "cores": meta}


def run(a, b, labels, trace=False, trace_kwargs=None):
    """Run on 8 NeuronCores; returns (loss, BassKernelResults, meta)."""
    in_maps, dims, meta = prepare_inputs(a, b, labels)
    nc = _get_nc(*dims)
    kw = {}
    if trace:
        kw = dict(trace=True, **(trace_kwargs or {}))
    res = run_bass_kernel_spmd(nc, in_maps, core_ids=list(range(NCORES)), **kw)

    counts = np.bincount(np.asarray(labels).astype(np.int64), minlength=NCLS)
    num_pos = float((counts.astype(np.float64) ** 2).sum() - N)

    total = 0.0
    for c in range(NCORES):
        r = res.results[c]
        total += (float(r["out_pos"][0, 0]) + float(r["out_pos"][1, 0])
                  - float(r["out_diag"][0, 0]))
    loss = total / (2.0 * num_pos)
    return np.asarray(np.float32(loss)), res, meta


def kernel(a, b, labels):
    loss, _, _ = run(a, b, labels)
    return loss


# revision 30
# speedup vs baseline: 2.7713x; 1.0441x over previous
"""Trainium2 Bass kernel for nn_MetricLoss (lifted-structure-style metric loss).

Reference computation (N=4096 rows, F=512 features, 16 label classes):
    Dsq = ||b_i||^2 + ||a_j||^2 - 2 b@a.T ;  D = sqrt(max(Dsq,0))   [N,N]
    Dexpm = exp(1 - D)
    row_negsum[i] = sum_{j: lbl_j != lbl_i} Dexpm[i,j]
    J = log(row_negsum[i] + row_negsum[j]) + D
    loss = sum_{i!=j, lbl_i==lbl_j} relu(J)^2 / (2 * num_pos)

v2 design — fully decoupled cores (NO collectives):
  * Rows are sorted by label on the host; label classes are PAIRED
    (largest-with-smallest) and each core owns all rows of its 2 classes
    (padded with zero-rows to a common R_pad). Every positive pair (i, j)
    then has both ns_i and ns_j computed locally, so the AllGather of
    row_negsum is gone: no inter-core dependency at all. Per-core HW exec
    time no longer includes multi-core launch-skew waits (the v1 kernel
    showed 96us on the last-launched core vs 200us+ on early cores).
  * Per-core column permutation sigma puts the core's window j-tiles
    (tiles overlapping its 2 classes) at slots 0..nt2-1, so the phase-2
    loop structure is core-independent (pure SPMD); all class masks are
    input data, not program structure.
  * negsum via 3-column one-hot matmuls per j-tile (ones/classP/classQ);
    ns = total - own-class, combined with a [3,1] +-1 matmul.
  * ns_j in partition layout (nsT[128, t]) is built with a 5-step
    transpose + per-chunk (TI mask x PM permutation-matmul) accumulation,
    all from per-core input matrices - SPMD-safe despite per-core offsets.
  * D_ii (diagonal) is host-precomputed (same O(N F) class as aa/bb).
  * ACT runs in 4 table blocks (sqrt 0-3, exp 0-3, sqrt 4-7 + ddiag,
    exp 4..7) = 3 visible table loads; exp of the LAST chunk runs first
    in its block... (actually last block order 4,5,6,7 with bylabel
    matmuls trailing each exp so only the last chunk's bylabel gates ns).
  * GEMM free dim is chunked 2x272 (R_pad=544) so matmul outputs stay
    within PSUM banks; weight loads stay at 128 (chunks share lhsT).
  * Input DMAs are spread across the sync/scalar/vector/tensor queues,
    with slot-0/1 `at` strips first so the first matmul fires early.

The GEMM runs in bf16 (b @ a.T as 256 matmuls of [128k,128j]x[128,272i]).
"""

import re
import operator
import numpy as np
import ml_dtypes
from contextlib import ExitStack

import concourse.bass as bass
import concourse.tile as tile
from concourse import bacc, mybir
from concourse import dve_ops
from concourse.dve_spec import Spec, Src0, Src1, C0, relu, sq
from concourse.bass_utils import run_bass_kernel_spmd
from concourse.tile_rust import add_dep_helper

F32 = mybir.dt.float32
BF16 = mybir.dt.bfloat16
FP8 = mybir.dt.float8e4
NPBF16 = ml_dtypes.bfloat16
NPFP8 = mybir.dt.np(mybir.dt.float8e4)
AF = mybir.ActivationFunctionType
ALU = mybir.AluOpType
DR = mybir.MatmulPerfMode.DoubleRow

N = 4096          # rows (a and b)
F = 512           # features
NCORES = 8
NT = N // 128     # j-tiles of 128 partitions = 32
NCLS = 16         # label classes
MARGIN = 1.0


def _register_sqrelu_add():
    """Custom fused DVE op: out = relu(in0 + in1)^2, accum_out = c0 + sum(out)."""
    name = "SQRELU_ADD_ANT"
    for op in dve_ops.OPS:
        if op.name == name:
            return op
    op = dve_ops.DveOp(
        name,
        Spec(body=sq(relu(Src0 + Src1)), accum=operator.add, accum_init=C0),
        subdim=False,
        uops_sha={},
    )
    dve_ops._SUB_OPCODE_FOR_NAME[name] = (
        max(dve_ops._SUB_OPCODE_FOR_NAME.values()) + 1)
    assert dve_ops._SUB_OPCODE_FOR_NAME[name] < 0x20
    for ver in ("v3", "v4"):
        try:
            op.compile(ver)
        except ValueError as e:
            m = re.search(r"\(%s: ([0-9a-f]+) " % ver, str(e))
            if not m:
                raise
            op.uops_sha[ver] = m.group(1)
            op.compile(ver)
    dve_ops.OPS.append(op)
    dve_ops.CUSTOM_DVE_SPECS[name] = op.spec
    return op


def _pin_combined_act_set(arch: str):
    """Make `natural_log_exp_and_others` the only ACT table set offering Exp
    and Ln, so Ln needs no extra load after the exp batches."""
    from concourse.hw_specs import get_activation_tables
    tabs = get_activation_tables(arch)
    assert AF.Exp in tabs["natural_log_exp_and_others"]
    assert AF.Ln in tabs["natural_log_exp_and_others"]
    for name, fns in tabs.items():
        if name != "natural_log_exp_and_others":
            fns.discard(AF.Exp)
            fns.discard(AF.Ln)


def build_bass(R_pad: int, nt2: int, nt2p: int):
    """R_pad: padded rows/core; nt2: window tiles; nt2p: dT slots (mult of 4)."""
    sqrelu_add = _register_sqrelu_add()
    CH = R_pad // 2           # psum free-dim chunk (<=512)
    assert CH <= 512
    nU = -(-R_pad // 128)     # 128-chunks of the local row range

    nc = bacc.Bacc("TRN2", target_bir_lowering=False, debug=False,
                   num_devices=NCORES)
    _pin_combined_act_set(nc.m.arch)

    # ---- kernel I/O (per-core shards prepared on host; j permuted) ----
    # at/bt2 are fp8e4m3: the -2ab cross term at fp8 perturbs the final
    # loss by ~1e-4 rel (validated offline); norm terms stay fp32 exact.
    at = nc.dram_tensor("at", [128, 4, N], FP8, kind="ExternalInput").ap()
    bt2 = nc.dram_tensor("bt2", [128, 4, R_pad], FP8, kind="ExternalInput").ap()
    aat = nc.dram_tensor("aat", [128, NT], F32, kind="ExternalInput").ap()
    bbbc = nc.dram_tensor("bbbc", [128, R_pad], F32, kind="ExternalInput").ap()
    oh3 = nc.dram_tensor("oh3", [128, NT * 3], BF16, kind="ExternalInput").ap()
    oh2 = nc.dram_tensor("oh2", [128, nt2 * 2], BF16, kind="ExternalInput").ap()
    sel3 = nc.dram_tensor("sel3", [3, R_pad], F32, kind="ExternalInput").ap()
    ohmy2 = nc.dram_tensor("ohmy2", [2, R_pad], F32, kind="ExternalInput").ap()
    ti = nc.dram_tensor("ti", [128, nU * nt2p], BF16, kind="ExternalInput").ap()
    pm = nc.dram_tensor("pm", [128, nU * 128], BF16, kind="ExternalInput").ap()
    fillm = nc.dram_tensor("fillm", [128, nt2p], F32, kind="ExternalInput").ap()
    validm = nc.dram_tensor("validm", [1, R_pad], F32, kind="ExternalInput").ap()
    ddiag = nc.dram_tensor("ddiag", [1, R_pad], F32, kind="ExternalInput").ap()

    out_pos = nc.dram_tensor("out_pos", [2, 1], F32, kind="ExternalOutput").ap()
    out_diag = nc.dram_tensor("out_diag", [1, 1], F32, kind="ExternalOutput").ap()
    out_ns = nc.dram_tensor("out_ns", [1, R_pad], F32, kind="ExternalOutput").ap()

    with tile.TileContext(nc) as tc, ExitStack() as ctx:
        sb = ctx.enter_context(tc.tile_pool(name="sb", bufs=1))
        lp = ctx.enter_context(tc.tile_pool(name="lp", bufs=4))      # Dsq chunks f32
        dp = ctx.enter_context(tc.tile_pool(name="dp", bufs=6))      # non-window D f32
        ep = ctx.enter_context(tc.tile_pool(name="ep", bufs=4))      # Dexpm bf16
        work = ctx.enter_context(tc.tile_pool(name="work", bufs=2))
        small = ctx.enter_context(tc.tile_pool(name="small", bufs=2))
        tail = ctx.enter_context(tc.tile_pool(name="tail", bufs=1))

        # tiny dummy ACT op: forces the first (sqrt) table load during DMA wait
        dummy = sb.tile([1, 8], F32)
        nc.vector.memset(dummy, 1.0)
        last_sc = nc.scalar.activation(out=dummy, in_=dummy, func=AF.Sqrt)

        def chain_sc(inst):
            # explicit scalar-queue order: keeps sqrt/exp in table batches
            nonlocal last_sc
            add_dep_helper(inst.ins, last_sc.ins, False, "scalar batch order")
            last_sc = inst

        # ---- resident SBUF tensors / DMA issue plan ----
        # scalar queue: bt2 (fp8, 139KB, lands early)
        bt_sb = sb.tile([128, 4, R_pad], FP8)
        nc.scalar.dma_start(out=bt_sb, in_=bt2)
        # gpsimd queue (own DMA engine): aat+bbbc first - the first stt
        # needs them; behind the sync queue's at stream they'd land ~19us
        aat_sb = sb.tile([128, NT], F32)
        bbbc_sb = sb.tile([128, R_pad], F32)
        nc.gpsimd.dma_start(out=aat_sb, in_=aat)
        nc.gpsimd.dma_start(out=bbbc_sb, in_=bbbc)
        # sync queue: at fully resident (fp8 = 16KB/partition), split so
        # the first slots land fast and the tail keeps ahead of the GEMM
        at_sb = sb.tile([128, 4, N], FP8)
        for lo, hi in ((0, 256), (256, 1280), (1280, 2560), (2560, N)):
            nc.sync.dma_start(out=at_sb[:, :, lo:hi], in_=at[:, :, lo:hi])

        # remaining residents ride the tensor/vector queues mid-GEMM
        oh3_sb = sb.tile([128, NT * 3], BF16)
        oh2_sb = sb.tile([128, nt2 * 2], BF16)
        sel3_sb = sb.tile([3, R_pad], F32)
        ohmy2_sb = sb.tile([2, R_pad], F32)
        ti_sb = sb.tile([128, nU * nt2p], BF16)
        pm_sb = sb.tile([128, nU * 128], BF16)
        fill_sb = sb.tile([128, nt2p], F32)
        valid_sb = sb.tile([1, R_pad], F32)
        ddiag_sb = sb.tile([1, R_pad], F32)

        dT = sb.tile([128, nt2p, R_pad], F32)      # window D
        ones128c = sb.tile([1, 128], BF16)
        nc.vector.memset(ones128c, 1.0)
        ones11 = sb.tile([1, 1], F32)
        nc.vector.memset(ones11, 1.0)
        w3 = sb.tile([3, 1], BF16)
        nc.vector.memset(w3, -1.0)
        nc.vector.memset(w3[0:1], 1.0)

        # ================= PHASE 1: GEMM -> +norms -> sqrt -> exp =========
        with tc.tile_pool(name="bl_ps", bufs=1, space="PSUM") as bl_pool:
            dsq_ctx = tc.tile_pool(name="dsq_ps", bufs=3, space="PSUM")
            dsq_pool = dsq_ctx.__enter__()

            bl_ps = bl_pool.tile([3, 2, 512], F32)   # negsum accumulator

            L4 = None
            pend_D = []    # (D tile, first slot, n slots) awaiting exp
            pend_E = []    # (E tile, first slot, n slots) awaiting bylabel
            nbl = 0        # bylabel slots emitted (0..NT)

            def emit_bylabel():
                nonlocal nbl
                E4, t0, ntiles = pend_E.pop(0)
                for r_ in range(ntiles):
                    t = t0 + r_
                    for c_ in range(2):
                        nc.tensor.matmul(
                            out=bl_ps[:, c_, 0:CH],
                            lhsT=oh3_sb[:, t * 3:(t + 1) * 3],
                            rhs=E4[:, r_, c_ * CH:(c_ + 1) * CH],
                            start=(nbl == 0), stop=(nbl == NT - 1))
                    nbl += 1

            def emit_exp(n=100):
                while pend_D and n > 0:
                    D4b, t0b, csz_ = pend_D.pop(0)
                    E4 = ep.tile([128, csz_, R_pad], BF16, tag="E4")
                    chain_sc(nc.scalar.activation(
                        out=E4, in_=D4b, func=AF.Exp,
                        scale=-1.0, bias=float(MARGIN)))
                    pend_E.append((E4, t0b, csz_))
                    n -= 1

            # sqrt chunk plan: two 2-slot chunks first (earlier ACT start),
            # then 4-slot chunks; table block 1 = chunks 0..4 (slots 0-15)
            chunk_sizes = [2, 2] + [4] * 7
            chunk_start = [0]
            for csz_ in chunk_sizes[:-1]:
                chunk_start.append(chunk_start[-1] + csz_)
            slot2chunk = {}
            for ci_, (cs_, csz_) in enumerate(zip(chunk_start, chunk_sizes)):
                for o_ in range(csz_):
                    slot2chunk[cs_ + o_] = (ci_, o_, csz_)
            NBLK1 = 3

            for jt in range(NT):
                dsq = dsq_pool.tile([128, 2, 512], F32, tag="dsq")
                for c_ in range(2):
                    for g_ in range(2):
                        nc.tensor.matmul(
                            out=dsq[:, c_, 0:CH],
                            lhsT=at_sb[:, 2 * g_:2 * g_ + 2,
                                       jt * 128:(jt + 1) * 128],
                            rhs=bt_sb[:, 2 * g_:2 * g_ + 2,
                                      c_ * CH:(c_ + 1) * CH],
                            start=(g_ == 0), stop=(g_ == 1), perf_mode=DR)

                # resident DMA issues ride the (otherwise idle) gpsimd queue
                if jt == 1:
                    nc.gpsimd.dma_start(out=oh3_sb, in_=oh3)
                    nc.gpsimd.dma_start(out=pm_sb, in_=pm)
                    nc.gpsimd.dma_start(out=oh2_sb, in_=oh2)
                elif jt == 3:
                    nc.gpsimd.dma_start(out=sel3_sb, in_=sel3)
                    nc.gpsimd.dma_start(out=ohmy2_sb, in_=ohmy2)
                    nc.gpsimd.dma_start(out=ti_sb, in_=ti)
                elif jt == 5:
                    nc.gpsimd.dma_start(out=fill_sb, in_=fillm)
                    nc.gpsimd.dma_start(out=valid_sb, in_=validm)
                    nc.gpsimd.dma_start(out=ddiag_sb, in_=ddiag)

                # DVE adds the norm terms: L4 = dsq + aa[j] + bb[i]
                ci_, off_, csz_ = slot2chunk[jt]
                if off_ == 0:
                    L4 = lp.tile([128, csz_, R_pad], F32, tag="L4")
                nc.vector.scalar_tensor_tensor(
                    out=L4[:, off_, :].rearrange("p (c f) -> p c f", c=2),
                    in0=dsq[:, :, 0:CH],
                    scalar=aat_sb[:, jt:jt + 1],
                    in1=bbbc_sb.rearrange("p (c f) -> p c f", c=2),
                    op0=ALU.add, op1=ALU.add)

                # interleave bylabel matmuls for block-1 exps late in GEMM
                if jt >= 21 and pend_E:
                    emit_bylabel()

                if off_ == csz_ - 1:
                    cs_ = chunk_start[ci_]
                    if cs_ + csz_ <= nt2p:
                        D4 = dT[:, cs_:cs_ + csz_, :]
                    else:
                        D4 = dp.tile([128, csz_, R_pad], F32, tag="D4")
                    chain_sc(nc.scalar.activation(out=D4, in_=L4,
                                                  func=AF.Sqrt))
                    pend_D.append((D4, cs_, csz_))
                    if ci_ == NBLK1 - 1:
                        emit_exp()     # exp chunks 0..NBLK1-1

            # sqrt chunks NBLK1.. happened above; now their exps with
            # bylabel trailing each exp so only the last chunk's bylabel
            # gates ns. The final chunk's exp is split in half so its
            # bylabel tail is ~1 us shorter.
            while pend_D:
                if len(pend_D) == 1:
                    D4b, t0b, csz_ = pend_D.pop(0)
                    h2sz = csz_ // 2
                    for h_ in range(2):
                        E2 = ep.tile([128, h2sz, R_pad], BF16, tag="E4",
                                     name=f"E2h{h_}")
                        chain_sc(nc.scalar.activation(
                            out=E2, in_=D4b[:, h_ * h2sz:(h_ + 1) * h2sz, :],
                            func=AF.Exp, scale=-1.0, bias=float(MARGIN)))
                        pend_E.append((E2, t0b + h_ * h2sz, h2sz))
                        while len(pend_E) > 1:
                            emit_bylabel()
                else:
                    emit_exp(1)
                while len(pend_E) > 1:
                    emit_bylabel()
            while pend_E:
                emit_bylabel()

            dsq_ctx.__exit__(None, None, None)   # free the 6 dsq banks

            with tc.tile_pool(name="ns_ps", bufs=1, space="PSUM") as ns_pool:
                # -- ns = total - own-class:  w3.T @ (bl * sel3) --
                prod_sb = tail.tile([3, 2, CH], BF16, tag="prod3")
                nc.vector.scalar_tensor_tensor(
                    out=prod_sb, in0=bl_ps[:, :, 0:CH], scalar=0.0,
                    in1=sel3_sb.rearrange("p (c f) -> p c f", c=2),
                    op0=ALU.bypass, op1=ALU.mult)
                ns_ps = ns_pool.tile([1, 2, 512], F32, name="ns_ps")
                for c_ in range(2):
                    nc.tensor.matmul(out=ns_ps[:, c_, 0:CH], lhsT=w3,
                                     rhs=prod_sb[:, c_, :],
                                     start=True, stop=True)
                ns_my = sb.tile([1, R_pad], F32)
                nc.vector.tensor_copy(
                    out=ns_my.rearrange("p (c f) -> p c f", c=2),
                    in_=ns_ps[:, :, 0:CH])
                ns_bf = sb.tile([1, R_pad], BF16)
                nc.vector.tensor_copy(out=ns_bf, in_=ns_my)

                # -- nsT: ns_j in [128, slot] layout via transpose+perm --
                nsL_ps = ns_pool.tile([128, nU], F32, name="nsL_ps")
                for u in range(nU):
                    lo = 128 * u
                    hi = min(R_pad, lo + 128)
                    nc.tensor.matmul(out=nsL_ps[0:hi - lo, u:u + 1],
                                     lhsT=ns_my[0:1, lo:hi], rhs=ones11,
                                     start=True, stop=True)
                # broadcast ns_my across partitions: [128, R_pad]
                nsbc_ps = ns_pool.tile([128, 2, 512], F32, name="nsbc_ps")
                for c_ in range(2):
                    nc.tensor.matmul(out=nsbc_ps[:, c_, 0:CH], lhsT=ones128c,
                                     rhs=ns_bf[:, c_ * CH:(c_ + 1) * CH],
                                     start=True, stop=True)

                # rhs_u reads nsL straight from PSUM as the per-partition
                # scalar (garbage partitions are masked by ti=0, and psum
                # holds only finite floats)
                nsT_ps = ns_pool.tile([128, nt2p], F32, name="nsT_ps")
                for u in range(nU):
                    rhs_u = small.tile([128, nt2p], BF16, tag="rhsu")
                    nc.vector.scalar_tensor_tensor(
                        out=rhs_u, in0=ti_sb[:, u * nt2p:(u + 1) * nt2p],
                        scalar=nsL_ps[:, u:u + 1],
                        in1=ti_sb[:, u * nt2p:(u + 1) * nt2p],
                        op0=ALU.mult, op1=ALU.bypass)
                    nc.tensor.matmul(out=nsT_ps,
                                     lhsT=pm_sb[:, u * 128:(u + 1) * 128],
                                     rhs=rhs_u,
                                     start=(u == 0), stop=(u == nU - 1))
                nsT_sb = sb.tile([128, nt2p], F32)
                nc.vector.scalar_tensor_tensor(
                    out=nsT_sb, in0=nsT_ps, scalar=0.0, in1=fill_sb,
                    op0=ALU.bypass, op1=ALU.add)
                ns_bc = sb.tile([128, R_pad], BF16)
                nc.vector.tensor_copy(
                    out=ns_bc.rearrange("p (c f) -> p c f", c=2),
                    in_=nsbc_ps[:, :, 0:CH])
                nc.sync.dma_start(out=out_ns, in_=ns_my)

        # ================= PHASE 2: J = ln(ns_i+ns_j) + D; hinge^2 =======
        with tc.tile_pool(name="hb_ps", bufs=1, space="PSUM") as hb_pool:
            hb_ps = hb_pool.tile([2, 2, 512], F32)

            for t in range(nt2):
                Lt = work.tile([128, R_pad], F32, tag="L")
                chain_sc(nc.scalar.activation(
                    out=Lt, in_=ns_bc, func=AF.Ln,
                    bias=nsT_sb[:, t:t + 1], scale=1.0))
                h2 = work.tile([128, R_pad], BF16, tag="h2")
                acc_d = small.tile([128, 1], F32, tag="accd")
                nc.vector._custom_dve(
                    sqrelu_add, out=h2, in0=Lt, in1=dT[:, t, :],
                    s0=0.0, accum_out=acc_d)
                for c_ in range(2):
                    nc.tensor.matmul(
                        out=hb_ps[:, c_, 0:CH],
                        lhsT=oh2_sb[:, t * 2:(t + 1) * 2],
                        rhs=h2[:, c_ * CH:(c_ + 1) * CH],
                        start=(t == 0), stop=(t == nt2 - 1))

            # diagonal correction relu(ln(2 ns_i) + D_ii)^2, masked by valid
            # (emitted after the phase-2 Lns so it doesn't delay Ln0; it
            # completes during the hb drain)
            lnterm = tail.tile([1, R_pad], F32, tag="lnt")
            chain_sc(nc.scalar.activation(out=lnterm, in_=ns_my, func=AF.Ln,
                                          scale=2.0))
            dh2 = tail.tile([1, R_pad], F32, tag="dh2")
            dummy_acc = small.tile([1, 1], F32, tag="dumacc")
            nc.vector._custom_dve(sqrelu_add, out=dh2, in0=lnterm,
                                  in1=ddiag_sb, s0=0.0, accum_out=dummy_acc)
            diag_acc = tail.tile([1, 1], F32, tag="dacc")
            dh2m = tail.tile([1, R_pad], F32, tag="dh2m")
            nc.vector.scalar_tensor_tensor(
                out=dh2m, in0=dh2, scalar=0.0, in1=valid_sb,
                op0=ALU.bypass, op1=ALU.mult, accum_out=diag_acc)
            nc.sync.dma_start(out=out_diag, in_=diag_acc)

            # -- combine: mask by i-side class match, accumulate --
            prod2 = tail.tile([2, 2, CH], F32, tag="prod2")
            acc2 = small.tile([2, 1], F32, tag="acc2")
            nc.vector.scalar_tensor_tensor(
                out=prod2, in0=hb_ps[:, :, 0:CH], scalar=0.0,
                in1=ohmy2_sb.rearrange("p (c f) -> p c f", c=2),
                op0=ALU.bypass, op1=ALU.mult, accum_out=acc2)
            nc.sync.dma_start(out=out_pos, in_=acc2)

    nc.compile()
    return nc


_CACHE: dict = {}


def _get_nc(R_pad: int, nt2: int, nt2p: int):
    key = ("nc", R_pad, nt2, nt2p)
    if key not in _CACHE:
        _CACHE[key] = build_bass(R_pad, nt2, nt2p)
    return _CACHE[key]


def prepare_inputs(a: np.ndarray, b: np.ndarray, labels: np.ndarray):
    """Host-side label sort, class pairing, per-core shard + mask prep.

    Returns (per-core input maps, (R_pad, nt2, nt2p), meta)."""
    a = np.asarray(a, np.float32)
    b = np.asarray(b, np.float32)
    labels = np.asarray(labels)

    order = np.argsort(labels, kind="stable")
    a_s = a[order]
    b_s = b[order]
    sl = labels[order]
    counts = np.bincount(sl.astype(np.int64), minlength=NCLS)
    startscum = np.concatenate([[0], np.cumsum(counts)])

    def pair_tiles(p, q):
        ta = set(range(int(startscum[p]) // 128,
                       -(-int(startscum[p + 1]) // 128)))
        tb = set(range(int(startscum[q]) // 128,
                       -(-int(startscum[q + 1]) // 128)))
        return len(ta | tb)

    def pairing_cost(pairs_):
        return (max(int(counts[p] + counts[q]) for p, q in pairs_),
                max(pair_tiles(p, q) for p, q in pairs_))

    # greedy largest-with-smallest, then 2-opt swaps minimizing
    # (max pair size, max window tiles) lexicographically
    co = np.argsort(counts)
    pairs = [(int(co[i]), int(co[NCLS - 1 - i])) for i in range(NCORES)]
    best = pairing_cost(pairs)
    improved = True
    while improved:
        improved = False
        for i in range(NCORES):
            for j in range(i + 1, NCORES):
                for swap in ((0, 0), (0, 1)):
                    cand = list(pairs)
                    a1, b1 = pairs[i]
                    a2, b2 = pairs[j]
                    if swap == (0, 0):
                        cand[i], cand[j] = (a2, b1), (a1, b2)
                    else:
                        cand[i], cand[j] = (b2, b1), (a2, a1)
                    c = pairing_cost(cand)
                    if c < best:
                        pairs, best, improved = cand, c, True
    R_pad = best[0]
    R_pad = -(-R_pad // 32) * 32
    nU = -(-R_pad // 128)

    cores = []
    nt2 = 0
    for p, q in pairs:
        grows = np.concatenate([
            np.arange(startscum[p], startscum[p + 1]),
            np.arange(startscum[q], startscum[q + 1])])
        wtiles = sorted(set((grows // 128).tolist()))
        nt2 = max(nt2, len(wtiles))
        cores.append((p, q, grows, wtiles))
    nt2p = -(-nt2 // 4) * 4

    at_full = np.ascontiguousarray(a_s.T)                  # [F, N] sorted
    aa = np.sum(a_s * a_s, axis=1, dtype=np.float32)
    bb_s = np.sum(b_s * b_s, axis=1, dtype=np.float32)

    in_maps = []
    meta = []
    for c in range(NCORES):
        p, q, grows, wtiles = cores[c]
        Rc = len(grows)
        rest = [t for t in range(NT) if t not in wtiles]
        sigma = np.array(list(wtiles) + rest)
        slot_of = {t: s_ for s_, t in enumerate(sigma)}

        cols = (sigma[:, None] * 128 + np.arange(128)[None, :]).reshape(-1)
        # [128 kpart, 4 ksub, N] fp8 for DoubleRow lhsT slices
        at_c = np.ascontiguousarray(
            at_full[:, cols].reshape(4, 128, N).transpose(1, 0, 2)
        ).astype(NPFP8)
        aat_c = np.ascontiguousarray(aa[cols].reshape(NT, 128).T)  # [128, NT]

        glbl = sl[cols].reshape(NT, 128)                   # labels per slot
        oh3_c = np.zeros((NT, 128, 3), np.float32)
        oh3_c[:, :, 0] = 1.0
        oh3_c[:, :, 1] = glbl == p
        oh3_c[:, :, 2] = glbl == q
        oh3_c = np.ascontiguousarray(
            oh3_c.transpose(1, 0, 2).reshape(128, NT * 3)).astype(NPBF16)
        oh2_c = np.zeros((nt2, 128, 2), np.float32)
        oh2_c[:, :, 0] = glbl[:nt2] == p
        oh2_c[:, :, 1] = glbl[:nt2] == q
        oh2_c = np.ascontiguousarray(
            oh2_c.transpose(1, 0, 2).reshape(128, nt2 * 2)).astype(NPBF16)

        b_loc = np.zeros((R_pad, F), np.float32)
        b_loc[:Rc] = b_s[grows]
        a_my = np.zeros((R_pad, F), np.float32)
        a_my[:Rc] = a_s[grows]
        bb_loc = np.zeros(R_pad, np.float32)
        bb_loc[:Rc] = bb_s[grows]
        bt2_c = np.ascontiguousarray(
            (-2.0 * b_loc).T.reshape(4, 128, R_pad).transpose(1, 0, 2)
        ).astype(NPFP8)
        bbbc_c = np.ascontiguousarray(
            np.broadcast_to(bb_loc[None, :], (128, R_pad)))

        lbl_loc = np.full(R_pad, -1, np.int64)
        lbl_loc[:Rc] = sl[grows]
        selP = (lbl_loc == p).astype(np.float32)
        selQ = (lbl_loc == q).astype(np.float32)
        sel3_c = np.ascontiguousarray(
            np.stack([np.ones(R_pad, np.float32), selP, selQ], 0))
        ohmy2_c = np.ascontiguousarray(np.stack([selP, selQ], 0))

        ti_c = np.zeros((nU, 128, nt2p), np.float32)
        pm_c = np.zeros((nU, 128, 128), np.float32)
        used = np.zeros((128, nt2p), bool)
        for r in range(Rc):
            gr = grows[r]
            u, cc = r // 128, r % 128
            t_ = slot_of[gr // 128]
            ti_c[u, cc, t_] = 1.0
            pm_c[u, cc, gr % 128] = 1.0
            used[gr % 128, t_] = True
        ti_c = np.ascontiguousarray(
            ti_c.transpose(1, 0, 2).reshape(128, nU * nt2p)).astype(NPBF16)
        pm_c = np.ascontiguousarray(
            pm_c.transpose(1, 0, 2).reshape(128, nU * 128)).astype(NPBF16)
        fill_c = np.where(used, 0.0, 1.0).astype(np.float32)

        valid_c = (np.arange(R_pad) < Rc).astype(np.float32).reshape(1, R_pad)
        dd = np.sum(np.square(b_loc - a_my), axis=1, dtype=np.float32)
        ddiag_c = np.sqrt(np.maximum(dd, 0.0)).reshape(1, R_pad)

        in_maps.append({
            "at": at_c, "bt2": bt2_c, "aat": aat_c, "bbbc": bbbc_c,
            "oh3": oh3_c, "oh2": oh2_c, "sel3": sel3_c, "ohmy2": ohmy2_c,
            "ti": ti_c, "pm": pm_c, "fillm": np.ascontiguousarray(fill_c),
            "validm": valid_c, "ddiag": ddiag_c,
        })
        meta.append({"grows": grows, "Rc": Rc})
    return in_maps, (R_pad, nt2, nt2p), {"order": order, "cores": meta}


def run(a, b, labels, trace=False, trace_kwargs=None):
    """Run on 8 NeuronCores; returns (loss, BassKernelResults, meta)."""
    in_maps, dims, meta = prepare_inputs(a, b, labels)
    nc = _get_nc(*dims)
    kw = {}
    if trace:
        kw = dict(trace=True, **(trace_kwargs or {}))
    res = run_bass_kernel_spmd(nc, in_maps, core_ids=list(range(NCORES)), **kw)

    counts = np.bincount(np.asarray(labels).astype(np.int64), minlength=NCLS)
    num_pos = float((counts.astype(np.float64) ** 2).sum() - N)

    total = 0.0
    for c in range(NCORES):
        r = res.results[c]
        total += (float(r["out_pos"][0, 0]) + float(r["out_pos"][1, 0])
                  - float(r["out_diag"][0, 0]))
    loss = total / (2.0 * num_pos)
    return np.asarray(np.float32(loss)), res, meta


def kernel(a, b, labels):
    loss, _, _ = run(a, b, labels)
    return loss


# revision 38
# speedup vs baseline: 2.8271x; 1.0201x over previous
"""Trainium2 Bass kernel for nn_MetricLoss (lifted-structure-style metric loss).

Reference computation (N=4096 rows, F=512 features, 16 label classes):
    Dsq = ||b_i||^2 + ||a_j||^2 - 2 b@a.T ;  D = sqrt(max(Dsq,0))   [N,N]
    Dexpm = exp(1 - D)
    row_negsum[i] = sum_{j: lbl_j != lbl_i} Dexpm[i,j]
    J = log(row_negsum[i] + row_negsum[j]) + D
    loss = sum_{i!=j, lbl_i==lbl_j} relu(J)^2 / (2 * num_pos)

v2 design — fully decoupled cores (NO collectives):
  * Rows are sorted by label on the host; label classes are PAIRED
    (largest-with-smallest) and each core owns all rows of its 2 classes
    (padded with zero-rows to a common R_pad). Every positive pair (i, j)
    then has both ns_i and ns_j computed locally, so the AllGather of
    row_negsum is gone: no inter-core dependency at all. Per-core HW exec
    time no longer includes multi-core launch-skew waits (the v1 kernel
    showed 96us on the last-launched core vs 200us+ on early cores).
  * Per-core column permutation sigma puts the core's window j-tiles
    (tiles overlapping its 2 classes) at slots 0..nt2-1, so the phase-2
    loop structure is core-independent (pure SPMD); all class masks are
    input data, not program structure.
  * negsum via 3-column one-hot matmuls per j-tile (ones/classP/classQ);
    ns = total - own-class, combined with a [3,1] +-1 matmul.
  * ns_j in partition layout (nsT[128, t]) is built with a 5-step
    transpose + per-chunk (TI mask x PM permutation-matmul) accumulation,
    all from per-core input matrices - SPMD-safe despite per-core offsets.
  * D_ii (diagonal) is host-precomputed (same O(N F) class as aa/bb).
  * ACT runs in 4 table blocks (sqrt 0-3, exp 0-3, sqrt 4-7 + ddiag,
    exp 4..7) = 3 visible table loads; exp of the LAST chunk runs first
    in its block... (actually last block order 4,5,6,7 with bylabel
    matmuls trailing each exp so only the last chunk's bylabel gates ns).
  * GEMM free dim is chunked 2x272 (R_pad=544) so matmul outputs stay
    within PSUM banks; weight loads stay at 128 (chunks share lhsT).
  * Input DMAs are spread across the sync/scalar/vector/tensor queues,
    with slot-0/1 `at` strips first so the first matmul fires early.

The GEMM runs in bf16 (b @ a.T as 256 matmuls of [128k,128j]x[128,272i]).
"""

import re
import operator
import numpy as np
import ml_dtypes
from contextlib import ExitStack

import concourse.bass as bass
import concourse.tile as tile
from concourse import bacc, mybir
from concourse import dve_ops
from concourse.dve_spec import Spec, Src0, Src1, C0, relu, sq
from concourse.bass_utils import run_bass_kernel_spmd
from concourse.tile_rust import add_dep_helper

F32 = mybir.dt.float32
BF16 = mybir.dt.bfloat16
FP8 = mybir.dt.float8e4
NPBF16 = ml_dtypes.bfloat16
NPFP8 = mybir.dt.np(mybir.dt.float8e4)
AF = mybir.ActivationFunctionType
ALU = mybir.AluOpType
DR = mybir.MatmulPerfMode.DoubleRow

N = 4096          # rows (a and b)
F = 512           # features
NCORES = 8
NT = N // 128     # j-tiles of 128 partitions = 32
NCLS = 16         # label classes
MARGIN = 1.0


def _register_sqrelu_add():
    """Custom fused DVE op: out = relu(in0 + in1)^2, accum_out = c0 + sum(out)."""
    name = "SQRELU_ADD_ANT"
    for op in dve_ops.OPS:
        if op.name == name:
            return op
    op = dve_ops.DveOp(
        name,
        Spec(body=sq(relu(Src0 + Src1)), accum=operator.add, accum_init=C0),
        subdim=False,
        uops_sha={},
    )
    dve_ops._SUB_OPCODE_FOR_NAME[name] = (
        max(dve_ops._SUB_OPCODE_FOR_NAME.values()) + 1)
    assert dve_ops._SUB_OPCODE_FOR_NAME[name] < 0x20
    for ver in ("v3", "v4"):
        try:
            op.compile(ver)
        except ValueError as e:
            m = re.search(r"\(%s: ([0-9a-f]+) " % ver, str(e))
            if not m:
                raise
            op.uops_sha[ver] = m.group(1)
            op.compile(ver)
    dve_ops.OPS.append(op)
    dve_ops.CUSTOM_DVE_SPECS[name] = op.spec
    return op


def _pin_combined_act_set(arch: str):
    """Make `natural_log_exp_and_others` the only ACT table set offering Exp
    and Ln, so Ln needs no extra load after the exp batches."""
    from concourse.hw_specs import get_activation_tables
    tabs = get_activation_tables(arch)
    assert AF.Exp in tabs["natural_log_exp_and_others"]
    assert AF.Ln in tabs["natural_log_exp_and_others"]
    for name, fns in tabs.items():
        if name != "natural_log_exp_and_others":
            fns.discard(AF.Exp)
            fns.discard(AF.Ln)


def build_bass(R_pad: int, nt2: int, nt2p: int):
    """R_pad: padded rows/core; nt2: window tiles; nt2p: dT slots (mult of 4)."""
    sqrelu_add = _register_sqrelu_add()
    CH = R_pad // 2           # psum free-dim chunk (<=512)
    assert CH <= 512
    nU = -(-R_pad // 128)     # 128-chunks of the local row range

    nc = bacc.Bacc("TRN2", target_bir_lowering=False, debug=False,
                   num_devices=NCORES)
    _pin_combined_act_set(nc.m.arch)

    # ---- kernel I/O (per-core shards prepared on host; j permuted) ----
    # at/bt2 are fp8e4m3: the -2ab cross term at fp8 perturbs the final
    # loss by ~1e-4 rel (validated offline); norm terms stay fp32 exact.
    at = nc.dram_tensor("at", [128, 4, N], FP8, kind="ExternalInput").ap()
    bt2 = nc.dram_tensor("bt2", [128, 4, R_pad], FP8, kind="ExternalInput").ap()
    aat = nc.dram_tensor("aat", [128, NT], F32, kind="ExternalInput").ap()
    bbbc = nc.dram_tensor("bbbc", [128, R_pad], F32, kind="ExternalInput").ap()
    oh3 = nc.dram_tensor("oh3", [128, NT * 3], BF16, kind="ExternalInput").ap()
    oh2 = nc.dram_tensor("oh2", [128, nt2 * 2], BF16, kind="ExternalInput").ap()
    sel3 = nc.dram_tensor("sel3", [3, R_pad], F32, kind="ExternalInput").ap()
    ohmy2 = nc.dram_tensor("ohmy2", [2, R_pad], F32, kind="ExternalInput").ap()
    ti = nc.dram_tensor("ti", [128, nU * nt2p], BF16, kind="ExternalInput").ap()
    # pm carries nU permutation blocks + one identity block (fill fold-in)
    pm = nc.dram_tensor("pm", [128, (nU + 1) * 128], BF16,
                        kind="ExternalInput").ap()
    fillm = nc.dram_tensor("fillm", [128, nt2p], BF16, kind="ExternalInput").ap()
    validm = nc.dram_tensor("validm", [1, R_pad], F32, kind="ExternalInput").ap()
    ddiag = nc.dram_tensor("ddiag", [1, R_pad], F32, kind="ExternalInput").ap()

    out_pos = nc.dram_tensor("out_pos", [2, 1], F32, kind="ExternalOutput").ap()
    out_diag = nc.dram_tensor("out_diag", [1, 1], F32, kind="ExternalOutput").ap()
    out_ns = nc.dram_tensor("out_ns", [1, R_pad], F32, kind="ExternalOutput").ap()

    with tile.TileContext(nc) as tc, ExitStack() as ctx:
        sb = ctx.enter_context(tc.tile_pool(name="sb", bufs=1))
        lp = ctx.enter_context(tc.tile_pool(name="lp", bufs=4))      # Dsq chunks f32
        dp = ctx.enter_context(tc.tile_pool(name="dp", bufs=6))      # non-window D f32
        ep = ctx.enter_context(tc.tile_pool(name="ep", bufs=4))      # Dexpm bf16
        work = ctx.enter_context(tc.tile_pool(name="work", bufs=2))
        small = ctx.enter_context(tc.tile_pool(name="small", bufs=2))
        tail = ctx.enter_context(tc.tile_pool(name="tail", bufs=1))

        # tiny dummy ACT op: forces the first (sqrt) table load during DMA wait
        dummy = sb.tile([1, 8], F32)
        nc.vector.memset(dummy, 1.0)
        last_sc = nc.scalar.activation(out=dummy, in_=dummy, func=AF.Sqrt)

        def chain_sc(inst):
            # explicit scalar-queue order: keeps sqrt/exp in table batches
            nonlocal last_sc
            add_dep_helper(inst.ins, last_sc.ins, False, "scalar batch order")
            last_sc = inst

        # ---- resident SBUF tensors / DMA issue plan ----
        # scalar queue: bt2 (fp8, 139KB, lands early)
        bt_sb = sb.tile([128, 4, R_pad], FP8)
        nc.scalar.dma_start(out=bt_sb, in_=bt2)
        # gpsimd queue (own DMA engine): aat+bbbc first - the first stt
        # needs them; behind the sync queue's at stream they'd land ~19us
        aat_sb = sb.tile([128, NT], F32)
        bbbc_sb = sb.tile([128, R_pad], F32)
        nc.gpsimd.dma_start(out=aat_sb, in_=aat)
        nc.gpsimd.dma_start(out=bbbc_sb, in_=bbbc)
        # sync queue: at fully resident (fp8 = 16KB/partition), split so
        # the first slots land fast and the tail keeps ahead of the GEMM
        at_sb = sb.tile([128, 4, N], FP8)
        for lo, hi in ((0, 256), (256, 1280), (1280, 2560), (2560, N)):
            nc.sync.dma_start(out=at_sb[:, :, lo:hi], in_=at[:, :, lo:hi])

        # remaining residents ride the tensor/vector queues mid-GEMM
        oh3_sb = sb.tile([128, NT * 3], BF16)
        oh2_sb = sb.tile([128, nt2 * 2], BF16)
        sel3_sb = sb.tile([3, R_pad], F32)
        ohmy2_sb = sb.tile([2, R_pad], F32)
        ti_sb = sb.tile([128, nU * nt2p], BF16)
        pm_sb = sb.tile([128, (nU + 1) * 128], BF16)
        fill_sb = sb.tile([128, nt2p], BF16)
        valid_sb = sb.tile([1, R_pad], F32)
        ddiag_sb = sb.tile([1, R_pad], F32)

        dT = sb.tile([128, nt2p, R_pad], F32)      # window D
        ones128c = sb.tile([1, 128], BF16)
        nc.vector.memset(ones128c, 1.0)
        ones11 = sb.tile([1, 1], F32)
        nc.vector.memset(ones11, 1.0)
        w3 = sb.tile([3, 1], BF16)
        nc.vector.memset(w3, -1.0)
        nc.vector.memset(w3[0:1], 1.0)

        # ================= PHASE 1: GEMM -> +norms -> sqrt -> exp =========
        with tc.tile_pool(name="bl_ps", bufs=1, space="PSUM") as bl_pool:
            dsq_ctx = tc.tile_pool(name="dsq_ps", bufs=3, space="PSUM")
            dsq_pool = dsq_ctx.__enter__()

            bl_ps = bl_pool.tile([3, 2, 512], F32)   # negsum accumulator

            L4 = None
            pend_D = []    # (D tile, first slot, n slots) awaiting exp
            pend_E = []    # (E tile, first slot, n slots) awaiting bylabel
            nbl = 0        # bylabel slots emitted (0..NT)

            def emit_bylabel():
                nonlocal nbl
                E4, t0, ntiles = pend_E.pop(0)
                for r_ in range(ntiles):
                    t = t0 + r_
                    for c_ in range(2):
                        nc.tensor.matmul(
                            out=bl_ps[:, c_, 0:CH],
                            lhsT=oh3_sb[:, t * 3:(t + 1) * 3],
                            rhs=E4[:, r_, c_ * CH:(c_ + 1) * CH],
                            start=(nbl == 0), stop=(nbl == NT - 1))
                    nbl += 1

            def emit_exp(n=100):
                while pend_D and n > 0:
                    D4b, t0b, csz_ = pend_D.pop(0)
                    E4 = ep.tile([128, csz_, R_pad], BF16, tag="E4")
                    chain_sc(nc.scalar.activation(
                        out=E4, in_=D4b, func=AF.Exp,
                        scale=-1.0, bias=float(MARGIN)))
                    pend_E.append((E4, t0b, csz_))
                    n -= 1

            # sqrt chunk plan: tiny chunks first (earlier ACT start),
            # then 4-slot chunks; table block 1 = chunks 0..2 (slots 0-3)
            chunk_sizes = [1, 1, 2] + [4] * 7
            chunk_start = [0]
            for csz_ in chunk_sizes[:-1]:
                chunk_start.append(chunk_start[-1] + csz_)
            slot2chunk = {}
            for ci_, (cs_, csz_) in enumerate(zip(chunk_start, chunk_sizes)):
                for o_ in range(csz_):
                    slot2chunk[cs_ + o_] = (ci_, o_, csz_)
            NBLK1 = 3

            for jt in range(NT):
                dsq = dsq_pool.tile([128, 2, 512], F32, tag="dsq")
                for c_ in range(2):
                    for g_ in range(2):
                        nc.tensor.matmul(
                            out=dsq[:, c_, 0:CH],
                            lhsT=at_sb[:, 2 * g_:2 * g_ + 2,
                                       jt * 128:(jt + 1) * 128],
                            rhs=bt_sb[:, 2 * g_:2 * g_ + 2,
                                      c_ * CH:(c_ + 1) * CH],
                            start=(g_ == 0), stop=(g_ == 1), perf_mode=DR)

                # resident DMA issues ride the (otherwise idle) gpsimd queue
                if jt == 1:
                    nc.gpsimd.dma_start(out=oh3_sb, in_=oh3)
                    nc.gpsimd.dma_start(out=pm_sb, in_=pm)
                    nc.gpsimd.dma_start(out=oh2_sb, in_=oh2)
                elif jt == 3:
                    nc.gpsimd.dma_start(out=sel3_sb, in_=sel3)
                    nc.gpsimd.dma_start(out=ohmy2_sb, in_=ohmy2)
                    nc.gpsimd.dma_start(out=ti_sb, in_=ti)
                elif jt == 5:
                    nc.gpsimd.dma_start(out=fill_sb, in_=fillm)
                    nc.gpsimd.dma_start(out=valid_sb, in_=validm)
                    nc.gpsimd.dma_start(out=ddiag_sb, in_=ddiag)

                # DVE adds the norm terms: L4 = dsq + aa[j] + bb[i]
                ci_, off_, csz_ = slot2chunk[jt]
                if off_ == 0:
                    L4 = lp.tile([128, csz_, R_pad], F32, tag="L4")
                nc.vector.scalar_tensor_tensor(
                    out=L4[:, off_, :].rearrange("p (c f) -> p c f", c=2),
                    in0=dsq[:, :, 0:CH],
                    scalar=aat_sb[:, jt:jt + 1],
                    in1=bbbc_sb.rearrange("p (c f) -> p c f", c=2),
                    op0=ALU.add, op1=ALU.add)

                # interleave bylabel matmuls for block-1 exps mid-GEMM
                if jt >= 12 and pend_E:
                    emit_bylabel()

                if off_ == csz_ - 1:
                    cs_ = chunk_start[ci_]
                    if cs_ + csz_ <= nt2p:
                        D4 = dT[:, cs_:cs_ + csz_, :]
                    else:
                        D4 = dp.tile([128, csz_, R_pad], F32, tag="D4")
                    chain_sc(nc.scalar.activation(out=D4, in_=L4,
                                                  func=AF.Sqrt))
                    pend_D.append((D4, cs_, csz_))
                    if ci_ == NBLK1 - 1:
                        emit_exp()     # exp chunks 0..NBLK1-1

            # sqrt chunks NBLK1.. happened above; now their exps with
            # bylabel trailing each exp so only the last chunk's bylabel
            # gates ns. The final chunk's exp is split in half so its
            # bylabel tail is ~1 us shorter.
            while pend_D:
                if len(pend_D) == 1:
                    D4b, t0b, csz_ = pend_D.pop(0)
                    h2sz = csz_ // 2
                    for h_ in range(2):
                        E2 = ep.tile([128, h2sz, R_pad], BF16, tag="E4",
                                     name=f"E2h{h_}")
                        chain_sc(nc.scalar.activation(
                            out=E2, in_=D4b[:, h_ * h2sz:(h_ + 1) * h2sz, :],
                            func=AF.Exp, scale=-1.0, bias=float(MARGIN)))
                        pend_E.append((E2, t0b + h_ * h2sz, h2sz))
                        while len(pend_E) > 1:
                            emit_bylabel()
                else:
                    emit_exp(1)
                while len(pend_E) > 1:
                    emit_bylabel()
            while pend_E:
                emit_bylabel()

            dsq_ctx.__exit__(None, None, None)   # free the 6 dsq banks

            with tc.tile_pool(name="ns_ps", bufs=1, space="PSUM") as ns_pool:
                # -- ns = total - own-class:  w3.T @ (bl * sel3) --
                prod_sb = tail.tile([3, 2, CH], BF16, tag="prod3")
                nc.vector.scalar_tensor_tensor(
                    out=prod_sb, in0=bl_ps[:, :, 0:CH], scalar=0.0,
                    in1=sel3_sb.rearrange("p (c f) -> p c f", c=2),
                    op0=ALU.bypass, op1=ALU.mult)
                ns_ps = ns_pool.tile([1, 2, 512], F32, name="ns_ps")
                for c_ in range(2):
                    nc.tensor.matmul(out=ns_ps[:, c_, 0:CH], lhsT=w3,
                                     rhs=prod_sb[:, c_, :],
                                     start=True, stop=True)
                ns_my = sb.tile([1, R_pad], F32)
                nc.vector.tensor_copy(
                    out=ns_my.rearrange("p (c f) -> p c f", c=2),
                    in_=ns_ps[:, :, 0:CH])

                # diag ln(2 ns_i): fills the ACT idle window while the
                # nsT/ns_bc machinery runs
                lnterm = tail.tile([1, R_pad], F32, tag="lnt")
                chain_sc(nc.scalar.activation(out=lnterm, in_=ns_my,
                                              func=AF.Ln, scale=2.0))

                ns_bf = sb.tile([1, R_pad], BF16)
                nc.vector.tensor_copy(out=ns_bf, in_=ns_my)

                # -- nsT: ns_j in [128, slot] layout via transpose+perm --
                nsL_ps = ns_pool.tile([128, nU], F32, name="nsL_ps")
                for u in range(nU):
                    lo = 128 * u
                    hi = min(R_pad, lo + 128)
                    nc.tensor.matmul(out=nsL_ps[0:hi - lo, u:u + 1],
                                     lhsT=ns_my[0:1, lo:hi], rhs=ones11,
                                     start=True, stop=True)
                # broadcast ns_my across partitions: [128, R_pad]
                nsbc_ps = ns_pool.tile([128, 2, 512], F32, name="nsbc_ps")
                for c_ in range(2):
                    nc.tensor.matmul(out=nsbc_ps[:, c_, 0:CH], lhsT=ones128c,
                                     rhs=ns_bf[:, c_ * CH:(c_ + 1) * CH],
                                     start=True, stop=True)

                # rhs_u reads nsL straight from PSUM as the per-partition
                # scalar (garbage partitions are masked by ti=0, and psum
                # holds only finite floats); the trailing identity-block
                # matmul folds in the 1.0 fill for alien partitions, so
                # phase 2 reads nsT straight from PSUM too.
                nsT_ps = ns_pool.tile([128, nt2p], F32, name="nsT_ps")
                for u in range(nU):
                    rhs_u = small.tile([128, nt2p], BF16, tag="rhsu")
                    nc.vector.scalar_tensor_tensor(
                        out=rhs_u, in0=ti_sb[:, u * nt2p:(u + 1) * nt2p],
                        scalar=nsL_ps[:, u:u + 1],
                        in1=ti_sb[:, u * nt2p:(u + 1) * nt2p],
                        op0=ALU.mult, op1=ALU.bypass)
                    nc.tensor.matmul(out=nsT_ps,
                                     lhsT=pm_sb[:, u * 128:(u + 1) * 128],
                                     rhs=rhs_u,
                                     start=(u == 0), stop=False)
                nc.tensor.matmul(out=nsT_ps,
                                 lhsT=pm_sb[:, nU * 128:(nU + 1) * 128],
                                 rhs=fill_sb, start=False, stop=True)
                # ACT bias APs must live in SBUF: one tiny copy
                nsT_sb = sb.tile([128, nt2p], F32)
                nc.vector.tensor_copy(out=nsT_sb, in_=nsT_ps)
                nc.sync.dma_start(out=out_ns, in_=ns_my)

                # ========= PHASE 2: J = ln(ns_i+ns_j) + D; hinge^2 =======
                # hinge-by-class accumulates into the (drained) bl banks
                for t in range(nt2):
                    Lt = work.tile([128, R_pad], F32, tag="L")
                    chain_sc(nc.scalar.activation(
                        out=Lt.rearrange("p (c f) -> p c f", c=2),
                        in_=nsbc_ps[:, :, 0:CH], func=AF.Ln,
                        bias=nsT_sb[:, t:t + 1], scale=1.0))
                    h2 = work.tile([128, R_pad], BF16, tag="h2")
                    acc_d = small.tile([128, 1], F32, tag="accd")
                    nc.vector._custom_dve(
                        sqrelu_add, out=h2, in0=Lt, in1=dT[:, t, :],
                        s0=0.0, accum_out=acc_d)
                    for c_ in range(2):
                        nc.tensor.matmul(
                            out=bl_ps[0:2, c_, 0:CH],
                            lhsT=oh2_sb[:, t * 2:(t + 1) * 2],
                            rhs=h2[:, c_ * CH:(c_ + 1) * CH],
                            start=(t == 0), stop=(t == nt2 - 1))

                # diagonal correction relu(ln(2 ns_i) + D_ii)^2 (masked)
                dh2 = tail.tile([1, R_pad], F32, tag="dh2")
                dummy_acc = small.tile([1, 1], F32, tag="dumacc")
                nc.vector._custom_dve(sqrelu_add, out=dh2, in0=lnterm,
                                      in1=ddiag_sb, s0=0.0,
                                      accum_out=dummy_acc)
                diag_acc = tail.tile([1, 1], F32, tag="dacc")
                dh2m = tail.tile([1, R_pad], F32, tag="dh2m")
                nc.vector.scalar_tensor_tensor(
                    out=dh2m, in0=dh2, scalar=0.0, in1=valid_sb,
                    op0=ALU.bypass, op1=ALU.mult, accum_out=diag_acc)
                nc.sync.dma_start(out=out_diag, in_=diag_acc)

                # -- combine: mask by i-side class match, accumulate --
                prod2 = tail.tile([2, 2, CH], F32, tag="prod2")
                acc2 = small.tile([2, 1], F32, tag="acc2")
                nc.vector.scalar_tensor_tensor(
                    out=prod2, in0=bl_ps[0:2, :, 0:CH], scalar=0.0,
                    in1=ohmy2_sb.rearrange("p (c f) -> p c f", c=2),
                    op0=ALU.bypass, op1=ALU.mult, accum_out=acc2)
                nc.sync.dma_start(out=out_pos, in_=acc2)

    nc.compile()
    return nc


_CACHE: dict = {}


def _get_nc(R_pad: int, nt2: int, nt2p: int):
    key = ("nc", R_pad, nt2, nt2p)
    if key not in _CACHE:
        _CACHE[key] = build_bass(R_pad, nt2, nt2p)
    return _CACHE[key]


def prepare_inputs(a: np.ndarray, b: np.ndarray, labels: np.ndarray):
    """Host-side label sort, class pairing, per-core shard + mask prep.

    Returns (per-core input maps, (R_pad, nt2, nt2p), meta)."""
    a = np.asarray(a, np.float32)
    b = np.asarray(b, np.float32)
    labels = np.asarray(labels)

    order = np.argsort(labels, kind="stable")
    a_s = a[order]
    b_s = b[order]
    sl = labels[order]
    counts = np.bincount(sl.astype(np.int64), minlength=NCLS)
    startscum = np.concatenate([[0], np.cumsum(counts)])

    def pair_tiles(p, q):
        ta = set(range(int(startscum[p]) // 128,
                       -(-int(startscum[p + 1]) // 128)))
        tb = set(range(int(startscum[q]) // 128,
                       -(-int(startscum[q + 1]) // 128)))
        return len(ta | tb)

    def pairing_cost(pairs_):
        return (max(int(counts[p] + counts[q]) for p, q in pairs_),
                max(pair_tiles(p, q) for p, q in pairs_))

    # greedy largest-with-smallest, then 2-opt swaps minimizing
    # (max pair size, max window tiles) lexicographically
    co = np.argsort(counts)
    pairs = [(int(co[i]), int(co[NCLS - 1 - i])) for i in range(NCORES)]
    best = pairing_cost(pairs)
    improved = True
    while improved:
        improved = False
        for i in range(NCORES):
            for j in range(i + 1, NCORES):
                for swap in ((0, 0), (0, 1)):
                    cand = list(pairs)
                    a1, b1 = pairs[i]
                    a2, b2 = pairs[j]
                    if swap == (0, 0):
                        cand[i], cand[j] = (a2, b1), (a1, b2)
                    else:
                        cand[i], cand[j] = (b2, b1), (a2, a1)
                    c = pairing_cost(cand)
                    if c < best:
                        pairs, best, improved = cand, c, True
    R_pad = best[0]
    R_pad = -(-R_pad // 32) * 32
    nU = -(-R_pad // 128)

    cores = []
    nt2 = 0
    for p, q in pairs:
        grows = np.concatenate([
            np.arange(startscum[p], startscum[p + 1]),
            np.arange(startscum[q], startscum[q + 1])])
        wtiles = sorted(set((grows // 128).tolist()))
        nt2 = max(nt2, len(wtiles))
        cores.append((p, q, grows, wtiles))
    nt2p = -(-nt2 // 4) * 4

    at_full = np.ascontiguousarray(a_s.T)                  # [F, N] sorted
    aa = np.sum(a_s * a_s, axis=1, dtype=np.float32)
    bb_s = np.sum(b_s * b_s, axis=1, dtype=np.float32)

    in_maps = []
    meta = []
    for c in range(NCORES):
        p, q, grows, wtiles = cores[c]
        Rc = len(grows)
        rest = [t for t in range(NT) if t not in wtiles]
        sigma = np.array(list(wtiles) + rest)
        slot_of = {t: s_ for s_, t in enumerate(sigma)}

        cols = (sigma[:, None] * 128 + np.arange(128)[None, :]).reshape(-1)
        # [128 kpart, 4 ksub, N] fp8 for DoubleRow lhsT slices
        at_c = np.ascontiguousarray(
            at_full[:, cols].reshape(4, 128, N).transpose(1, 0, 2)
        ).astype(NPFP8)
        aat_c = np.ascontiguousarray(aa[cols].reshape(NT, 128).T)  # [128, NT]

        glbl = sl[cols].reshape(NT, 128)                   # labels per slot
        oh3_c = np.zeros((NT, 128, 3), np.float32)
        oh3_c[:, :, 0] = 1.0
        oh3_c[:, :, 1] = glbl == p
        oh3_c[:, :, 2] = glbl == q
        oh3_c = np.ascontiguousarray(
            oh3_c.transpose(1, 0, 2).reshape(128, NT * 3)).astype(NPBF16)
        oh2_c = np.zeros((nt2, 128, 2), np.float32)
        oh2_c[:, :, 0] = glbl[:nt2] == p
        oh2_c[:, :, 1] = glbl[:nt2] == q
        oh2_c = np.ascontiguousarray(
            oh2_c.transpose(1, 0, 2).reshape(128, nt2 * 2)).astype(NPBF16)

        b_loc = np.zeros((R_pad, F), np.float32)
        b_loc[:Rc] = b_s[grows]
        a_my = np.zeros((R_pad, F), np.float32)
        a_my[:Rc] = a_s[grows]
        bb_loc = np.zeros(R_pad, np.float32)
        bb_loc[:Rc] = bb_s[grows]
        bt2_c = np.ascontiguousarray(
            (-2.0 * b_loc).T.reshape(4, 128, R_pad).transpose(1, 0, 2)
        ).astype(NPFP8)
        bbbc_c = np.ascontiguousarray(
            np.broadcast_to(bb_loc[None, :], (128, R_pad)))

        lbl_loc = np.full(R_pad, -1, np.int64)
        lbl_loc[:Rc] = sl[grows]
        selP = (lbl_loc == p).astype(np.float32)
        selQ = (lbl_loc == q).astype(np.float32)
        sel3_c = np.ascontiguousarray(
            np.stack([np.ones(R_pad, np.float32), selP, selQ], 0))
        ohmy2_c = np.ascontiguousarray(np.stack([selP, selQ], 0))

        ti_c = np.zeros((nU, 128, nt2p), np.float32)
        pm_c = np.zeros((nU + 1, 128, 128), np.float32)
        pm_c[nU] = np.eye(128, dtype=np.float32)
        used = np.zeros((128, nt2p), bool)
        for r in range(Rc):
            gr = grows[r]
            u, cc = r // 128, r % 128
            t_ = slot_of[gr // 128]
            ti_c[u, cc, t_] = 1.0
            pm_c[u, cc, gr % 128] = 1.0
            used[gr % 128, t_] = True
        ti_c = np.ascontiguousarray(
            ti_c.transpose(1, 0, 2).reshape(128, nU * nt2p)).astype(NPBF16)
        pm_c = np.ascontiguousarray(
            pm_c.transpose(1, 0, 2).reshape(128, (nU + 1) * 128)).astype(NPBF16)
        fill_c = np.where(used, 0.0, 1.0).astype(NPBF16)

        valid_c = (np.arange(R_pad) < Rc).astype(np.float32).reshape(1, R_pad)
        dd = np.sum(np.square(b_loc - a_my), axis=1, dtype=np.float32)
        ddiag_c = np.sqrt(np.maximum(dd, 0.0)).reshape(1, R_pad)

        in_maps.append({
            "at": at_c, "bt2": bt2_c, "aat": aat_c, "bbbc": bbbc_c,
            "oh3": oh3_c, "oh2": oh2_c, "sel3": sel3_c, "ohmy2": ohmy2_c,
            "ti": ti_c, "pm": pm_c, "fillm": np.ascontiguousarray(fill_c),
            "validm": valid_c, "ddiag": ddiag_c,
        })
        meta.append({"grows": grows, "Rc": Rc})
    return in_maps, (R_pad, nt2, nt2p), {"order": order, "cores": meta}


def run(a, b, labels, trace=False, trace_kwargs=None):
    """Run on 8 NeuronCores; returns (loss, BassKernelResults, meta)."""
    in_maps, dims, meta = prepare_inputs(a, b, labels)
    nc = _get_nc(*dims)
    kw = {}
    if trace:
        kw = dict(trace=True, **(trace_kwargs or {}))
    res = run_bass_kernel_spmd(nc, in_maps, core_ids=list(range(NCORES)), **kw)

    counts = np.bincount(np.asarray(labels).astype(np.int64), minlength=NCLS)
    num_pos = float((counts.astype(np.float64) ** 2).sum() - N)

    total = 0.0
    for c in range(NCORES):
        r = res.results[c]
        total += (float(r["out_pos"][0, 0]) + float(r["out_pos"][1, 0])
                  - float(r["out_diag"][0, 0]))
    loss = total / (2.0 * num_pos)
    return np.asarray(np.float32(loss)), res, meta


def kernel(a, b, labels):
    loss, _, _ = run(a, b, labels)
    return loss


# revision 46
# speedup vs baseline: 2.8569x; 1.0105x over previous
"""Trainium2 Bass kernel for nn_MetricLoss (lifted-structure-style metric loss).

Reference computation (N=4096 rows, F=512 features, 16 label classes):
    Dsq = ||b_i||^2 + ||a_j||^2 - 2 b@a.T ;  D = sqrt(max(Dsq,0))   [N,N]
    Dexpm = exp(1 - D)
    row_negsum[i] = sum_{j: lbl_j != lbl_i} Dexpm[i,j]
    J = log(row_negsum[i] + row_negsum[j]) + D
    loss = sum_{i!=j, lbl_i==lbl_j} relu(J)^2 / (2 * num_pos)

v2 design — fully decoupled cores (NO collectives):
  * Rows are sorted by label on the host; label classes are PAIRED
    (largest-with-smallest) and each core owns all rows of its 2 classes
    (padded with zero-rows to a common R_pad). Every positive pair (i, j)
    then has both ns_i and ns_j computed locally, so the AllGather of
    row_negsum is gone: no inter-core dependency at all. Per-core HW exec
    time no longer includes multi-core launch-skew waits (the v1 kernel
    showed 96us on the last-launched core vs 200us+ on early cores).
  * Per-core column permutation sigma puts the core's window j-tiles
    (tiles overlapping its 2 classes) at slots 0..nt2-1, so the phase-2
    loop structure is core-independent (pure SPMD); all class masks are
    input data, not program structure.
  * negsum via 3-column one-hot matmuls per j-tile (ones/classP/classQ);
    ns = total - own-class, combined with a [3,1] +-1 matmul.
  * ns_j in partition layout (nsT[128, t]) is built with a 5-step
    transpose + per-chunk (TI mask x PM permutation-matmul) accumulation,
    all from per-core input matrices - SPMD-safe despite per-core offsets.
  * D_ii (diagonal) is host-precomputed (same O(N F) class as aa/bb).
  * ACT runs in 4 table blocks (sqrt 0-3, exp 0-3, sqrt 4-7 + ddiag,
    exp 4..7) = 3 visible table loads; exp of the LAST chunk runs first
    in its block... (actually last block order 4,5,6,7 with bylabel
    matmuls trailing each exp so only the last chunk's bylabel gates ns).
  * GEMM free dim is chunked 2x272 (R_pad=544) so matmul outputs stay
    within PSUM banks; weight loads stay at 128 (chunks share lhsT).
  * Input DMAs are spread across the sync/scalar/vector/tensor queues,
    with slot-0/1 `at` strips first so the first matmul fires early.

The GEMM runs in bf16 (b @ a.T as 256 matmuls of [128k,128j]x[128,272i]).
"""

import re
import operator
import numpy as np
import ml_dtypes
from contextlib import ExitStack

import concourse.bass as bass
import concourse.tile as tile
from concourse import bacc, mybir
from concourse import dve_ops
from concourse.dve_spec import Spec, Src0, Src1, C0, relu, sq
from concourse.bass_utils import run_bass_kernel_spmd
from concourse.tile_rust import add_dep_helper

F32 = mybir.dt.float32
BF16 = mybir.dt.bfloat16
FP8 = mybir.dt.float8e4
NPBF16 = ml_dtypes.bfloat16
NPFP8 = mybir.dt.np(mybir.dt.float8e4)
AF = mybir.ActivationFunctionType
ALU = mybir.AluOpType
DR = mybir.MatmulPerfMode.DoubleRow

N = 4096          # rows (a and b)
F = 512           # features
NCORES = 8
NT = N // 128     # j-tiles of 128 partitions = 32
NCLS = 16         # label classes
MARGIN = 1.0


def _register_sqrelu_add():
    """Custom fused DVE op: out = relu(in0 + in1)^2, accum_out = c0 + sum(out)."""
    name = "SQRELU_ADD_ANT"
    for op in dve_ops.OPS:
        if op.name == name:
            return op
    op = dve_ops.DveOp(
        name,
        Spec(body=sq(relu(Src0 + Src1)), accum=operator.add, accum_init=C0),
        subdim=False,
        uops_sha={},
    )
    dve_ops._SUB_OPCODE_FOR_NAME[name] = (
        max(dve_ops._SUB_OPCODE_FOR_NAME.values()) + 1)
    assert dve_ops._SUB_OPCODE_FOR_NAME[name] < 0x20
    for ver in ("v3", "v4"):
        try:
            op.compile(ver)
        except ValueError as e:
            m = re.search(r"\(%s: ([0-9a-f]+) " % ver, str(e))
            if not m:
                raise
            op.uops_sha[ver] = m.group(1)
            op.compile(ver)
    dve_ops.OPS.append(op)
    dve_ops.CUSTOM_DVE_SPECS[name] = op.spec
    return op


def _pin_combined_act_set(arch: str):
    """Make `natural_log_exp_and_others` the only ACT table set offering Exp
    and Ln, so Ln needs no extra load after the exp batches."""
    from concourse.hw_specs import get_activation_tables
    tabs = get_activation_tables(arch)
    assert AF.Exp in tabs["natural_log_exp_and_others"]
    assert AF.Ln in tabs["natural_log_exp_and_others"]
    for name, fns in tabs.items():
        if name != "natural_log_exp_and_others":
            fns.discard(AF.Exp)
            fns.discard(AF.Ln)


def build_bass(R_pad: int, nt2: int, nt2p: int):
    """R_pad: padded rows/core; nt2: window tiles; nt2p: dT slots (mult of 4)."""
    sqrelu_add = _register_sqrelu_add()
    CH = R_pad // 2           # psum free-dim chunk (<=512)
    assert CH <= 512
    nU = -(-R_pad // 128)     # 128-chunks of the local row range

    nc = bacc.Bacc("TRN2", target_bir_lowering=False, debug=False,
                   num_devices=NCORES)
    _pin_combined_act_set(nc.m.arch)

    # ---- kernel I/O (per-core shards prepared on host; j permuted) ----
    # at/bt2 are fp8e4m3: the -2ab cross term at fp8 perturbs the final
    # loss by ~1e-4 rel (validated offline); norm terms stay fp32 exact.
    at = nc.dram_tensor("at", [128, 4, N], FP8, kind="ExternalInput").ap()
    bt2 = nc.dram_tensor("bt2", [128, 4, R_pad], FP8, kind="ExternalInput").ap()
    aat = nc.dram_tensor("aat", [128, NT], F32, kind="ExternalInput").ap()
    bbv = nc.dram_tensor("bbv", [1, R_pad], F32, kind="ExternalInput").ap()
    oh3 = nc.dram_tensor("oh3", [128, NT * 3], BF16, kind="ExternalInput").ap()
    oh2 = nc.dram_tensor("oh2", [128, nt2 * 2], BF16, kind="ExternalInput").ap()
    sel3 = nc.dram_tensor("sel3", [3, R_pad], F32, kind="ExternalInput").ap()
    ohmy2 = nc.dram_tensor("ohmy2", [2, R_pad], F32, kind="ExternalInput").ap()
    ti = nc.dram_tensor("ti", [128, nU * nt2p], BF16, kind="ExternalInput").ap()
    # pm carries nU permutation blocks + one identity block (fill fold-in)
    pm = nc.dram_tensor("pm", [128, (nU + 1) * 128], BF16,
                        kind="ExternalInput").ap()
    fillm = nc.dram_tensor("fillm", [128, nt2p], BF16, kind="ExternalInput").ap()
    validm = nc.dram_tensor("validm", [1, R_pad], F32, kind="ExternalInput").ap()
    ddiag = nc.dram_tensor("ddiag", [1, R_pad], F32, kind="ExternalInput").ap()

    out_pos = nc.dram_tensor("out_pos", [2, 1], F32, kind="ExternalOutput").ap()
    out_diag = nc.dram_tensor("out_diag", [1, 1], F32, kind="ExternalOutput").ap()
    out_ns = nc.dram_tensor("out_ns", [1, R_pad], F32, kind="ExternalOutput").ap()

    with tile.TileContext(nc) as tc, ExitStack() as ctx:
        sb = ctx.enter_context(tc.tile_pool(name="sb", bufs=1))
        lp = ctx.enter_context(tc.tile_pool(name="lp", bufs=4))      # Dsq chunks f32
        dp = ctx.enter_context(tc.tile_pool(name="dp", bufs=6))      # non-window D f32
        ep = ctx.enter_context(tc.tile_pool(name="ep", bufs=4))      # Dexpm bf16
        work = ctx.enter_context(tc.tile_pool(name="work", bufs=2))
        small = ctx.enter_context(tc.tile_pool(name="small", bufs=2))
        tail = ctx.enter_context(tc.tile_pool(name="tail", bufs=1))

        # tiny dummy ACT op: forces the first (sqrt) table load during DMA wait
        dummy = sb.tile([1, 8], F32)
        nc.vector.memset(dummy, 1.0)
        last_sc = nc.scalar.activation(out=dummy, in_=dummy, func=AF.Sqrt)

        def chain_sc(inst):
            # explicit scalar-queue order: keeps sqrt/exp in table batches
            nonlocal last_sc
            add_dep_helper(inst.ins, last_sc.ins, False, "scalar batch order")
            last_sc = inst

        # ---- resident SBUF tensors / DMA issue plan ----
        # scalar queue: bb (tiny, feeds the on-device broadcast), then bt2
        bb_sb = sb.tile([1, R_pad], F32)
        nc.scalar.dma_start(out=bb_sb, in_=bbv)
        bt_sb = sb.tile([128, 4, R_pad], FP8)
        nc.scalar.dma_start(out=bt_sb, in_=bt2)
        # gpsimd queue (own DMA engine): aat first - the first stt needs it
        aat_sb = sb.tile([128, NT], F32)
        nc.gpsimd.dma_start(out=aat_sb, in_=aat)
        # sync queue: at fully resident (fp8 = 16KB/partition), split so
        # the first slots land fast and the tail keeps ahead of the GEMM
        at_sb = sb.tile([128, 4, N], FP8)
        for lo, hi in ((0, 256), (256, 1280), (1280, 2560), (2560, N)):
            nc.sync.dma_start(out=at_sb[:, :, lo:hi], in_=at[:, :, lo:hi])

        # remaining residents ride the tensor/vector queues mid-GEMM
        oh3_sb = sb.tile([128, NT * 3], BF16)
        oh2_sb = sb.tile([128, nt2 * 2], BF16)
        sel3_sb = sb.tile([3, R_pad], F32)
        ohmy2_sb = sb.tile([2, R_pad], F32)
        ti_sb = sb.tile([128, nU * nt2p], BF16)
        pm_sb = sb.tile([128, (nU + 1) * 128], BF16)
        fill_sb = sb.tile([128, nt2p], BF16)
        valid_sb = sb.tile([1, R_pad], F32)
        ddiag_sb = sb.tile([1, R_pad], F32)

        dT = sb.tile([128, nt2p, R_pad], F32)      # window D
        ones128c = sb.tile([1, 128], BF16)
        nc.vector.memset(ones128c, 1.0)
        ones128f = sb.tile([1, 128], F32)
        nc.vector.memset(ones128f, 1.0)
        ones11 = sb.tile([1, 1], F32)
        nc.vector.memset(ones11, 1.0)
        w3 = sb.tile([3, 1], BF16)
        nc.vector.memset(w3, -1.0)
        nc.vector.memset(w3[0:1], 1.0)

        # bb broadcast to all partitions built on-device (fp32-exact; saves
        # a 278KB DMA from the critical startup window)
        bbbc_sb = sb.tile([128, R_pad], F32)
        with tc.tile_pool(name="bc_ps", bufs=1, space="PSUM") as bc_pool:
            bcps = bc_pool.tile([128, 2, 512], F32)
            for c_ in range(2):
                nc.tensor.matmul(out=bcps[:, c_, 0:CH], lhsT=ones128f,
                                 rhs=bb_sb[:, c_ * CH:(c_ + 1) * CH],
                                 start=True, stop=True)
            nc.vector.tensor_copy(
                out=bbbc_sb.rearrange("p (c f) -> p c f", c=2),
                in_=bcps[:, :, 0:CH])

        # ================= PHASE 1: GEMM -> +norms -> sqrt -> exp =========
        with tc.tile_pool(name="bl_ps", bufs=1, space="PSUM") as bl_pool:
            dsq_ctx = tc.tile_pool(name="dsq_ps", bufs=3, space="PSUM")
            dsq_pool = dsq_ctx.__enter__()

            bl_ps = bl_pool.tile([3, 2, 512], F32)   # negsum accumulator

            L4 = None
            pend_D = []    # (D tile, first slot, n slots) awaiting exp
            pend_E = []    # (E tile, first slot, n slots) awaiting bylabel
            nbl = 0        # bylabel slots emitted (0..NT)

            def emit_bylabel():
                nonlocal nbl
                E4, t0, ntiles = pend_E.pop(0)
                for r_ in range(ntiles):
                    t = t0 + r_
                    for c_ in range(2):
                        nc.tensor.matmul(
                            out=bl_ps[:, c_, 0:CH],
                            lhsT=oh3_sb[:, t * 3:(t + 1) * 3],
                            rhs=E4[:, r_, c_ * CH:(c_ + 1) * CH],
                            start=(nbl == 0), stop=(nbl == NT - 1))
                    nbl += 1

            def emit_exp(n=100):
                while pend_D and n > 0:
                    D4b, t0b, csz_ = pend_D.pop(0)
                    E4 = ep.tile([128, csz_, R_pad], BF16, tag="E4")
                    chain_sc(nc.scalar.activation(
                        out=E4, in_=D4b, func=AF.Exp,
                        scale=-1.0, bias=float(MARGIN)))
                    pend_E.append((E4, t0b, csz_))
                    n -= 1

            # sqrt chunk plan: tiny chunks first (earlier ACT start),
            # then 4-slot chunks; table block 1 = chunks 0..2 (slots 0-3)
            chunk_sizes = [1, 1, 2] + [4] * 7
            chunk_start = [0]
            for csz_ in chunk_sizes[:-1]:
                chunk_start.append(chunk_start[-1] + csz_)
            slot2chunk = {}
            for ci_, (cs_, csz_) in enumerate(zip(chunk_start, chunk_sizes)):
                for o_ in range(csz_):
                    slot2chunk[cs_ + o_] = (ci_, o_, csz_)
            NBLK1 = 4

            for jt in range(NT):
                dsq = dsq_pool.tile([128, 2, 512], F32, tag="dsq")
                for c_ in range(2):
                    for g_ in range(2):
                        nc.tensor.matmul(
                            out=dsq[:, c_, 0:CH],
                            lhsT=at_sb[:, 2 * g_:2 * g_ + 2,
                                       jt * 128:(jt + 1) * 128],
                            rhs=bt_sb[:, 2 * g_:2 * g_ + 2,
                                      c_ * CH:(c_ + 1) * CH],
                            start=(g_ == 0), stop=(g_ == 1), perf_mode=DR)

                # resident DMA issues ride the (otherwise idle) gpsimd queue
                if jt == 1:
                    nc.gpsimd.dma_start(out=oh3_sb, in_=oh3)
                    nc.gpsimd.dma_start(out=pm_sb, in_=pm)
                    nc.gpsimd.dma_start(out=oh2_sb, in_=oh2)
                elif jt == 3:
                    nc.gpsimd.dma_start(out=sel3_sb, in_=sel3)
                    nc.gpsimd.dma_start(out=ohmy2_sb, in_=ohmy2)
                    nc.gpsimd.dma_start(out=ti_sb, in_=ti)
                elif jt == 5:
                    nc.gpsimd.dma_start(out=fill_sb, in_=fillm)
                    nc.gpsimd.dma_start(out=valid_sb, in_=validm)
                    nc.gpsimd.dma_start(out=ddiag_sb, in_=ddiag)

                # DVE adds the norm terms: L4 = dsq + aa[j] + bb[i]
                ci_, off_, csz_ = slot2chunk[jt]
                if off_ == 0:
                    L4 = lp.tile([128, csz_, R_pad], F32, tag="L4")
                nc.vector.scalar_tensor_tensor(
                    out=L4[:, off_, :].rearrange("p (c f) -> p c f", c=2),
                    in0=dsq[:, :, 0:CH],
                    scalar=aat_sb[:, jt:jt + 1],
                    in1=bbbc_sb.rearrange("p (c f) -> p c f", c=2),
                    op0=ALU.add, op1=ALU.add)

                # interleave bylabel matmuls for block-1 exps mid-GEMM
                if jt >= 12 and pend_E:
                    emit_bylabel()

                if off_ == csz_ - 1:
                    cs_ = chunk_start[ci_]
                    if cs_ + csz_ <= nt2p:
                        D4 = dT[:, cs_:cs_ + csz_, :]
                    else:
                        D4 = dp.tile([128, csz_, R_pad], F32, tag="D4")
                    chain_sc(nc.scalar.activation(out=D4, in_=L4,
                                                  func=AF.Sqrt))
                    pend_D.append((D4, cs_, csz_))
                    if ci_ == NBLK1 - 1:
                        emit_exp()     # exp chunks 0..NBLK1-1

            # sqrt chunks NBLK1.. happened above; now their exps with
            # bylabel trailing each exp so only the last chunk's bylabel
            # gates ns. The final chunk's exp is split in half so its
            # bylabel tail is ~1 us shorter.
            while pend_D:
                if len(pend_D) == 1:
                    D4b, t0b, csz_ = pend_D.pop(0)
                    h2sz = csz_ // 2
                    for h_ in range(2):
                        E2 = ep.tile([128, h2sz, R_pad], BF16, tag="E4",
                                     name=f"E2h{h_}")
                        chain_sc(nc.scalar.activation(
                            out=E2, in_=D4b[:, h_ * h2sz:(h_ + 1) * h2sz, :],
                            func=AF.Exp, scale=-1.0, bias=float(MARGIN)))
                        pend_E.append((E2, t0b + h_ * h2sz, h2sz))
                        while len(pend_E) > 1:
                            emit_bylabel()
                else:
                    emit_exp(1)
                while len(pend_E) > 1:
                    emit_bylabel()
            while pend_E:
                emit_bylabel()

            dsq_ctx.__exit__(None, None, None)   # free the 6 dsq banks

            with tc.tile_pool(name="ns_ps", bufs=1, space="PSUM") as ns_pool:
                # -- ns = total - own-class:  w3.T @ (bl * sel3) --
                prod_sb = tail.tile([3, 2, CH], BF16, tag="prod3")
                nc.vector.scalar_tensor_tensor(
                    out=prod_sb, in0=bl_ps[:, :, 0:CH], scalar=0.0,
                    in1=sel3_sb.rearrange("p (c f) -> p c f", c=2),
                    op0=ALU.bypass, op1=ALU.mult)
                ns_ps = ns_pool.tile([1, 2, 512], F32, name="ns_ps")
                for c_ in range(2):
                    nc.tensor.matmul(out=ns_ps[:, c_, 0:CH], lhsT=w3,
                                     rhs=prod_sb[:, c_, :],
                                     start=True, stop=True)
                ns_my = sb.tile([1, R_pad], F32)
                nc.vector.tensor_copy(
                    out=ns_my.rearrange("p (c f) -> p c f", c=2),
                    in_=ns_ps[:, :, 0:CH])

                # diag ln(2 ns_i): fills the ACT idle window while the
                # nsT/ns_bc machinery runs
                lnterm = tail.tile([1, R_pad], F32, tag="lnt")
                chain_sc(nc.scalar.activation(out=lnterm, in_=ns_my,
                                              func=AF.Ln, scale=2.0))

                ns_bf = sb.tile([1, R_pad], BF16)
                nc.vector.tensor_copy(out=ns_bf, in_=ns_my)

                # -- nsT: ns_j in [128, slot] layout via transpose+perm --
                nsL_ps = ns_pool.tile([128, nU], F32, name="nsL_ps")
                for u in range(nU):
                    lo = 128 * u
                    hi = min(R_pad, lo + 128)
                    nc.tensor.matmul(out=nsL_ps[0:hi - lo, u:u + 1],
                                     lhsT=ns_my[0:1, lo:hi], rhs=ones11,
                                     start=True, stop=True)
                # broadcast ns_my across partitions: [128, R_pad]
                nsbc_ps = ns_pool.tile([128, 2, 512], F32, name="nsbc_ps")
                for c_ in range(2):
                    nc.tensor.matmul(out=nsbc_ps[:, c_, 0:CH], lhsT=ones128c,
                                     rhs=ns_bf[:, c_ * CH:(c_ + 1) * CH],
                                     start=True, stop=True)

                # rhs_u reads nsL straight from PSUM as the per-partition
                # scalar (garbage partitions are masked by ti=0, and psum
                # holds only finite floats); the trailing identity-block
                # matmul folds in the 1.0 fill for alien partitions.
                nsT_ps = ns_pool.tile([128, nt2p], F32, name="nsT_ps")
                for u in range(nU):
                    rhs_u = small.tile([128, nt2p], BF16, tag="rhsu")
                    nc.vector.scalar_tensor_tensor(
                        out=rhs_u, in0=ti_sb[:, u * nt2p:(u + 1) * nt2p],
                        scalar=nsL_ps[:, u:u + 1],
                        in1=ti_sb[:, u * nt2p:(u + 1) * nt2p],
                        op0=ALU.mult, op1=ALU.bypass)
                    nc.tensor.matmul(out=nsT_ps,
                                     lhsT=pm_sb[:, u * 128:(u + 1) * 128],
                                     rhs=rhs_u,
                                     start=(u == 0), stop=False)
                nc.tensor.matmul(out=nsT_ps,
                                 lhsT=pm_sb[:, nU * 128:(nU + 1) * 128],
                                 rhs=fill_sb, start=False, stop=True)
                # ACT bias APs must live in SBUF: one tiny copy
                nsT_sb = sb.tile([128, nt2p], F32)
                nc.vector.tensor_copy(out=nsT_sb, in_=nsT_ps)
                nc.sync.dma_start(out=out_ns, in_=ns_my)

                # ========= PHASE 2: J = ln(ns_i+ns_j) + D; hinge^2 =======
                # hinge-by-class accumulates into the (drained) bl banks
                for t in range(nt2):
                    Lt = work.tile([128, R_pad], F32, tag="L")
                    chain_sc(nc.scalar.activation(
                        out=Lt.rearrange("p (c f) -> p c f", c=2),
                        in_=nsbc_ps[:, :, 0:CH], func=AF.Ln,
                        bias=nsT_sb[:, t:t + 1], scale=1.0))
                    h2 = work.tile([128, R_pad], BF16, tag="h2")
                    acc_d = small.tile([128, 1], F32, tag="accd")
                    nc.vector._custom_dve(
                        sqrelu_add, out=h2, in0=Lt, in1=dT[:, t, :],
                        s0=0.0, accum_out=acc_d)
                    for c_ in range(2):
                        nc.tensor.matmul(
                            out=bl_ps[0:2, c_, 0:CH],
                            lhsT=oh2_sb[:, t * 2:(t + 1) * 2],
                            rhs=h2[:, c_ * CH:(c_ + 1) * CH],
                            start=(t == 0), stop=(t == nt2 - 1))

                # diagonal correction relu(ln(2 ns_i) + D_ii)^2 (masked)
                dh2 = tail.tile([1, R_pad], F32, tag="dh2")
                dummy_acc = small.tile([1, 1], F32, tag="dumacc")
                nc.vector._custom_dve(sqrelu_add, out=dh2, in0=lnterm,
                                      in1=ddiag_sb, s0=0.0,
                                      accum_out=dummy_acc)
                diag_acc = tail.tile([1, 1], F32, tag="dacc")
                dh2m = tail.tile([1, R_pad], F32, tag="dh2m")
                nc.vector.scalar_tensor_tensor(
                    out=dh2m, in0=dh2, scalar=0.0, in1=valid_sb,
                    op0=ALU.bypass, op1=ALU.mult, accum_out=diag_acc)
                nc.sync.dma_start(out=out_diag, in_=diag_acc)

                # -- combine: mask by i-side class match, accumulate --
                prod2 = tail.tile([2, 2, CH], F32, tag="prod2")
                acc2 = small.tile([2, 1], F32, tag="acc2")
                nc.vector.scalar_tensor_tensor(
                    out=prod2, in0=bl_ps[0:2, :, 0:CH], scalar=0.0,
                    in1=ohmy2_sb.rearrange("p (c f) -> p c f", c=2),
                    op0=ALU.bypass, op1=ALU.mult, accum_out=acc2)
                nc.sync.dma_start(out=out_pos, in_=acc2)

    nc.compile()
    return nc


_CACHE: dict = {}


def _get_nc(R_pad: int, nt2: int, nt2p: int):
    key = ("nc", R_pad, nt2, nt2p)
    if key not in _CACHE:
        _CACHE[key] = build_bass(R_pad, nt2, nt2p)
    return _CACHE[key]


def prepare_inputs(a: np.ndarray, b: np.ndarray, labels: np.ndarray):
    """Host-side label sort, class pairing, per-core shard + mask prep.

    Returns (per-core input maps, (R_pad, nt2, nt2p), meta)."""
    a = np.asarray(a, np.float32)
    b = np.asarray(b, np.float32)
    labels = np.asarray(labels)

    order = np.argsort(labels, kind="stable")
    a_s = a[order]
    b_s = b[order]
    sl = labels[order]
    counts = np.bincount(sl.astype(np.int64), minlength=NCLS)
    startscum = np.concatenate([[0], np.cumsum(counts)])

    def pair_tiles(p, q):
        ta = set(range(int(startscum[p]) // 128,
                       -(-int(startscum[p + 1]) // 128)))
        tb = set(range(int(startscum[q]) // 128,
                       -(-int(startscum[q + 1]) // 128)))
        return len(ta | tb)

    def pairing_cost(pairs_):
        return (max(int(counts[p] + counts[q]) for p, q in pairs_),
                max(pair_tiles(p, q) for p, q in pairs_))

    # greedy largest-with-smallest, then 2-opt swaps minimizing
    # (max pair size, max window tiles) lexicographically
    co = np.argsort(counts)
    pairs = [(int(co[i]), int(co[NCLS - 1 - i])) for i in range(NCORES)]
    best = pairing_cost(pairs)
    improved = True
    while improved:
        improved = False
        for i in range(NCORES):
            for j in range(i + 1, NCORES):
                for swap in ((0, 0), (0, 1)):
                    cand = list(pairs)
                    a1, b1 = pairs[i]
                    a2, b2 = pairs[j]
                    if swap == (0, 0):
                        cand[i], cand[j] = (a2, b1), (a1, b2)
                    else:
                        cand[i], cand[j] = (b2, b1), (a2, a1)
                    c = pairing_cost(cand)
                    if c < best:
                        pairs, best, improved = cand, c, True
    R_pad = best[0]
    R_pad = -(-R_pad // 32) * 32
    nU = -(-R_pad // 128)

    cores = []
    nt2 = 0
    for p, q in pairs:
        grows = np.concatenate([
            np.arange(startscum[p], startscum[p + 1]),
            np.arange(startscum[q], startscum[q + 1])])
        wtiles = sorted(set((grows // 128).tolist()))
        nt2 = max(nt2, len(wtiles))
        cores.append((p, q, grows, wtiles))
    nt2p = -(-nt2 // 4) * 4

    at_full = np.ascontiguousarray(a_s.T)                  # [F, N] sorted
    aa = np.sum(a_s * a_s, axis=1, dtype=np.float32)
    bb_s = np.sum(b_s * b_s, axis=1, dtype=np.float32)

    in_maps = []
    meta = []
    for c in range(NCORES):
        p, q, grows, wtiles = cores[c]
        Rc = len(grows)
        rest = [t for t in range(NT) if t not in wtiles]
        sigma = np.array(list(wtiles) + rest)
        slot_of = {t: s_ for s_, t in enumerate(sigma)}

        cols = (sigma[:, None] * 128 + np.arange(128)[None, :]).reshape(-1)
        # [128 kpart, 4 ksub, N] fp8 for DoubleRow lhsT slices
        at_c = np.ascontiguousarray(
            at_full[:, cols].reshape(4, 128, N).transpose(1, 0, 2)
        ).astype(NPFP8)
        aat_c = np.ascontiguousarray(aa[cols].reshape(NT, 128).T)  # [128, NT]

        glbl = sl[cols].reshape(NT, 128)                   # labels per slot
        oh3_c = np.zeros((NT, 128, 3), np.float32)
        oh3_c[:, :, 0] = 1.0
        oh3_c[:, :, 1] = glbl == p
        oh3_c[:, :, 2] = glbl == q
        oh3_c = np.ascontiguousarray(
            oh3_c.transpose(1, 0, 2).reshape(128, NT * 3)).astype(NPBF16)
        oh2_c = np.zeros((nt2, 128, 2), np.float32)
        oh2_c[:, :, 0] = glbl[:nt2] == p
        oh2_c[:, :, 1] = glbl[:nt2] == q
        oh2_c = np.ascontiguousarray(
            oh2_c.transpose(1, 0, 2).reshape(128, nt2 * 2)).astype(NPBF16)

        b_loc = np.zeros((R_pad, F), np.float32)
        b_loc[:Rc] = b_s[grows]
        a_my = np.zeros((R_pad, F), np.float32)
        a_my[:Rc] = a_s[grows]
        bb_loc = np.zeros(R_pad, np.float32)
        bb_loc[:Rc] = bb_s[grows]
        bt2_c = np.ascontiguousarray(
            (-2.0 * b_loc).T.reshape(4, 128, R_pad).transpose(1, 0, 2)
        ).astype(NPFP8)
        bbv_c = bb_loc.reshape(1, R_pad).copy()

        lbl_loc = np.full(R_pad, -1, np.int64)
        lbl_loc[:Rc] = sl[grows]
        selP = (lbl_loc == p).astype(np.float32)
        selQ = (lbl_loc == q).astype(np.float32)
        sel3_c = np.ascontiguousarray(
            np.stack([np.ones(R_pad, np.float32), selP, selQ], 0))
        ohmy2_c = np.ascontiguousarray(np.stack([selP, selQ], 0))

        ti_c = np.zeros((nU, 128, nt2p), np.float32)
        pm_c = np.zeros((nU + 1, 128, 128), np.float32)
        pm_c[nU] = np.eye(128, dtype=np.float32)
        used = np.zeros((128, nt2p), bool)
        for r in range(Rc):
            gr = grows[r]
            u, cc = r // 128, r % 128
            t_ = slot_of[gr // 128]
            ti_c[u, cc, t_] = 1.0
            pm_c[u, cc, gr % 128] = 1.0
            used[gr % 128, t_] = True
        ti_c = np.ascontiguousarray(
            ti_c.transpose(1, 0, 2).reshape(128, nU * nt2p)).astype(NPBF16)
        pm_c = np.ascontiguousarray(
            pm_c.transpose(1, 0, 2).reshape(128, (nU + 1) * 128)).astype(NPBF16)
        fill_c = np.where(used, 0.0, 1.0).astype(NPBF16)

        valid_c = (np.arange(R_pad) < Rc).astype(np.float32).reshape(1, R_pad)
        dd = np.sum(np.square(b_loc - a_my), axis=1, dtype=np.float32)
        ddiag_c = np.sqrt(np.maximum(dd, 0.0)).reshape(1, R_pad)

        in_maps.append({
            "at": at_c, "bt2": bt2_c, "aat": aat_c, "bbv": bbv_c,
            "oh3": oh3_c, "oh2": oh2_c, "sel3": sel3_c, "ohmy2": ohmy2_c,
            "ti": ti_c, "pm": pm_c, "fillm": np.ascontiguousarray(fill_c),
            "validm": valid_c, "ddiag": ddiag_c,
        })
        meta.append({"grows": grows, "Rc": Rc})
    return in_maps, (R_pad, nt2, nt2p), {"order": order, "cores": meta}


def run(a, b, labels, trace=False, trace_kwargs=None):
    """Run on 8 NeuronCores; returns (loss, BassKernelResults, meta)."""
    in_maps, dims, meta = prepare_inputs(a, b, labels)
    nc = _get_nc(*dims)
    kw = {}
    if trace:
        kw = dict(trace=True, **(trace_kwargs or {}))
    res = run_bass_kernel_spmd(nc, in_maps, core_ids=list(range(NCORES)), **kw)

    counts = np.bincount(np.asarray(labels).astype(np.int64), minlength=NCLS)
    num_pos = float((counts.astype(np.float64) ** 2).sum() - N)

    total = 0.0
    for c in range(NCORES):
        r = res.results[c]
        total += (float(r["out_pos"][0, 0]) + float(r["out_pos"][1, 0])
                  - float(r["out_diag"][0, 0]))
    loss = total / (2.0 * num_pos)
    return np.asarray(np.float32(loss)), res, meta


def kernel(a, b, labels):
    loss, _, _ = run(a, b, labels)
    return loss
